# revision 1
# baseline (speedup 1.0000x reference)
"""Trainium2 Bass kernel for nn_Attention_spd (dense transformer attention with
pairwise score bias `spd`, head-drop rescale, and output projection).

Reference computation (b=4, n=1024, dim=512, heads=8, dim_head=64):
    qkv = x @ w_qkv ; q,k,v = split
    dots = q @ k^T * scale + spd
    attn = softmax(dots) * (head_keep * H / sum(head_keep))
    out  = (attn @ v) @ w_out + b_out

Sharding across 8 NeuronCores: core c handles batch c//2 and heads
4*(c%2) .. 4*(c%2)+3 (data parallel on batch x tensor parallel on heads).
Each core computes a partial output projection over its 4 heads; the host
sums the two partials per batch (cheap 2-way reduce) and adds b_out.

Device-side choices:
  - q/k/v/proj matmuls in fp32r (fp32 rounded to 11 mantissa bits, full PE
    speed, ~1e-4 relative error); attention probabilities in bf16.
  - Attention computed transposed: dotsT[j,i] = k @ q^T so the exp'd scores
    are directly the [K=j, N=i] moving operand of attn@v.
  - v augmented with a ones column (M=65): the attn@v matmul also emits the
    softmax denominator (row 64 of the PSUM output).
  - softmax skips max-subtraction (logits ~N(0,2); exp safe in fp32) —
    mathematically identical.
  - exp(dots + spd) = exp(dots) * exp(spd); exp(spd) is precomputed on the
    host in bf16 (halves the dominant DMA stream), and the combine is a bf16
    DVE multiply (2x mode) instead of an f32 add.
  - Head-PAIR batching through 2-bank (128x1024) PSUM tiles: the two heads'
    dots share one PSUM tile so exp / multiply / normalize run as single
    wide ops (ACT is the pacing engine; this halves its per-op overhead).
  - scale folded into wq on host; head_keep rescale folded into w_out rows;
    softmax normalization broadcast via a K=1 fp32r matmul (gpsimd
    partition_broadcast mis-handles base!=0 APs on HW).
  - DMA instruction count minimized (each dma_start costs ~650ns of
    sequencer + shared-HWDGE time), ordered so the first attention phase's
    dependencies land first.
"""
import os
import sys

for _p in ("/opt/trn_rl_repo", os.path.expanduser("~/.axon_site/_ro/trn_rl_repo")):
    if os.path.isdir(_p) and _p not in sys.path:
        sys.path.insert(0, _p)

import numpy as np
import ml_dtypes

import concourse.bass as bass  # noqa: F401
import concourse.tile as tile
from concourse import bacc, mybir
from concourse.bass_utils import run_bass_kernel_spmd

P = 128
B, N, DIM = 4, 1024, 512
HEADS = 8
DIM_HEAD = 64
SCALE = DIM_HEAD ** -0.5
HL = 4          # heads per core (local)
F32 = mybir.dt.float32
F32R = mybir.dt.float32r
BF16 = mybir.dt.bfloat16
ADD = mybir.AluOpType.add
MULT = mybir.AluOpType.mult
EXP = mybir.ActivationFunctionType.Exp

VARIANT = "bf16mul"

_NC = {}


def build_nc(variant=VARIANT):
    """Build the SPMD Bass program (identical on all 8 cores)."""
    nc = bacc.Bacc("TRN2", target_bir_lowering=False, debug=False, num_devices=8)
    xT = nc.dram_tensor("xT", [DIM, N], F32R, kind="ExternalInput").ap()
    # [qm0 | km0 | v | qm1 | km1] so a small early DMA unblocks the first phase
    w3 = nc.dram_tensor("w3", [DIM, 3 * HL * DIM_HEAD], F32R, kind="ExternalInput").ap()
    wo = nc.dram_tensor("wo", [DIM_HEAD, HL, DIM], F32R, kind="ExternalInput").ap()
    # exp(spd) in bf16: [hp, ib, jj, jb, s, ii] — per (hp, ib) contiguous,
    # with the head pair's (s) tiles adjacent so one DVE op covers both
    spdT = nc.dram_tensor("spdT", [2, 2, P, 8, 2, 512], BF16, kind="ExternalInput").ap()
    y = nc.dram_tensor("y", [N, DIM], F32, kind="ExternalOutput").ap()

    from contextlib import ExitStack

    with tile.TileContext(nc) as tc, ExitStack() as ctx:
        const = ctx.enter_context(tc.tile_pool(name="const", bufs=1))
        sb = ctx.enter_context(tc.tile_pool(name="sb", bufs=1))
        spd_pool = ctx.enter_context(tc.tile_pool(name="spd", bufs=3))
        ex_pool = ctx.enter_context(tc.tile_pool(name="ex", bufs=3))
        pr_pool = ctx.enter_context(tc.tile_pool(name="pr", bufs=3))
        nrm_pool = ctx.enter_context(tc.tile_pool(name="nrm", bufs=2))
        ps = ctx.enter_context(tc.tile_pool(name="ps", bufs=2, space="PSUM"))
        ps4 = ctx.enter_context(tc.tile_pool(name="ps4", bufs=4, space="PSUM"))

        # ---- resident loads -------------------------------------------------
        xT_sb = sb.tile([P, 4, N], F32R)
        w3_sb = sb.tile([P, 4, 768], F32R, tag="w3")
        xT_r = xT.rearrange("(kb p) n -> p kb n", p=P)
        w3_r = w3.rearrange("(kb p) m -> p kb m", p=P)
        nc.sync.dma_start(xT_sb[:], xT_r[:])
        nc.sync.dma_start(w3_sb[:, :, 0:256], w3_r[:, :, 0:256])      # q/k m0
        nc.sync.dma_start(w3_sb[:, :, 256:512], w3_r[:, :, 256:512])  # v
        wo_sb = sb.tile([DIM_HEAD, HL, DIM], F32R, tag="wo")

        ones32 = const.tile([P, 1], F32)
        nc.vector.memset(ones32[:], 1.0)
        # ones row at partition 64: lhsT of the K=1 rowsum-reciprocal
        # broadcast matmul (both operands at partition 64 — HW-exact)
        ones65f = const.tile([65, DIM_HEAD], F32, tag="ones65f")
        nc.vector.memset(ones65f[:], 1.0)
        ones65 = const.tile([65, DIM_HEAD], F32R, tag="ones65")
        nc.vector.tensor_copy(ones65[:], ones65f[:])
        wrowf = const.tile([65, 512], F32, tag="wrowf")
        nc.vector.memset(wrowf[:], 1.0)
        wrow = const.tile([65, 512], F32R, tag="wrow")
        nc.vector.tensor_copy(wrow[64:65, :], wrowf[64:65, :])

        # PE warm-up during the initial DMA wait: the PE clock-gate (HAM)
        # starts throttled; ~3.5us of dummy matmuls bring it to full rate
        # before the qkv projections arrive
        warm = ps.tile([P, 1024], F32, tag="big", name="warm")
        for w in range(16):
            nc.tensor.matmul(warm[0:64, 0:512], ones65[64:65, :], wrow[64:65, :],
                             start=True, stop=True)

        # ---- qkv projections ------------------------------------------------
        qT_sb = sb.tile([P, 2, N], F32R, tag="qT")
        kT_sb = sb.tile([P, 2, N], F32R, tag="kT")
        v_aug = sb.tile([P, 8, HL * 65], BF16, tag="vaug")
        v_cols = v_aug[:].rearrange("p jb (h c) -> p jb h c", c=65)
        nc.vector.tensor_copy(
            v_cols[:, :, :, 64:65],
            ones32[:, None, :, None].to_broadcast((P, 8, HL, 1)),
        )

        def qk_proj(qk, dst, m):
            wofs = (512 if m else 0) + qk * 128
            pq = ps.tile([P, 1024], F32, tag="big", name=f"pq_{qk}_{m}")
            for nb in range(2):
                for kb in range(4):
                    nc.tensor.matmul(
                        pq[:, nb * 512:(nb + 1) * 512],
                        w3_sb[:, kb, wofs:wofs + 128],
                        xT_sb[:, kb, nb * 512:(nb + 1) * 512],
                        start=(kb == 0),
                        stop=(kb == 3),
                    )
            nc.scalar.copy(dst[:, m, :], pq[:])

        qk_proj(0, qT_sb, 0)
        qk_proj(1, kT_sb, 0)
        # v: narrow tiles on the 4-slot ring (keeps the wide ring free for
        # the first attention phase's dots)
        for jb in range(8):
            pv = ps4.tile([P, 512], F32, tag="po", name=f"pv_{jb}")
            for kb in range(4):
                nc.tensor.matmul(
                    pv[:, :256],
                    xT_sb[:, kb, jb * 128:(jb + 1) * 128],
                    w3_sb[:, kb, 256:512],
                    start=(kb == 0),
                    stop=(kb == 3),
                )
            nc.vector.tensor_copy(
                v_cols[:, jb, :, :64],
                pv[:, :256].rearrange("p (h c) -> p h c", c=64),
            )

        # first attention phase's spd transfers go on the DMA queue ahead of
        # the late weight loads and the m1 q/k projections
        st00 = spd_pool.tile([P, 8, 2, 512], BF16, tag="spd", name="spd_0_0")
        nc.sync.dma_start(st00[:, 0:4], spdT[0, 0, :, 0:4])
        nc.sync.dma_start(st00[:, 4:8], spdT[0, 0, :, 4:8])
        nc.sync.dma_start(w3_sb[:, :, 512:768], w3_r[:, :, 512:768])  # q/k m1
        nc.sync.dma_start(wo_sb[:], wo[:])

        # ---- attention ------------------------------------------------------
        # scaled attention output, transposed: [d, h, i] (d on partitions)
        scaled = sb.tile([DIM_HEAD, HL, N], F32R, tag="scaled")
        y_all = sb.tile([P, 8, 512], F32, tag="yall")

        def do_norm(po, hp, ib):
            # head-pair normalization: 1/rowsums -> broadcast via K=1
            # matmuls -> rescale into `scaled`
            rc = nrm_pool.tile([65, 1024], F32R, tag="rc", name=f"rc_{hp}_{ib}")
            with nc.allow_low_precision(reason="f32r recip is plenty for softmax denom"):
                for s in range(2):
                    nc.vector.reciprocal(rc[64:65, s * 512:(s + 1) * 512],
                                         po[s][64:65, :])
            pb = ps.tile([P, 1024], F32, tag="big", name=f"pb_{hp}_{ib}")
            for s in range(2):
                nc.tensor.matmul(pb[0:64, s * 512:(s + 1) * 512],
                                 ones65[64:65, :], rc[64:65, s * 512:(s + 1) * 512],
                                 start=True, stop=True)
            bc = nrm_pool.tile([64, 1024], F32, tag="bc", name=f"bc_{hp}_{ib}")
            nc.vector.tensor_copy(bc[:], pb[0:64, :])
            for s in range(2):
                nc.vector.tensor_tensor(
                    scaled[:, 2 * hp + s, ib * 512:(ib + 1) * 512],
                    po[s][0:64, :],
                    bc[:, s * 512:(s + 1) * 512],
                    MULT,
                )

        def proj(iop):
            # narrow tiles from the 4-slot ring (the wide ring keeps feeding
            # the dots/exp stream)
            for half in range(2):
                io = 2 * iop + half
                py = ps4.tile([P, 512], F32, tag="po", name=f"py_{io}")
                for h in range(HL):
                    nc.tensor.matmul(
                        py[:],
                        scaled[:, h, io * 128:(io + 1) * 128],
                        wo_sb[:, h, :],
                        start=(h == 0),
                        stop=(h == HL - 1),
                    )
                nc.vector.tensor_copy(y_all[:, io, :], py[:])
            # gpsimd/SWDGE queue: an output DMA waiting on its copy must not
            # block the spd stream on the SP HWDGE queue
            nc.gpsimd.dma_start(
                y[iop * 256:(iop + 1) * 256, :].rearrange("(half p) q -> p half q", p=P),
                y_all[:, 2 * iop:2 * iop + 2, :])

        prev = None
        for ib in range(2):          # i block of 512 (outer: frees proj early)
            for hp in range(2):      # head pair (local heads 2hp, 2hp+1)
                def m1_chunk(qk, dst):
                    wofs = 512 + qk * 128
                    for nb in range(2):
                        pq1 = ps4.tile([P, 512], F32, tag="po",
                                       name=f"pq1_{qk}_{nb}")
                        for kb in range(4):
                            nc.tensor.matmul(
                                pq1[:],
                                w3_sb[:, kb, wofs:wofs + 128],
                                xT_sb[:, kb, nb * 512:(nb + 1) * 512],
                                start=(kb == 0),
                                stop=(kb == 3),
                            )
                        nc.vector.tensor_copy(
                            dst[:, 1, nb * 512:(nb + 1) * 512], pq1[:])

                if ib == 0 and hp == 0:
                    st = st00
                else:
                    st = spd_pool.tile([P, 8, 2, 512], BF16, tag="spd",
                                       name=f"spd_{hp}_{ib}")
                    nc.sync.dma_start(st[:, 0:4], spdT[hp, ib, :, 0:4])
                    nc.sync.dma_start(st[:, 4:8], spdT[hp, ib, :, 4:8])
                po = [ps4.tile([128, 512], F32, tag="po", name=f"po_{hp}_{ib}_{s}")
                      for s in range(2)]
                for jb in range(8):
                    pd = ps.tile([P, 1024], F32, tag="big", name=f"pd_{hp}_{ib}_{jb}")
                    # the pair's dots back-to-back: disjoint K=64 row groups
                    # can overlap in the PE array
                    for s in range(2):
                        nc.tensor.matmul(
                            pd[:, s * 512:(s + 1) * 512],
                            kT_sb[64 * s:64 * s + 64, hp, jb * 128:(jb + 1) * 128],
                            qT_sb[64 * s:64 * s + 64, hp, ib * 512:(ib + 1) * 512],
                            start=True,
                            stop=True,
                        )
                    # one wide exp + one wide bf16 multiply for both heads
                    ex = ex_pool.tile([P, 1024], BF16, tag="ex", name=f"ex_{hp}_{ib}_{jb}")
                    nc.scalar.activation(ex[:], pd[:], EXP)
                    pr = pr_pool.tile([P, 1024], BF16, tag="pr", name=f"pr_{hp}_{ib}_{jb}")
                    nc.vector.tensor_tensor(
                        pr[:], ex[:],
                        st[:, jb].rearrange("p s i -> p (s i)"),
                        MULT,
                    )
                    for s in range(2):
                        h = 2 * hp + s
                        nc.tensor.matmul(
                            po[s][0:65, :],
                            v_aug[:, jb, h * 65:(h + 1) * 65],
                            pr[:, s * 512:(s + 1) * 512],
                            start=(jb == 0),
                            stop=(jb == 7),
                        )
                    # interleave previous-phase epilogue work into this
                    # phase's mid-stream PE slack instead of its boundary
                    if prev is not None:
                        if jb == 2:
                            do_norm(*prev)
                        if prev[1] == 1:      # prev phase completed its ib
                            if jb == 4:
                                proj(prev[2] * 2)
                            if jb == 6:
                                proj(prev[2] * 2 + 1)
                    if ib == 0 and hp == 0:
                        if jb == 4:
                            m1_chunk(0, qT_sb)
                        if jb == 6:
                            m1_chunk(1, kT_sb)
                prev = (po, hp, ib)

        # flush: last phase's normalization + remaining projections
        p_po, p_hp, p_ib = prev
        do_norm(p_po, p_hp, p_ib)
        proj(2)
        proj(3)

    nc.compile()
    return nc


def _get_nc(variant=VARIANT):
    if variant not in _NC:
        _NC[variant] = build_nc(variant)
    return _NC[variant]


def make_in_maps(x, spd, head_keep, w_qkv, w_out, variant=VARIANT):
    x = np.asarray(x, np.float32)
    spd = np.asarray(spd, np.float32)
    keep = np.asarray(head_keep, np.float32)
    w_qkv = np.asarray(w_qkv, np.float32)
    w_out = np.asarray(w_out, np.float32)
    cfac = keep * (HEADS / keep.sum())

    in_maps = []
    for c in range(8):
        bi, hh = divmod(c, 2)
        h0 = hh * HL
        hs = slice(h0 * DIM_HEAD, (h0 + HL) * DIM_HEAD)
        xT = np.ascontiguousarray(x[bi].T)
        q_cols = w_qkv[:, hs] * np.float32(SCALE)
        k_cols = w_qkv[:, DIM + h0 * DIM_HEAD:DIM + (h0 + HL) * DIM_HEAD]
        v_cols_h = w_qkv[:, 2 * DIM + h0 * DIM_HEAD:2 * DIM + (h0 + HL) * DIM_HEAD]
        w3 = np.ascontiguousarray(np.concatenate(
            [q_cols[:, :128], k_cols[:, :128], v_cols_h,
             q_cols[:, 128:], k_cols[:, 128:]],
            axis=1,
        ))
        wo_rows = w_out[hs, :] * np.repeat(cfac[h0:h0 + HL], DIM_HEAD)[:, None]
        wo = np.ascontiguousarray(wo_rows.reshape(HL, DIM_HEAD, DIM).transpose(1, 0, 2))
        sp = spd[bi, h0:h0 + HL]  # [HL, i, j] with h = 2*hp + s
        # [hp, s, ib, ii, jb, jj] -> [hp, ib, jj, jb, s, ii]
        spdT = sp.reshape(2, 2, 2, 512, 8, 128).transpose(0, 2, 5, 4, 1, 3)
        spdT = np.exp(spdT).astype(ml_dtypes.bfloat16)
        in_maps.append({"xT": xT, "w3": w3, "wo": wo, "spdT": np.ascontiguousarray(spdT)})
    return in_maps


def kernel(x, spd, head_keep, w_qkv, w_out, b_out):
    assert x.shape == (B, N, DIM) and spd.shape == (B, HEADS, N, N)
    nc = _get_nc()
    in_maps = make_in_maps(x, spd, head_keep, w_qkv, w_out)
    res = run_bass_kernel_spmd(nc, in_maps, core_ids=list(range(8)))
    out = np.empty((B, N, DIM), np.float32)
    for bi in range(B):
        out[bi] = res.results[2 * bi]["y"] + res.results[2 * bi + 1]["y"]
    out += np.asarray(b_out, np.float32)[None, None, :]
    return out



# revision 2
# speedup vs baseline: 1.0119x; 1.0119x over previous
"""Trainium2 Bass kernel v2 for nn_Attention_spd.

Reference computation (b=4, n=1024, dim=512, heads=8, dim_head=64):
    qkv = x @ w_qkv ; q,k,v = split
    dots = q @ k^T * scale + spd
    attn = softmax(dots) * (head_keep * H / sum(head_keep))
    out  = (attn @ v) @ w_out + b_out

Sharding: core c handles batch c//2, local heads 4*(c%2)..+3 (DP x TP).
Host sums the two bf16 partial outputs per batch and adds b_out.

Design notes:
  - All DRAM traffic bf16 (x, w_qkv, w_out, exp(spd), y partials): ~10.9MB/core.
  - attn@v computed TRANSPOSED with v as the *moving* operand:
    out[i, d] tiles of [128 i, 65] cost only 65 PE columns each (v augmented
    with a ones column so col 64 accumulates the softmax denominator).
    The 8 accumulation groups (2 heads x 4 i-blocks) share two PSUM banks via
    the lazy bank-zero semantics: only the first group issues start=True
    (wiping the whole bank); the other 3 start with start=False and are
    zero-seeded by the pending-zero region.
  - Softmax normalization is a per-partition scalar op (reciprocal of col 64
    + tensor_scalar_mul -> bf16); no broadcast matmul.
  - Normalized [128 i, 64 d] tiles are PE-transposed (identity matmul) into
    [(s,d), i] layout packing the head pair on 128 partitions, so the output
    projection runs with K=128 (half the matmuls of the K=64 version).
  - ACT (exp) is the pacing engine: 32 x [128,1024] exp ops. The jb loop is
    software-pipelined: dots(jb+1) is emitted BEFORE attnv(jb) so the
    in-order PE queue never lockstep-stalls the ACT stream.
  - PSUM: wide pd ring (2x2 banks) + po2 ring (2x1) + two 1-bank scratch
    rings (tp/tq) for v/qk-chunk/transpose/proj tiles, used alternately.
  - q/k m0 computed for i-cols 0:512 first (minimal head before the first
    dots); remaining q/k chunks + v projection interleave into phase slack.
"""
import os
import sys

for _p in ("/opt/trn_rl_repo", os.path.expanduser("~/.axon_site/_ro/trn_rl_repo")):
    if os.path.isdir(_p) and _p not in sys.path:
        sys.path.insert(0, _p)

import numpy as np
import ml_dtypes

import concourse.bass as bass  # noqa: F401
import concourse.tile as tile
from concourse import bacc, mybir
from concourse.bass_utils import run_bass_kernel_spmd

P = 128
B, N, DIM = 4, 1024, 512
HEADS = 8
DIM_HEAD = 64
SCALE = DIM_HEAD ** -0.5
HL = 4          # heads per core (local)
F32 = mybir.dt.float32
BF16 = mybir.dt.bfloat16
MULT = mybir.AluOpType.mult
EXP = mybir.ActivationFunctionType.Exp

_NC = {}


def build_nc():
    nc = bacc.Bacc("TRN2", target_bir_lowering=False, debug=False, num_devices=8)
    xT = nc.dram_tensor("xT", [DIM, N], BF16, kind="ExternalInput").ap()
    # [qm0 | km0 | v | qm1 | km1] column blocks (q pre-scaled by SCALE)
    w3 = nc.dram_tensor("w3", [DIM, 3 * HL * DIM_HEAD], BF16, kind="ExternalInput").ap()
    # packed for K=128 proj: [(s,d), hp, dim]
    wo2 = nc.dram_tensor("wo2", [P, 2, DIM], BF16, kind="ExternalInput").ap()
    ident = nc.dram_tensor("ident", [P, P], BF16, kind="ExternalInput").ap()
    # exp(spd) bf16: [hp, ib, j, jb, s, i]
    spdT = nc.dram_tensor("spdT", [2, 2, P, 8, 2, 512], BF16, kind="ExternalInput").ap()
    y = nc.dram_tensor("y", [N, DIM], BF16, kind="ExternalOutput").ap()

    from contextlib import ExitStack

    with tile.TileContext(nc) as tc, ExitStack() as ctx:
        sb = ctx.enter_context(tc.tile_pool(name="sb", bufs=1))
        spd_pool = ctx.enter_context(tc.tile_pool(name="spd", bufs=4))
        ex_pool = ctx.enter_context(tc.tile_pool(name="ex", bufs=4))
        pr_pool = ctx.enter_context(tc.tile_pool(name="pr", bufs=4))
        attn_pool = ctx.enter_context(tc.tile_pool(name="attn", bufs=2))
        rc_pool = ctx.enter_context(tc.tile_pool(name="rc", bufs=2))
        sc2_pool = ctx.enter_context(tc.tile_pool(name="sc2", bufs=2))
        # PSUM: 4 (wide pd ring) + 2 (po2) + 1 (tp) + 1 (tq) = 8 banks
        wide = ctx.enter_context(tc.tile_pool(name="wide", bufs=2, space="PSUM"))
        npo = ctx.enter_context(tc.tile_pool(name="npo", bufs=2, space="PSUM"))
        tp = ctx.enter_context(tc.tile_pool(name="tp", bufs=1, space="PSUM"))
        tq = ctx.enter_context(tc.tile_pool(name="tq", bufs=1, space="PSUM"))

        def scratch(i):
            return tp if i % 2 == 0 else tq

        # ---- consts + warm-up ----------------------------------------------
        seed = sb.tile([P, 512], BF16, tag="seed")
        nc.gpsimd.memset(seed[:], 1.0)
        # PE p-state ramp: busy early so real matmuls hit full speed; the
        # warm matmuls also bridge the initial DMA wait
        warm = tp.tile([P, 512], F32, tag="tp", name="warm")
        for _ in range(5):
            nc.tensor.matmul(warm[:, :512], seed[0:1, 0:128], seed[0:1, 0:512],
                             start=True, stop=True)

        # ---- resident loads -------------------------------------------------
        xT_sb = sb.tile([P, 4, N], BF16)
        w3_sb = sb.tile([P, 4, 768], BF16, tag="w3")
        wo2_sb = sb.tile([P, 2, DIM], BF16, tag="wo2")
        ident_sb = sb.tile([P, P], BF16, tag="ident")
        xT_r = xT.rearrange("(kb p) n -> p kb n", p=P)
        w3_r = w3.rearrange("(kb p) m -> p kb m", p=P)
        # ordered so the first q/k projections + first spd tile land earliest
        nc.sync.dma_start(w3_sb[:, :, 0:256], w3_r[:, :, 0:256])      # q/k m0
        nc.sync.dma_start(xT_sb[:, 0:2, 0:512], xT_r[:, 0:2, 0:512])
        nc.sync.dma_start(xT_sb[:, 2:4, 0:512], xT_r[:, 2:4, 0:512])
        nc.sync.dma_start(w3_sb[:, :, 256:512], w3_r[:, :, 256:512])  # v
        st00 = spd_pool.tile([P, 8, 2, 512], BF16, tag="spd", name="spd_0_0")
        nc.sync.dma_start(st00[:, 0:4], spdT[0, 0, :, 0:4])
        nc.sync.dma_start(xT_sb[:, 0:2, 512:1024], xT_r[:, 0:2, 512:1024])
        nc.sync.dma_start(xT_sb[:, 2:4, 512:1024], xT_r[:, 2:4, 512:1024])
        nc.sync.dma_start(st00[:, 4:8], spdT[0, 0, :, 4:8])
        nc.sync.dma_start(w3_sb[:, :, 512:768], w3_r[:, :, 512:768])  # q/k m1
        nc.sync.dma_start(wo2_sb[:], wo2[:])
        nc.sync.dma_start(ident_sb[:], ident[:])

        qT_sb = sb.tile([P, 2, N], BF16, tag="qT")
        kT_sb = sb.tile([P, 2, N], BF16, tag="kT")
        v_aug = sb.tile([P, 8, HL * 65], BF16, tag="vaug")
        v_cols = v_aug[:].rearrange("p jb (h c) -> p jb h c", c=65)
        nc.vector.memset(v_cols[:, :, :, 64:65], 1.0)
        y_all = sb.tile([P, 8, DIM], BF16, tag="yall")

        # ---- minimal head: q/k m0 for i-cols 0:512 only ---------------------
        # qk: 0=q, 1=k; m: 0=heads 0/1, 1=heads 2/3; nb: i-col half
        def qk_chunk(qk, m, nb, copy_engine, pool):
            wofs = (512 if m else 0) + qk * 128
            dst = qT_sb if qk == 0 else kT_sb
            pq = pool.tile([P, 512], F32, tag=pool.name,
                           name=f"pq_{qk}_{m}_{nb}")
            for kb in range(4):
                nc.tensor.matmul(
                    pq[:],
                    w3_sb[:, kb, wofs:wofs + 128],
                    xT_sb[:, kb, nb * 512:(nb + 1) * 512],
                    start=(kb == 0),
                    stop=(kb == 3),
                )
            if copy_engine == "pool":
                nc.gpsimd.tensor_copy(dst[:, m, nb * 512:(nb + 1) * 512], pq[:])
            else:
                nc.vector.tensor_copy(dst[:, m, nb * 512:(nb + 1) * 512], pq[:])

        # q/k m0 nb0 with split copies so the first dots' last dependency
        # lands as early as possible
        pqq = tp.tile([P, 512], F32, tag="tp", name="pq_0_0_0")
        for kb in range(4):
            nc.tensor.matmul(
                pqq[:],
                w3_sb[:, kb, 0:128],
                xT_sb[:, kb, 0:512],
                start=(kb == 0),
                stop=(kb == 3),
            )
        nc.vector.tensor_copy(qT_sb[:, 0, 0:256], pqq[:, 0:256])
        nc.vector.tensor_copy(qT_sb[:, 0, 256:512], pqq[:, 256:512])
        pqk = tq.tile([P, 512], F32, tag="tq", name="pq_1_0_0")
        for kb in range(4):
            nc.tensor.matmul(
                pqk[:],
                w3_sb[:, kb, 128:256],
                xT_sb[:, kb, 0:512],
                start=(kb == 0),
                stop=(kb == 3),
            )
        nc.vector.tensor_copy(kT_sb[:, 0, 0:256], pqk[:, 0:256])
        nc.scalar.copy(kT_sb[:, 0, 256:512], pqk[:, 256:512])

        def v_proj(jb):
            pv = scratch(jb).tile([P, 512], F32, tag=scratch(jb).name,
                                  name=f"pv_{jb}")
            for kb in range(4):
                nc.tensor.matmul(
                    pv[:, :256],
                    xT_sb[:, kb, jb * 128:(jb + 1) * 128],
                    w3_sb[:, kb, 256:512],
                    start=(kb == 0),
                    stop=(kb == 3),
                )
            nc.vector.tensor_copy(
                v_cols[:, jb, :, :64],
                pv[:, :256].rearrange("p (h c) -> p h c", c=64),
            )

        # ---- attention ------------------------------------------------------
        sc2 = {}

        deferred = []

        def proj(sc, io, dst_io, ycopy_engine, pool):
            py = pool.tile([P, 512], F32, tag=pool.name, name=f"py_{dst_io}")
            for u in range(2):
                nc.tensor.matmul(
                    py[:],
                    sc[:, u, io * 128:(io + 1) * 128],
                    wo2_sb[:, u, :],
                    start=(u == 0),
                    stop=(u == 1),
                )
            if ycopy_engine == "act":
                nc.scalar.copy(y_all[:, dst_io, :], py[:])
            elif ycopy_engine == "dve":
                nc.vector.tensor_copy(y_all[:, dst_io, :], py[:])
            return py

        prev = None
        for hp in range(2):
            for ib in range(2):
                if ib == 0 and hp == 0:
                    st = st00
                else:
                    st = spd_pool.tile([P, 8, 2, 512], BF16, tag="spd",
                                       name=f"spd_{hp}_{ib}")
                    nc.sync.dma_start(st[:, 0:4], spdT[hp, ib, :, 0:4])
                    nc.sync.dma_start(st[:, 4:8], spdT[hp, ib, :, 4:8])
                if ib not in sc2:
                    sc2[ib] = sc2_pool.tile([P, 2, 512], BF16, tag="sc2",
                                            name=f"sc2_{ib}")

                # ---- prev phase normalization FIRST (frees its po2 slots
                # before this phase's attnv reuses the 2-slot ring) ----
                if prev is not None:
                    p_po2, p_hp, p_ib = prev
                    p_sc = sc2[p_ib]
                    rc = rc_pool.tile([P, 2, 4], F32, tag="rc",
                                      name=f"rc_{p_hp}_{p_ib}")
                    att_n = attn_pool.tile([P, 2, 256], BF16, tag="attn",
                                           name=f"attn_{p_hp}_{p_ib}")
                    for s in range(2):
                        nc.vector.reciprocal(
                            rc[:, s, :],
                            p_po2[s][:].rearrange("p (ic c) -> p ic c", c=128)[:, :, 64],
                        )
                    # normalization on DVE (GPSIMD cannot access PSUM on HW)
                    for s in range(2):
                        for ic in range(4):
                            nc.vector.tensor_scalar_mul(
                                att_n[:, s, ic * 64:(ic + 1) * 64],
                                p_po2[s][:, ic * 128:ic * 128 + 64],
                                rc[:, s, ic:ic + 1],
                            )

                po2 = [npo.tile([P, 512], F32, tag="po", name=f"po2_{hp}_{ib}_{s}")
                       for s in range(2)]

                def dots(jb):
                    pd = wide.tile([P, 1024], F32, tag="big",
                                   name=f"pd_{hp}_{ib}_{jb}")
                    for s in range(2):
                        nc.tensor.matmul(
                            pd[:, s * 512:(s + 1) * 512],
                            kT_sb[64 * s:64 * s + 64, hp, jb * 128:(jb + 1) * 128],
                            qT_sb[64 * s:64 * s + 64, hp, ib * 512:(ib + 1) * 512],
                            start=True,
                            stop=True,
                        )
                    return pd

                if hp == 0 and ib == 0:
                    # first dots split into i-halves: each sub-matmul starts
                    # as soon as its half of the q copy lands
                    pd = wide.tile([P, 1024], F32, tag="big", name="pd_0_0_0")
                    for s in range(2):
                        for ihalf in range(2):
                            nc.tensor.matmul(
                                pd[:, s * 512 + ihalf * 256:s * 512 + (ihalf + 1) * 256],
                                kT_sb[64 * s:64 * s + 64, 0, 0:128],
                                qT_sb[64 * s:64 * s + 64, 0, ihalf * 256:(ihalf + 1) * 256],
                                start=True,
                                stop=True,
                            )
                else:
                    pd = dots(0)
                for jb in range(8):
                    ex = ex_pool.tile([P, 1024], BF16, tag="ex",
                                      name=f"ex_{hp}_{ib}_{jb}")
                    pr = pr_pool.tile([P, 1024], BF16, tag="pr",
                                      name=f"pr_{hp}_{ib}_{jb}")
                    nc.scalar.activation(ex[:], pd[:], EXP)
                    nc.vector.tensor_tensor(
                        pr[:], ex[:],
                        st[:, jb].rearrange("p s i -> p (s i)"),
                        MULT,
                    )
                    # software pipelining: next dots queued on PE BEFORE this
                    # unit's attnv (which waits on ACT+DVE)
                    if jb < 7:
                        pd = dots(jb + 1)
                    # v projection feeds attnv of phase (0,0) just in time
                    if ib == 0 and hp == 0:
                        v_proj(jb)
                    for s in range(2):
                        h = 2 * hp + s
                        for ic in range(4):
                            nc.tensor.matmul(
                                po2[s][:, ic * 128:ic * 128 + 65],
                                pr[:, s * 512 + ic * 128:s * 512 + (ic + 1) * 128],
                                v_aug[:, jb, h * 65:(h + 1) * 65],
                                start=(jb == 0 and ic == 0),
                                stop=(jb == 7),
                                skip_group_check=(ic > 0),
                            )

                    # ---- interleaved work in this phase's PE slack ----
                    # phase order (hp,ib): (0,0) (0,1) (1,0) (1,1); remaining
                    # q/k chunks staged 1+ phase before their first use
                    if hp == 0 and ib == 0:
                        if jb == 1:
                            qk_chunk(1, 0, 1, "dve", tq)   # k m0 nb1 (dots jb4+)
                        if jb == 2:
                            qk_chunk(0, 0, 1, "dve", tp)   # q m0 nb1 (phase (0,1))
                    if hp == 0 and ib == 1:
                        if jb == 1:
                            qk_chunk(1, 1, 0, "dve", tq)   # k m1 nb0 (phase (1,0))
                        if jb == 3:
                            qk_chunk(0, 1, 0, "dve", tp)   # q m1 nb0 (phase (1,0))
                    if hp == 1 and ib == 0:
                        if jb == 0:
                            qk_chunk(1, 1, 1, "dve", tq)   # k m1 nb1 (dots jb4+)
                        if jb == 2:
                            qk_chunk(0, 1, 1, "dve", tp)   # q m1 nb1 (phase (1,1))

                    if prev is not None:
                        p_po2, p_hp, p_ib = prev
                        p_sc = sc2[p_ib]
                        if jb in (2, 3, 4, 5):
                            ic = jb - 2
                            pool = scratch(ic)
                            tt = pool.tile([P, P], BF16, tag=pool.name,
                                           name=f"tt_{p_hp}_{p_ib}_{ic}")
                            for s in range(2):
                                nc.tensor.transpose(
                                    tt[64 * s:64 * s + 64, :],
                                    att_n[:, s, ic * 64:(ic + 1) * 64],
                                    ident_sb[:],
                                )
                            nc.vector.tensor_copy(
                                p_sc[:, p_hp, ic * 128:(ic + 1) * 128], tt[:])
                        if p_hp == 1:
                            if jb in (4, 5, 6, 7):
                                io = jb - 4
                                eng = "dve" if io < 3 else "defer"
                                pyt = proj(p_sc, io, p_ib * 4 + io, eng,
                                           scratch(io + 1))
                                if io == 2:
                                    nc.sync.dma_start(
                                        y[p_ib * 512:p_ib * 512 + 384, :]
                                        .rearrange("(io p) q -> p io q", p=P),
                                        y_all[:, p_ib * 4:p_ib * 4 + 3, :])
                                deferred.append((pyt, p_ib * 4 + io))

                prev = (po2, hp, ib)

        # ---- flush: last phase's epilogue + proj(ib=1), engine-parallel -----
        # deferred io2/io3 y-copies of ib0 first: ACT is free once the exp
        # stream ends, and this keeps them off phase (1,1)'s busy DVE
        for pyt, dst_io in deferred[3:]:
            nc.scalar.copy(y_all[:, dst_io, :], pyt[:])
        if deferred:
            nc.sync.dma_start(y[384:512, :], y_all[:, 3, :])
        p_po2, p_hp, p_ib = prev
        p_sc = sc2[p_ib]
        rc = rc_pool.tile([P, 2, 4], F32, tag="rc", name="rc_flush")
        att_n = attn_pool.tile([P, 2, 256], BF16, tag="attn", name="attn_flush")
        for s in range(2):
            nc.vector.reciprocal(
                rc[:, s, :],
                p_po2[s][:].rearrange("p (ic c) -> p ic c", c=128)[:, :, 64],
            )
        for ic in range(4):
            # split normalization across DVE (s=0) and Pool (s=1)
            nc.vector.tensor_scalar_mul(
                att_n[:, 0, ic * 64:(ic + 1) * 64],
                p_po2[0][:, ic * 128:ic * 128 + 64],
                rc[:, 0, ic:ic + 1],
            )
            nc.scalar.activation(
                att_n[:, 1, ic * 64:(ic + 1) * 64],
                p_po2[1][:, ic * 128:ic * 128 + 64],
                mybir.ActivationFunctionType.Copy,
                scale=rc[:, 1, ic:ic + 1],
            )
            # transposes from the (now idle) wide pool
            tt = wide.tile([P, P], BF16, tag="big", name=f"tt_flush_{ic}")
            for s in range(2):
                nc.tensor.transpose(
                    tt[64 * s:64 * s + 64, :],
                    att_n[:, s, ic * 64:(ic + 1) * 64],
                    ident_sb[:],
                )
            nc.vector.tensor_copy(p_sc[:, p_hp, ic * 128:(ic + 1) * 128], tt[:])
            py = scratch(ic).tile([P, 512], F32, tag=scratch(ic).name,
                                  name=f"py_flush_{ic}")
            for u in range(2):
                nc.tensor.matmul(
                    py[:],
                    p_sc[:, u, ic * 128:(ic + 1) * 128],
                    wo2_sb[:, u, :],
                    start=(u == 0),
                    stop=(u == 1),
                )
            # y copies alternate ACT/DVE so neither serializes the tail
            if ic % 2 == 0:
                nc.scalar.copy(y_all[:, p_ib * 4 + ic, :], py[:])
            else:
                nc.vector.tensor_copy(y_all[:, p_ib * 4 + ic, :], py[:])
            nc.sync.dma_start(
                y[p_ib * 512 + ic * 128:p_ib * 512 + (ic + 1) * 128, :],
                y_all[:, p_ib * 4 + ic, :])

    nc.compile()
    return nc


def _get_nc():
    if "v2" not in _NC:
        _NC["v2"] = build_nc()
    return _NC["v2"]


def make_in_maps(x, spd, head_keep, w_qkv, w_out):
    x = np.asarray(x, np.float32)
    spd = np.asarray(spd, np.float32)
    keep = np.asarray(head_keep, np.float32)
    w_qkv = np.asarray(w_qkv, np.float32)
    w_out = np.asarray(w_out, np.float32)
    cfac = keep * (HEADS / keep.sum())
    ident = np.eye(P, dtype=ml_dtypes.bfloat16)

    in_maps = []
    for c in range(8):
        bi, hh = divmod(c, 2)
        h0 = hh * HL
        hs = slice(h0 * DIM_HEAD, (h0 + HL) * DIM_HEAD)
        xT = np.ascontiguousarray(x[bi].T.astype(ml_dtypes.bfloat16))
        q_cols = w_qkv[:, hs] * np.float32(SCALE)
        k_cols = w_qkv[:, DIM + h0 * DIM_HEAD:DIM + (h0 + HL) * DIM_HEAD]
        v_cols_h = w_qkv[:, 2 * DIM + h0 * DIM_HEAD:2 * DIM + (h0 + HL) * DIM_HEAD]
        w3 = np.ascontiguousarray(np.concatenate(
            [q_cols[:, :128], k_cols[:, :128], v_cols_h,
             q_cols[:, 128:], k_cols[:, 128:]],
            axis=1,
        ).astype(ml_dtypes.bfloat16))
        # wo2[(s,d), hp, :] = w_out row of head (h0+2hp+s), dim d, * cfac
        wo_rows = w_out[hs, :] * np.repeat(cfac[h0:h0 + HL], DIM_HEAD)[:, None]
        wo4 = wo_rows.reshape(2, 2, DIM_HEAD, DIM)      # [hp, s, d, dim]
        wo2 = np.ascontiguousarray(
            wo4.transpose(1, 2, 0, 3).reshape(2 * DIM_HEAD, 2, DIM)
            .astype(ml_dtypes.bfloat16))
        sp = spd[bi, h0:h0 + HL]  # [HL, i, j] with h = 2*hp + s
        # [hp, s, ib, ii, jb, jj] -> [hp, ib, jj, jb, s, ii]
        spdT = sp.reshape(2, 2, 2, 512, 8, 128).transpose(0, 2, 5, 4, 1, 3)
        spdT = np.exp(spdT).astype(ml_dtypes.bfloat16)
        in_maps.append({"xT": xT, "w3": w3, "wo2": wo2, "ident": ident,
                        "spdT": np.ascontiguousarray(spdT)})
    return in_maps


def kernel(x, spd, head_keep, w_qkv, w_out, b_out):
    assert x.shape == (B, N, DIM) and spd.shape == (B, HEADS, N, N)
    nc = _get_nc()
    in_maps = make_in_maps(x, spd, head_keep, w_qkv, w_out)
    res = run_bass_kernel_spmd(nc, in_maps, core_ids=list(range(8)))
    out = np.empty((B, N, DIM), np.float32)
    for bi in range(B):
        out[bi] = (res.results[2 * bi]["y"].astype(np.float32)
                   + res.results[2 * bi + 1]["y"].astype(np.float32))
    out += np.asarray(b_out, np.float32)[None, None, :]
    return out


# revision 3
# speedup vs baseline: 1.0196x; 1.0076x over previous
"""Trainium2 Bass kernel v2 for nn_Attention_spd.

Reference computation (b=4, n=1024, dim=512, heads=8, dim_head=64):
    qkv = x @ w_qkv ; q,k,v = split
    dots = q @ k^T * scale + spd
    attn = softmax(dots) * (head_keep * H / sum(head_keep))
    out  = (attn @ v) @ w_out + b_out

Sharding: core c handles batch c//2, local heads 4*(c%2)..+3 (DP x TP).
Host sums the two bf16 partial outputs per batch and adds b_out.

Design notes:
  - All DRAM traffic bf16 (x, w_qkv, w_out, exp(spd), y partials): ~10.9MB/core.
  - attn@v computed TRANSPOSED with v as the *moving* operand:
    out[i, d] tiles of [128 i, 65] cost only 65 PE columns each (v augmented
    with a ones column so col 64 accumulates the softmax denominator).
    The 8 accumulation groups (2 heads x 4 i-blocks) share two PSUM banks via
    the lazy bank-zero semantics: only the first group issues start=True
    (wiping the whole bank); the other 3 start with start=False and are
    zero-seeded by the pending-zero region.
  - Softmax normalization is a per-partition scalar op (reciprocal of col 64
    + tensor_scalar_mul -> bf16); no broadcast matmul.
  - Normalized [128 i, 64 d] tiles are PE-transposed (identity matmul) into
    [(s,d), i] layout packing the head pair on 128 partitions, so the output
    projection runs with K=128 (half the matmuls of the K=64 version).
  - ACT (exp) is the pacing engine: 32 x [128,1024] exp ops. The jb loop is
    software-pipelined: dots(jb+1) is emitted BEFORE attnv(jb) so the
    in-order PE queue never lockstep-stalls the ACT stream.
  - PSUM: wide pd ring (2x2 banks) + po2 ring (2x1) + two 1-bank scratch
    rings (tp/tq) for v/qk-chunk/transpose/proj tiles, used alternately.
  - q/k m0 computed for i-cols 0:512 first (minimal head before the first
    dots); remaining q/k chunks + v projection interleave into phase slack.
"""
import os
import sys

for _p in ("/opt/trn_rl_repo", os.path.expanduser("~/.axon_site/_ro/trn_rl_repo")):
    if os.path.isdir(_p) and _p not in sys.path:
        sys.path.insert(0, _p)

import numpy as np
import ml_dtypes

import concourse.bass as bass  # noqa: F401
import concourse.tile as tile
from concourse import bacc, mybir
from concourse.bass_utils import run_bass_kernel_spmd

P = 128
B, N, DIM = 4, 1024, 512
HEADS = 8
DIM_HEAD = 64
SCALE = DIM_HEAD ** -0.5
HL = 4          # heads per core (local)
F32 = mybir.dt.float32
BF16 = mybir.dt.bfloat16
MULT = mybir.AluOpType.mult
EXP = mybir.ActivationFunctionType.Exp

_NC = {}


def build_nc():
    nc = bacc.Bacc("TRN2", target_bir_lowering=False, debug=False, num_devices=8)
    xT = nc.dram_tensor("xT", [DIM, N], BF16, kind="ExternalInput").ap()
    # [qm0 | km0 | v | qm1 | km1] column blocks (q pre-scaled by SCALE)
    w3 = nc.dram_tensor("w3", [DIM, 3 * HL * DIM_HEAD], BF16, kind="ExternalInput").ap()
    # packed for K=128 proj: [(s,d), hp, dim]
    wo2 = nc.dram_tensor("wo2", [P, 2, DIM], BF16, kind="ExternalInput").ap()
    ident = nc.dram_tensor("ident", [P, P], BF16, kind="ExternalInput").ap()
    # exp(spd) bf16: [hp, ib, j, jb, s, i]
    spdT = nc.dram_tensor("spdT", [2, 2, P, 8, 2, 512], BF16, kind="ExternalInput").ap()
    y = nc.dram_tensor("y", [N, DIM], BF16, kind="ExternalOutput").ap()

    from contextlib import ExitStack

    with tile.TileContext(nc) as tc, ExitStack() as ctx:
        sb = ctx.enter_context(tc.tile_pool(name="sb", bufs=1))
        spd_pool = ctx.enter_context(tc.tile_pool(name="spd", bufs=4))
        ex_pool = ctx.enter_context(tc.tile_pool(name="ex", bufs=5))
        pr_pool = ctx.enter_context(tc.tile_pool(name="pr", bufs=5))
        attn_pool = ctx.enter_context(tc.tile_pool(name="attn", bufs=2))
        rc_pool = ctx.enter_context(tc.tile_pool(name="rc", bufs=2))
        sc2_pool = ctx.enter_context(tc.tile_pool(name="sc2", bufs=2))
        # PSUM: 4 (wide pd ring) + 2 (po2) + 1 (tp) + 1 (tq) = 8 banks
        wide = ctx.enter_context(tc.tile_pool(name="wide", bufs=2, space="PSUM"))
        npo = ctx.enter_context(tc.tile_pool(name="npo", bufs=2, space="PSUM"))
        tp = ctx.enter_context(tc.tile_pool(name="tp", bufs=1, space="PSUM"))
        tq = ctx.enter_context(tc.tile_pool(name="tq", bufs=1, space="PSUM"))

        def scratch(i):
            return tp if i % 2 == 0 else tq

        # ---- consts + warm-up ----------------------------------------------
        seed = sb.tile([P, 512], BF16, tag="seed")
        nc.gpsimd.memset(seed[:], 1.0)
        # PE p-state ramp: busy early so real matmuls hit full speed; the
        # warm matmuls also bridge the initial DMA wait
        warm = tp.tile([P, 512], F32, tag="tp", name="warm")
        for _ in range(5):
            nc.tensor.matmul(warm[:, :512], seed[0:1, 0:128], seed[0:1, 0:512],
                             start=True, stop=True)

        # ---- resident loads -------------------------------------------------
        xT_sb = sb.tile([P, 4, N], BF16)
        w3_sb = sb.tile([P, 4, 768], BF16, tag="w3")
        wo2_sb = sb.tile([P, 2, DIM], BF16, tag="wo2")
        ident_sb = sb.tile([P, P], BF16, tag="ident")
        xT_r = xT.rearrange("(kb p) n -> p kb n", p=P)
        w3_r = w3.rearrange("(kb p) m -> p kb m", p=P)
        # ordered so the first q/k projections + first spd tile land earliest
        nc.sync.dma_start(w3_sb[:, :, 0:256], w3_r[:, :, 0:256])      # q/k m0
        nc.sync.dma_start(xT_sb[:, 0:2, 0:512], xT_r[:, 0:2, 0:512])
        nc.sync.dma_start(xT_sb[:, 2:4, 0:512], xT_r[:, 2:4, 0:512])
        nc.sync.dma_start(w3_sb[:, :, 256:512], w3_r[:, :, 256:512])  # v
        st00 = spd_pool.tile([P, 8, 2, 512], BF16, tag="spd", name="spd_0_0")
        nc.sync.dma_start(st00[:, 0:4], spdT[0, 0, :, 0:4])
        nc.sync.dma_start(xT_sb[:, 0:2, 512:1024], xT_r[:, 0:2, 512:1024])
        nc.sync.dma_start(xT_sb[:, 2:4, 512:1024], xT_r[:, 2:4, 512:1024])
        nc.sync.dma_start(st00[:, 4:8], spdT[0, 0, :, 4:8])
        nc.sync.dma_start(w3_sb[:, :, 512:768], w3_r[:, :, 512:768])  # q/k m1
        nc.sync.dma_start(wo2_sb[:], wo2[:])
        nc.sync.dma_start(ident_sb[:], ident[:])

        qT_sb = sb.tile([P, 2, N], BF16, tag="qT")
        kT_sb = sb.tile([P, 2, N], BF16, tag="kT")
        v_aug = sb.tile([P, 8, HL * 65], BF16, tag="vaug")
        v_cols = v_aug[:].rearrange("p jb (h c) -> p jb h c", c=65)
        nc.vector.memset(v_cols[:, :, :, 64:65], 1.0)
        y_all = sb.tile([P, 8, DIM], BF16, tag="yall")

        # ---- minimal head: q/k m0 for i-cols 0:512 only ---------------------
        # qk: 0=q, 1=k; m: 0=heads 0/1, 1=heads 2/3; nb: i-col half
        def qk_chunk(qk, m, nb, copy_engine, pool):
            wofs = (512 if m else 0) + qk * 128
            dst = qT_sb if qk == 0 else kT_sb
            pq = pool.tile([P, 512], F32, tag=pool.name,
                           name=f"pq_{qk}_{m}_{nb}")
            for kb in range(4):
                nc.tensor.matmul(
                    pq[:],
                    w3_sb[:, kb, wofs:wofs + 128],
                    xT_sb[:, kb, nb * 512:(nb + 1) * 512],
                    start=(kb == 0),
                    stop=(kb == 3),
                )
            if copy_engine == "pool":
                nc.gpsimd.tensor_copy(dst[:, m, nb * 512:(nb + 1) * 512], pq[:])
            else:
                nc.vector.tensor_copy(dst[:, m, nb * 512:(nb + 1) * 512], pq[:])

        # q/k m0 nb0 with split copies so the first dots' last dependency
        # lands as early as possible
        pqq = tp.tile([P, 512], F32, tag="tp", name="pq_0_0_0")
        for kb in range(4):
            nc.tensor.matmul(
                pqq[:],
                w3_sb[:, kb, 0:128],
                xT_sb[:, kb, 0:512],
                start=(kb == 0),
                stop=(kb == 3),
            )
        nc.vector.tensor_copy(qT_sb[:, 0, 0:256], pqq[:, 0:256])
        nc.vector.tensor_copy(qT_sb[:, 0, 256:512], pqq[:, 256:512])
        pqk = tq.tile([P, 512], F32, tag="tq", name="pq_1_0_0")
        for kb in range(4):
            nc.tensor.matmul(
                pqk[:],
                w3_sb[:, kb, 128:256],
                xT_sb[:, kb, 0:512],
                start=(kb == 0),
                stop=(kb == 3),
            )
        nc.vector.tensor_copy(kT_sb[:, 0, 0:256], pqk[:, 0:256])
        nc.scalar.copy(kT_sb[:, 0, 256:512], pqk[:, 256:512])

        def v_proj(jb):
            pv = scratch(jb).tile([P, 512], F32, tag=scratch(jb).name,
                                  name=f"pv_{jb}")
            for kb in range(4):
                nc.tensor.matmul(
                    pv[:, :256],
                    xT_sb[:, kb, jb * 128:(jb + 1) * 128],
                    w3_sb[:, kb, 256:512],
                    start=(kb == 0),
                    stop=(kb == 3),
                )
            nc.vector.tensor_copy(
                v_cols[:, jb, :, :64],
                pv[:, :256].rearrange("p (h c) -> p h c", c=64),
            )

        # ---- attention ------------------------------------------------------
        sc2 = {}

        deferred = []

        def proj(sc, io, dst_io, ycopy_engine, pool):
            py = pool.tile([P, 512], F32, tag=pool.name, name=f"py_{dst_io}")
            for u in range(2):
                nc.tensor.matmul(
                    py[:],
                    sc[:, u, io * 128:(io + 1) * 128],
                    wo2_sb[:, u, :],
                    start=(u == 0),
                    stop=(u == 1),
                )
            if ycopy_engine == "act":
                nc.scalar.copy(y_all[:, dst_io, :], py[:])
            elif ycopy_engine == "dve":
                nc.vector.tensor_copy(y_all[:, dst_io, :], py[:])
            return py

        prev = None
        for hp in range(2):
            for ib in range(2):
                if ib == 0 and hp == 0:
                    st = st00
                else:
                    st = spd_pool.tile([P, 8, 2, 512], BF16, tag="spd",
                                       name=f"spd_{hp}_{ib}")
                    nc.sync.dma_start(st[:, 0:4], spdT[hp, ib, :, 0:4])
                    nc.sync.dma_start(st[:, 4:8], spdT[hp, ib, :, 4:8])
                if ib not in sc2:
                    sc2[ib] = sc2_pool.tile([P, 2, 512], BF16, tag="sc2",
                                            name=f"sc2_{ib}")

                # ---- prev phase normalization FIRST (frees its po2 slots
                # before this phase's attnv reuses the 2-slot ring) ----
                if prev is not None:
                    p_po2, p_hp, p_ib = prev
                    p_sc = sc2[p_ib]
                    rc = rc_pool.tile([P, 2, 4], F32, tag="rc",
                                      name=f"rc_{p_hp}_{p_ib}")
                    att_n = attn_pool.tile([P, 2, 256], BF16, tag="attn",
                                           name=f"attn_{p_hp}_{p_ib}")
                    for s in range(2):
                        nc.vector.reciprocal(
                            rc[:, s, :],
                            p_po2[s][:].rearrange("p (ic c) -> p ic c", c=128)[:, :, 64],
                        )
                    # normalization on DVE (GPSIMD cannot access PSUM on HW)
                    for s in range(2):
                        for ic in range(4):
                            nc.vector.tensor_scalar_mul(
                                att_n[:, s, ic * 64:(ic + 1) * 64],
                                p_po2[s][:, ic * 128:ic * 128 + 64],
                                rc[:, s, ic:ic + 1],
                            )

                po2 = [npo.tile([P, 512], F32, tag="po", name=f"po2_{hp}_{ib}_{s}")
                       for s in range(2)]

                def dots(jb):
                    pd = wide.tile([P, 1024], F32, tag="big",
                                   name=f"pd_{hp}_{ib}_{jb}")
                    for s in range(2):
                        nc.tensor.matmul(
                            pd[:, s * 512:(s + 1) * 512],
                            kT_sb[64 * s:64 * s + 64, hp, jb * 128:(jb + 1) * 128],
                            qT_sb[64 * s:64 * s + 64, hp, ib * 512:(ib + 1) * 512],
                            start=True,
                            stop=True,
                        )
                    return pd

                if hp == 0 and ib == 0:
                    # first dots split into i-halves: each sub-matmul starts
                    # as soon as its half of the q copy lands
                    pd = wide.tile([P, 1024], F32, tag="big", name="pd_0_0_0")
                    for s in range(2):
                        for ihalf in range(2):
                            nc.tensor.matmul(
                                pd[:, s * 512 + ihalf * 256:s * 512 + (ihalf + 1) * 256],
                                kT_sb[64 * s:64 * s + 64, 0, 0:128],
                                qT_sb[64 * s:64 * s + 64, 0, ihalf * 256:(ihalf + 1) * 256],
                                start=True,
                                stop=True,
                            )
                else:
                    pd = dots(0)
                for jb in range(8):
                    ex = ex_pool.tile([P, 1024], BF16, tag="ex",
                                      name=f"ex_{hp}_{ib}_{jb}")
                    pr = pr_pool.tile([P, 1024], BF16, tag="pr",
                                      name=f"pr_{hp}_{ib}_{jb}")
                    nc.scalar.activation(ex[:], pd[:], EXP)
                    nc.vector.tensor_tensor(
                        pr[:], ex[:],
                        st[:, jb].rearrange("p s i -> p (s i)"),
                        MULT,
                    )
                    # software pipelining: next dots queued on PE BEFORE this
                    # unit's attnv (which waits on ACT+DVE)
                    if jb < 7:
                        pd = dots(jb + 1)
                    # v projection feeds attnv of phase (0,0) just in time
                    if ib == 0 and hp == 0:
                        v_proj(jb)
                    for s in range(2):
                        h = 2 * hp + s
                        for ic in range(4):
                            nc.tensor.matmul(
                                po2[s][:, ic * 128:ic * 128 + 65],
                                pr[:, s * 512 + ic * 128:s * 512 + (ic + 1) * 128],
                                v_aug[:, jb, h * 65:(h + 1) * 65],
                                start=(jb == 0 and ic == 0),
                                stop=(jb == 7),
                                skip_group_check=(ic > 0),
                            )

                    # ---- interleaved work in this phase's PE slack ----
                    # phase order (hp,ib): (0,0) (0,1) (1,0) (1,1); remaining
                    # q/k chunks staged 1+ phase before their first use
                    if hp == 0 and ib == 0:
                        if jb == 1:
                            qk_chunk(1, 0, 1, "dve", tq)   # k m0 nb1 (dots jb4+)
                        if jb == 2:
                            qk_chunk(0, 0, 1, "dve", tp)   # q m0 nb1 (phase (0,1))
                    if hp == 0 and ib == 1:
                        if jb == 1:
                            qk_chunk(1, 1, 0, "dve", tq)   # k m1 nb0 (phase (1,0))
                        if jb == 3:
                            qk_chunk(0, 1, 0, "dve", tp)   # q m1 nb0 (phase (1,0))
                    if hp == 1 and ib == 0:
                        if jb == 0:
                            qk_chunk(1, 1, 1, "dve", tq)   # k m1 nb1 (dots jb4+)
                        if jb == 2:
                            qk_chunk(0, 1, 1, "dve", tp)   # q m1 nb1 (phase (1,1))

                    if prev is not None:
                        p_po2, p_hp, p_ib = prev
                        p_sc = sc2[p_ib]
                        if jb in (3, 4, 5, 6):
                            ic = jb - 3
                            pool = scratch(ic)
                            tt = pool.tile([P, P], BF16, tag=pool.name,
                                           name=f"tt_{p_hp}_{p_ib}_{ic}")
                            for s in range(2):
                                nc.tensor.transpose(
                                    tt[64 * s:64 * s + 64, :],
                                    att_n[:, s, ic * 64:(ic + 1) * 64],
                                    ident_sb[:],
                                )
                            nc.vector.tensor_copy(
                                p_sc[:, p_hp, ic * 128:(ic + 1) * 128], tt[:])
                        if p_hp == 1:
                            if jb in (4, 5, 6, 7):
                                io = jb - 4
                                eng = "dve" if io < 2 else "defer"
                                pyt = proj(p_sc, io, p_ib * 4 + io, eng,
                                           scratch(io + 1))
                                if io == 1:
                                    nc.sync.dma_start(
                                        y[p_ib * 512:p_ib * 512 + 256, :]
                                        .rearrange("(io p) q -> p io q", p=P),
                                        y_all[:, p_ib * 4:p_ib * 4 + 2, :])
                                deferred.append((pyt, p_ib * 4 + io))

                prev = (po2, hp, ib)

        # ---- flush: last phase's epilogue + proj(ib=1), engine-parallel -----
        # deferred io2/io3 y-copies of ib0 first: ACT is free once the exp
        # stream ends, and this keeps them off phase (1,1)'s busy DVE
        for pyt, dst_io in deferred[2:]:
            nc.scalar.copy(y_all[:, dst_io, :], pyt[:])
            nc.sync.dma_start(y[dst_io * 128:(dst_io + 1) * 128, :],
                              y_all[:, dst_io, :])
        p_po2, p_hp, p_ib = prev
        p_sc = sc2[p_ib]
        rc = rc_pool.tile([P, 2, 4], F32, tag="rc", name="rc_flush")
        att_n = attn_pool.tile([P, 2, 256], BF16, tag="attn", name="attn_flush")
        for s in range(2):
            nc.vector.reciprocal(
                rc[:, s, :],
                p_po2[s][:].rearrange("p (ic c) -> p ic c", c=128)[:, :, 64],
            )
        for ic in range(4):
            # split normalization across DVE (s=0) and Pool (s=1)
            nc.vector.tensor_scalar_mul(
                att_n[:, 0, ic * 64:(ic + 1) * 64],
                p_po2[0][:, ic * 128:ic * 128 + 64],
                rc[:, 0, ic:ic + 1],
            )
            nc.scalar.activation(
                att_n[:, 1, ic * 64:(ic + 1) * 64],
                p_po2[1][:, ic * 128:ic * 128 + 64],
                mybir.ActivationFunctionType.Copy,
                scale=rc[:, 1, ic:ic + 1],
            )
            # transposes from the (now idle) wide pool
            tt = wide.tile([P, P], BF16, tag="big", name=f"tt_flush_{ic}")
            for s in range(2):
                nc.tensor.transpose(
                    tt[64 * s:64 * s + 64, :],
                    att_n[:, s, ic * 64:(ic + 1) * 64],
                    ident_sb[:],
                )
            nc.vector.tensor_copy(p_sc[:, p_hp, ic * 128:(ic + 1) * 128], tt[:])
            py = scratch(ic).tile([P, 512], F32, tag=scratch(ic).name,
                                  name=f"py_flush_{ic}")
            for u in range(2):
                nc.tensor.matmul(
                    py[:],
                    p_sc[:, u, ic * 128:(ic + 1) * 128],
                    wo2_sb[:, u, :],
                    start=(u == 0),
                    stop=(u == 1),
                )
            # y copies alternate ACT/DVE so neither serializes the tail
            if ic % 2 == 0:
                nc.scalar.copy(y_all[:, p_ib * 4 + ic, :], py[:])
            else:
                nc.vector.tensor_copy(y_all[:, p_ib * 4 + ic, :], py[:])
            nc.sync.dma_start(
                y[p_ib * 512 + ic * 128:p_ib * 512 + (ic + 1) * 128, :],
                y_all[:, p_ib * 4 + ic, :])

    nc.compile()
    return nc


def _get_nc():
    if "v2" not in _NC:
        _NC["v2"] = build_nc()
    return _NC["v2"]


def make_in_maps(x, spd, head_keep, w_qkv, w_out):
    x = np.asarray(x, np.float32)
    spd = np.asarray(spd, np.float32)
    keep = np.asarray(head_keep, np.float32)
    w_qkv = np.asarray(w_qkv, np.float32)
    w_out = np.asarray(w_out, np.float32)
    cfac = keep * (HEADS / keep.sum())
    ident = np.eye(P, dtype=ml_dtypes.bfloat16)

    in_maps = []
    for c in range(8):
        bi, hh = divmod(c, 2)
        h0 = hh * HL
        hs = slice(h0 * DIM_HEAD, (h0 + HL) * DIM_HEAD)
        xT = np.ascontiguousarray(x[bi].T.astype(ml_dtypes.bfloat16))
        q_cols = w_qkv[:, hs] * np.float32(SCALE)
        k_cols = w_qkv[:, DIM + h0 * DIM_HEAD:DIM + (h0 + HL) * DIM_HEAD]
        v_cols_h = w_qkv[:, 2 * DIM + h0 * DIM_HEAD:2 * DIM + (h0 + HL) * DIM_HEAD]
        w3 = np.ascontiguousarray(np.concatenate(
            [q_cols[:, :128], k_cols[:, :128], v_cols_h,
             q_cols[:, 128:], k_cols[:, 128:]],
            axis=1,
        ).astype(ml_dtypes.bfloat16))
        # wo2[(s,d), hp, :] = w_out row of head (h0+2hp+s), dim d, * cfac
        wo_rows = w_out[hs, :] * np.repeat(cfac[h0:h0 + HL], DIM_HEAD)[:, None]
        wo4 = wo_rows.reshape(2, 2, DIM_HEAD, DIM)      # [hp, s, d, dim]
        wo2 = np.ascontiguousarray(
            wo4.transpose(1, 2, 0, 3).reshape(2 * DIM_HEAD, 2, DIM)
            .astype(ml_dtypes.bfloat16))
        sp = spd[bi, h0:h0 + HL]  # [HL, i, j] with h = 2*hp + s
        # [hp, s, ib, ii, jb, jj] -> [hp, ib, jj, jb, s, ii]
        spdT = sp.reshape(2, 2, 2, 512, 8, 128).transpose(0, 2, 5, 4, 1, 3)
        spdT = np.exp(spdT).astype(ml_dtypes.bfloat16)
        in_maps.append({"xT": xT, "w3": w3, "wo2": wo2, "ident": ident,
                        "spdT": np.ascontiguousarray(spdT)})
    return in_maps


def kernel(x, spd, head_keep, w_qkv, w_out, b_out):
    assert x.shape == (B, N, DIM) and spd.shape == (B, HEADS, N, N)
    nc = _get_nc()
    in_maps = make_in_maps(x, spd, head_keep, w_qkv, w_out)
    res = run_bass_kernel_spmd(nc, in_maps, core_ids=list(range(8)))
    out = np.empty((B, N, DIM), np.float32)
    for bi in range(B):
        out[bi] = (res.results[2 * bi]["y"].astype(np.float32)
                   + res.results[2 * bi + 1]["y"].astype(np.float32))
    out += np.asarray(b_out, np.float32)[None, None, :]
    return out


# revision 4
# speedup vs baseline: 1.0266x; 1.0069x over previous
"""Trainium2 Bass kernel v2 for nn_Attention_spd.

Reference computation (b=4, n=1024, dim=512, heads=8, dim_head=64):
    qkv = x @ w_qkv ; q,k,v = split
    dots = q @ k^T * scale + spd
    attn = softmax(dots) * (head_keep * H / sum(head_keep))
    out  = (attn @ v) @ w_out + b_out

Sharding: core c handles batch c//2, local heads 4*(c%2)..+3 (DP x TP).
Host sums the two bf16 partial outputs per batch and adds b_out.

Design notes:
  - All DRAM traffic bf16 (x, w_qkv, w_out, exp(spd), y partials): ~10.9MB/core.
  - attn@v computed TRANSPOSED with v as the *moving* operand:
    out[i, d] tiles of [128 i, 65] cost only 65 PE columns each (v augmented
    with a ones column so col 64 accumulates the softmax denominator).
    The 8 accumulation groups (2 heads x 4 i-blocks) share two PSUM banks via
    the lazy bank-zero semantics: only the first group issues start=True
    (wiping the whole bank); the other 3 start with start=False and are
    zero-seeded by the pending-zero region.
  - Softmax normalization is a per-partition scalar op (reciprocal of col 64
    + tensor_scalar_mul -> bf16); no broadcast matmul.
  - Normalized [128 i, 64 d] tiles are PE-transposed (identity matmul) into
    [(s,d), i] layout packing the head pair on 128 partitions, so the output
    projection runs with K=128 (half the matmuls of the K=64 version).
  - ACT (exp) is the pacing engine: 32 x [128,1024] exp ops. The jb loop is
    software-pipelined: dots(jb+1) is emitted BEFORE attnv(jb) so the
    in-order PE queue never lockstep-stalls the ACT stream.
  - PSUM: wide pd ring (2x2 banks) + po2 ring (2x1) + two 1-bank scratch
    rings (tp/tq) for v/qk-chunk/transpose/proj tiles, used alternately.
  - q/k m0 computed for i-cols 0:512 first (minimal head before the first
    dots); remaining q/k chunks + v projection interleave into phase slack.
"""
import os
import sys

for _p in ("/opt/trn_rl_repo", os.path.expanduser("~/.axon_site/_ro/trn_rl_repo")):
    if os.path.isdir(_p) and _p not in sys.path:
        sys.path.insert(0, _p)

import numpy as np
import ml_dtypes

import concourse.bass as bass  # noqa: F401
import concourse.tile as tile
from concourse import bacc, mybir
from concourse.bass_utils import run_bass_kernel_spmd

P = 128
B, N, DIM = 4, 1024, 512
HEADS = 8
DIM_HEAD = 64
SCALE = DIM_HEAD ** -0.5
HL = 4          # heads per core (local)
F32 = mybir.dt.float32
BF16 = mybir.dt.bfloat16
MULT = mybir.AluOpType.mult
EXP = mybir.ActivationFunctionType.Exp

_NC = {}


def build_nc():
    nc = bacc.Bacc("TRN2", target_bir_lowering=False, debug=False, num_devices=8)
    xT = nc.dram_tensor("xT", [DIM, N], BF16, kind="ExternalInput").ap()
    # [qm0 | km0 | v | qm1 | km1] column blocks (q pre-scaled by SCALE)
    w3 = nc.dram_tensor("w3", [DIM, 3 * HL * DIM_HEAD], BF16, kind="ExternalInput").ap()
    # packed for K=128 proj: [(s,d), hp, dim]
    wo2 = nc.dram_tensor("wo2", [P, 2, DIM], BF16, kind="ExternalInput").ap()
    ident = nc.dram_tensor("ident", [P, P], BF16, kind="ExternalInput").ap()
    # exp(spd) bf16: [hp, ib, j, jb, s, i]
    spdT = nc.dram_tensor("spdT", [2, 2, P, 8, 2, 512], BF16, kind="ExternalInput").ap()
    y = nc.dram_tensor("y", [N, DIM], BF16, kind="ExternalOutput").ap()

    from contextlib import ExitStack

    with tile.TileContext(nc) as tc, ExitStack() as ctx:
        sb = ctx.enter_context(tc.tile_pool(name="sb", bufs=1))
        spd_pool = ctx.enter_context(tc.tile_pool(name="spd", bufs=4))
        ex_pool = ctx.enter_context(tc.tile_pool(name="ex", bufs=5))
        pr_pool = ctx.enter_context(tc.tile_pool(name="pr", bufs=5))
        attn_pool = ctx.enter_context(tc.tile_pool(name="attn", bufs=2))
        rc_pool = ctx.enter_context(tc.tile_pool(name="rc", bufs=2))
        sc2_pool = ctx.enter_context(tc.tile_pool(name="sc2", bufs=2))
        # PSUM: 4 (wide pd ring) + 2 (po2) + 1 (tp) + 1 (tq) = 8 banks
        wide = ctx.enter_context(tc.tile_pool(name="wide", bufs=2, space="PSUM"))
        npo = ctx.enter_context(tc.tile_pool(name="npo", bufs=2, space="PSUM"))
        tp = ctx.enter_context(tc.tile_pool(name="tp", bufs=1, space="PSUM"))
        tq = ctx.enter_context(tc.tile_pool(name="tq", bufs=1, space="PSUM"))

        def scratch(i):
            return tp if i % 2 == 0 else tq

        # ---- consts + warm-up ----------------------------------------------
        seed = sb.tile([P, 512], BF16, tag="seed")
        nc.gpsimd.memset(seed[:], 1.0)
        # PE p-state ramp: busy early so real matmuls hit full speed; the
        # warm matmuls also bridge the initial DMA wait
        warm = tp.tile([P, 512], F32, tag="tp", name="warm")
        for _ in range(5):
            nc.tensor.matmul(warm[:, :512], seed[0:1, 0:128], seed[0:1, 0:512],
                             start=True, stop=True)

        # ---- resident loads -------------------------------------------------
        xT_sb = sb.tile([P, 4, N], BF16)
        w3_sb = sb.tile([P, 4, 768], BF16, tag="w3")
        wo2_sb = sb.tile([P, 2, DIM], BF16, tag="wo2")
        ident_sb = sb.tile([P, P], BF16, tag="ident")
        xT_r = xT.rearrange("(kb p) n -> p kb n", p=P)
        w3_r = w3.rearrange("(kb p) m -> p kb m", p=P)
        # ordered so the first q/k projections + first spd tile land earliest
        nc.sync.dma_start(w3_sb[:, :, 0:256], w3_r[:, :, 0:256])      # q/k m0
        nc.sync.dma_start(xT_sb[:, 0:2, 0:512], xT_r[:, 0:2, 0:512])
        nc.sync.dma_start(xT_sb[:, 2:4, 0:512], xT_r[:, 2:4, 0:512])
        nc.sync.dma_start(w3_sb[:, :, 256:512], w3_r[:, :, 256:512])  # v
        st00 = spd_pool.tile([P, 8, 2, 512], BF16, tag="spd", name="spd_0_0")
        nc.sync.dma_start(st00[:, 0:4], spdT[0, 0, :, 0:4])
        nc.sync.dma_start(xT_sb[:, 0:2, 512:1024], xT_r[:, 0:2, 512:1024])
        nc.sync.dma_start(xT_sb[:, 2:4, 512:1024], xT_r[:, 2:4, 512:1024])
        nc.sync.dma_start(st00[:, 4:8], spdT[0, 0, :, 4:8])
        nc.sync.dma_start(w3_sb[:, :, 512:768], w3_r[:, :, 512:768])  # q/k m1
        nc.sync.dma_start(wo2_sb[:], wo2[:])
        nc.sync.dma_start(ident_sb[:], ident[:])

        qT_sb = sb.tile([P, 2, N], BF16, tag="qT")
        kT_sb = sb.tile([P, 2, N], BF16, tag="kT")
        v_aug = sb.tile([P, 8, HL * 65], BF16, tag="vaug")
        v_cols = v_aug[:].rearrange("p jb (h c) -> p jb h c", c=65)
        nc.vector.memset(v_cols[:, :, :, 64:65], 1.0)
        y_all = sb.tile([P, 8, DIM], BF16, tag="yall")

        # ---- minimal head: q/k m0 for i-cols 0:512 only ---------------------
        # qk: 0=q, 1=k; m: 0=heads 0/1, 1=heads 2/3; nb: i-col half
        def qk_chunk(qk, m, nb, copy_engine, pool):
            wofs = (512 if m else 0) + qk * 128
            dst = qT_sb if qk == 0 else kT_sb
            pq = pool.tile([P, 512], F32, tag=pool.name,
                           name=f"pq_{qk}_{m}_{nb}")
            for kb in range(4):
                nc.tensor.matmul(
                    pq[:],
                    w3_sb[:, kb, wofs:wofs + 128],
                    xT_sb[:, kb, nb * 512:(nb + 1) * 512],
                    start=(kb == 0),
                    stop=(kb == 3),
                )
            if copy_engine == "pool":
                nc.gpsimd.tensor_copy(dst[:, m, nb * 512:(nb + 1) * 512], pq[:])
            else:
                nc.vector.tensor_copy(dst[:, m, nb * 512:(nb + 1) * 512], pq[:])

        # q/k m0 nb0 with split copies so the first dots' last dependency
        # lands as early as possible
        pqq = tp.tile([P, 512], F32, tag="tp", name="pq_0_0_0")
        for kb in range(4):
            nc.tensor.matmul(
                pqq[:],
                w3_sb[:, kb, 0:128],
                xT_sb[:, kb, 0:512],
                start=(kb == 0),
                stop=(kb == 3),
            )
        nc.vector.tensor_copy(qT_sb[:, 0, 0:256], pqq[:, 0:256])
        nc.vector.tensor_copy(qT_sb[:, 0, 256:512], pqq[:, 256:512])
        pqk = tq.tile([P, 512], F32, tag="tq", name="pq_1_0_0")
        for kb in range(4):
            nc.tensor.matmul(
                pqk[:],
                w3_sb[:, kb, 128:256],
                xT_sb[:, kb, 0:512],
                start=(kb == 0),
                stop=(kb == 3),
            )
        nc.vector.tensor_copy(kT_sb[:, 0, 0:256], pqk[:, 0:256])
        nc.scalar.copy(kT_sb[:, 0, 256:512], pqk[:, 256:512])

        def v_proj(jb):
            pv = scratch(jb).tile([P, 512], F32, tag=scratch(jb).name,
                                  name=f"pv_{jb}")
            for kb in range(4):
                nc.tensor.matmul(
                    pv[:, :256],
                    xT_sb[:, kb, jb * 128:(jb + 1) * 128],
                    w3_sb[:, kb, 256:512],
                    start=(kb == 0),
                    stop=(kb == 3),
                )
            nc.vector.tensor_copy(
                v_cols[:, jb, :, :64],
                pv[:, :256].rearrange("p (h c) -> p h c", c=64),
            )

        # ---- attention ------------------------------------------------------
        sc2 = {}

        deferred = []

        def proj(sc, io, dst_io, ycopy_engine, pool):
            py = pool.tile([P, 512], F32, tag=pool.name, name=f"py_{dst_io}")
            for u in range(2):
                nc.tensor.matmul(
                    py[:],
                    sc[:, u, io * 128:(io + 1) * 128],
                    wo2_sb[:, u, :],
                    start=(u == 0),
                    stop=(u == 1),
                )
            if ycopy_engine == "act":
                nc.scalar.copy(y_all[:, dst_io, :], py[:])
            elif ycopy_engine == "dve":
                nc.vector.tensor_copy(y_all[:, dst_io, :], py[:])
            return py

        prev = None
        for hp in range(2):
            for ib in range(2):
                if ib == 0 and hp == 0:
                    st = st00
                else:
                    st = spd_pool.tile([P, 8, 2, 512], BF16, tag="spd",
                                       name=f"spd_{hp}_{ib}")
                    nc.sync.dma_start(st[:, 0:4], spdT[hp, ib, :, 0:4])
                    nc.sync.dma_start(st[:, 4:8], spdT[hp, ib, :, 4:8])
                if ib not in sc2:
                    sc2[ib] = sc2_pool.tile([P, 2, 512], BF16, tag="sc2",
                                            name=f"sc2_{ib}")

                # ---- prev phase normalization FIRST (frees its po2 slots
                # before this phase's attnv reuses the 2-slot ring) ----
                if prev is not None:
                    p_po2, p_hp, p_ib = prev
                    p_sc = sc2[p_ib]
                    rc = rc_pool.tile([P, 2, 4], F32, tag="rc",
                                      name=f"rc_{p_hp}_{p_ib}")
                    att_n = attn_pool.tile([P, 2, 256], BF16, tag="attn",
                                           name=f"attn_{p_hp}_{p_ib}")
                    for s in range(2):
                        nc.vector.reciprocal(
                            rc[:, s, :],
                            p_po2[s][:].rearrange("p (ic c) -> p ic c", c=128)[:, :, 64],
                        )
                    # normalization on DVE (GPSIMD cannot access PSUM on HW)
                    for s in range(2):
                        for ic in range(4):
                            nc.vector.tensor_scalar_mul(
                                att_n[:, s, ic * 64:(ic + 1) * 64],
                                p_po2[s][:, ic * 128:ic * 128 + 64],
                                rc[:, s, ic:ic + 1],
                            )

                po2 = [npo.tile([P, 512], F32, tag="po", name=f"po2_{hp}_{ib}_{s}")
                       for s in range(2)]
                prs = {}

                def dots(jb):
                    pd = wide.tile([P, 1024], F32, tag="big",
                                   name=f"pd_{hp}_{ib}_{jb}")
                    for s in range(2):
                        nc.tensor.matmul(
                            pd[:, s * 512:(s + 1) * 512],
                            kT_sb[64 * s:64 * s + 64, hp, jb * 128:(jb + 1) * 128],
                            qT_sb[64 * s:64 * s + 64, hp, ib * 512:(ib + 1) * 512],
                            start=True,
                            stop=True,
                        )
                    return pd

                if hp == 0 and ib == 0:
                    # first dots split into i-halves: each sub-matmul starts
                    # as soon as its half of the q copy lands
                    pd = wide.tile([P, 1024], F32, tag="big", name="pd_0_0_0")
                    for s in range(2):
                        for ihalf in range(2):
                            nc.tensor.matmul(
                                pd[:, s * 512 + ihalf * 256:s * 512 + (ihalf + 1) * 256],
                                kT_sb[64 * s:64 * s + 64, 0, 0:128],
                                qT_sb[64 * s:64 * s + 64, 0, ihalf * 256:(ihalf + 1) * 256],
                                start=True,
                                stop=True,
                            )
                else:
                    pd = dots(0)
                for jb in range(8):
                    ex = ex_pool.tile([P, 1024], BF16, tag="ex",
                                      name=f"ex_{hp}_{ib}_{jb}")
                    pr = pr_pool.tile([P, 1024], BF16, tag="pr",
                                      name=f"pr_{hp}_{ib}_{jb}")
                    nc.scalar.activation(ex[:], pd[:], EXP)
                    nc.vector.tensor_tensor(
                        pr[:], ex[:],
                        st[:, jb].rearrange("p s i -> p (s i)"),
                        MULT,
                    )
                    # software pipelining: next dots queued on PE BEFORE this
                    # unit's attnv (which waits on ACT+DVE)
                    if jb < 7:
                        pd = dots(jb + 1)
                    # v projection feeds attnv of phase (0,0) just in time
                    if ib == 0 and hp == 0:
                        v_proj(jb)
                    def attnv(jbx, s):
                        h = 2 * hp + s
                        for ic in range(4):
                            nc.tensor.matmul(
                                po2[s][:, ic * 128:ic * 128 + 65],
                                prs[jbx][:, s * 512 + ic * 128:s * 512 + (ic + 1) * 128],
                                v_aug[:, jbx, h * 65:(h + 1) * 65],
                                start=(jbx == 0 and ic == 0),
                                stop=(jbx == 7),
                                skip_group_check=(ic > 0),
                            )

                    prs[jb] = pr
                    attnv(jb, 0)
                    # s1 attnv one slot behind: if its pr isn't ready yet it
                    # parks in the wait queue WITHOUT blocking next-jb dots
                    if jb > 0:
                        attnv(jb - 1, 1)
                    if jb == 7:
                        attnv(7, 1)

                    # ---- interleaved work in this phase's PE slack ----
                    # phase order (hp,ib): (0,0) (0,1) (1,0) (1,1); remaining
                    # q/k chunks staged 1+ phase before their first use
                    if hp == 0 and ib == 0:
                        if jb == 1:
                            qk_chunk(1, 0, 1, "dve", tq)   # k m0 nb1 (dots jb4+)
                        if jb == 2:
                            qk_chunk(0, 0, 1, "dve", tp)   # q m0 nb1 (phase (0,1))
                    if hp == 0 and ib == 1:
                        if jb == 1:
                            qk_chunk(1, 1, 0, "dve", tq)   # k m1 nb0 (phase (1,0))
                        if jb == 3:
                            qk_chunk(0, 1, 0, "dve", tp)   # q m1 nb0 (phase (1,0))
                    if hp == 1 and ib == 0:
                        if jb == 0:
                            qk_chunk(1, 1, 1, "dve", tq)   # k m1 nb1 (dots jb4+)
                        if jb == 2:
                            qk_chunk(0, 1, 1, "dve", tp)   # q m1 nb1 (phase (1,1))

                    if prev is not None:
                        p_po2, p_hp, p_ib = prev
                        p_sc = sc2[p_ib]
                        # deferred io0/io1 ycopies EARLY in this jb slot (and
                        # before the T block, whose tiles reuse the py slots)
                        if p_hp == 1 and jb in (6, 7):
                            dio = jb - 6
                            nc.vector.tensor_copy(
                                y_all[:, p_ib * 4 + dio, :],
                                deferred[dio][0][:])
                            if jb == 7:
                                nc.sync.dma_start(
                                    y[p_ib * 512:p_ib * 512 + 256, :]
                                    .rearrange("(io p) q -> p io q", p=P),
                                    y_all[:, p_ib * 4:p_ib * 4 + 2, :])
                        if jb in (3, 4, 5, 6):
                            ic = jb - 3
                            pool = scratch(ic)
                            tt = pool.tile([P, P], BF16, tag=pool.name,
                                           name=f"tt_{p_hp}_{p_ib}_{ic}")
                            for s in range(2):
                                nc.tensor.transpose(
                                    tt[64 * s:64 * s + 64, :],
                                    att_n[:, s, ic * 64:(ic + 1) * 64],
                                    ident_sb[:],
                                )
                            nc.vector.tensor_copy(
                                p_sc[:, p_hp, ic * 128:(ic + 1) * 128], tt[:])
                        if p_hp == 1:
                            if jb in (4, 5, 6, 7):
                                io = jb - 4
                                pyt = proj(p_sc, io, p_ib * 4 + io, "defer",
                                           scratch(io + 1))
                                deferred.append((pyt, p_ib * 4 + io))

                prev = (po2, hp, ib)

        # ---- flush: last phase's epilogue + proj(ib=1), engine-parallel -----
        # deferred io2/io3 y-copies of ib0 first: ACT is free once the exp
        # stream ends, and this keeps them off phase (1,1)'s busy DVE
        for pyt, dst_io in deferred[2:]:
            nc.scalar.copy(y_all[:, dst_io, :], pyt[:])
            nc.sync.dma_start(y[dst_io * 128:(dst_io + 1) * 128, :],
                              y_all[:, dst_io, :])
        p_po2, p_hp, p_ib = prev
        p_sc = sc2[p_ib]
        rc = rc_pool.tile([P, 2, 4], F32, tag="rc", name="rc_flush")
        att_n = attn_pool.tile([P, 2, 256], BF16, tag="attn", name="attn_flush")
        for s in range(2):
            nc.vector.reciprocal(
                rc[:, s, :],
                p_po2[s][:].rearrange("p (ic c) -> p ic c", c=128)[:, :, 64],
            )
        for ic in range(4):
            # normalization: s0 on DVE, s1 on ACT (scaled copy)
            nc.vector.tensor_scalar_mul(
                att_n[:, 0, ic * 64:(ic + 1) * 64],
                p_po2[0][:, ic * 128:ic * 128 + 64],
                rc[:, 0, ic:ic + 1],
            )
            nc.scalar.activation(
                att_n[:, 1, ic * 64:(ic + 1) * 64],
                p_po2[1][:, ic * 128:ic * 128 + 64],
                mybir.ActivationFunctionType.Copy,
                scale=rc[:, 1, ic:ic + 1],
            )
        tts = []
        for ic in range(4):
            tt = wide.tile([P, P], BF16, tag="big", name=f"tt_flush_{ic}")
            for s in range(2):
                nc.tensor.transpose(
                    tt[64 * s:64 * s + 64, :],
                    att_n[:, s, ic * 64:(ic + 1) * 64],
                    ident_sb[:],
                )
            tts.append(tt)
            # wide ring is 2-deep: copy must follow within the pair
            if ic % 2 == 1:
                for icc in (ic - 1, ic):
                    nc.vector.tensor_copy(
                        p_sc[:, p_hp, icc * 128:(icc + 1) * 128], tts[icc][:])
        for ic in range(4):
            py = scratch(ic).tile([P, 512], F32, tag=scratch(ic).name,
                                  name=f"py_flush_{ic}")
            for u in range(2):
                nc.tensor.matmul(
                    py[:],
                    p_sc[:, u, ic * 128:(ic + 1) * 128],
                    wo2_sb[:, u, :],
                    start=(u == 0),
                    stop=(u == 1),
                )
            if ic % 2 == 0:
                nc.scalar.copy(y_all[:, p_ib * 4 + ic, :], py[:])
            else:
                nc.vector.tensor_copy(y_all[:, p_ib * 4 + ic, :], py[:])
            nc.sync.dma_start(
                y[p_ib * 512 + ic * 128:p_ib * 512 + (ic + 1) * 128, :],
                y_all[:, p_ib * 4 + ic, :])

    nc.compile()
    return nc


def _get_nc():
    if "v2" not in _NC:
        _NC["v2"] = build_nc()
    return _NC["v2"]


def make_in_maps(x, spd, head_keep, w_qkv, w_out):
    x = np.asarray(x, np.float32)
    spd = np.asarray(spd, np.float32)
    keep = np.asarray(head_keep, np.float32)
    w_qkv = np.asarray(w_qkv, np.float32)
    w_out = np.asarray(w_out, np.float32)
    cfac = keep * (HEADS / keep.sum())
    ident = np.eye(P, dtype=ml_dtypes.bfloat16)

    in_maps = []
    for c in range(8):
        bi, hh = divmod(c, 2)
        h0 = hh * HL
        hs = slice(h0 * DIM_HEAD, (h0 + HL) * DIM_HEAD)
        xT = np.ascontiguousarray(x[bi].T.astype(ml_dtypes.bfloat16))
        q_cols = w_qkv[:, hs] * np.float32(SCALE)
        k_cols = w_qkv[:, DIM + h0 * DIM_HEAD:DIM + (h0 + HL) * DIM_HEAD]
        v_cols_h = w_qkv[:, 2 * DIM + h0 * DIM_HEAD:2 * DIM + (h0 + HL) * DIM_HEAD]
        w3 = np.ascontiguousarray(np.concatenate(
            [q_cols[:, :128], k_cols[:, :128], v_cols_h,
             q_cols[:, 128:], k_cols[:, 128:]],
            axis=1,
        ).astype(ml_dtypes.bfloat16))
        # wo2[(s,d), hp, :] = w_out row of head (h0+2hp+s), dim d, * cfac
        wo_rows = w_out[hs, :] * np.repeat(cfac[h0:h0 + HL], DIM_HEAD)[:, None]
        wo4 = wo_rows.reshape(2, 2, DIM_HEAD, DIM)      # [hp, s, d, dim]
        wo2 = np.ascontiguousarray(
            wo4.transpose(1, 2, 0, 3).reshape(2 * DIM_HEAD, 2, DIM)
            .astype(ml_dtypes.bfloat16))
        sp = spd[bi, h0:h0 + HL]  # [HL, i, j] with h = 2*hp + s
        # [hp, s, ib, ii, jb, jj] -> [hp, ib, jj, jb, s, ii]
        spdT = sp.reshape(2, 2, 2, 512, 8, 128).transpose(0, 2, 5, 4, 1, 3)
        spdT = np.exp(spdT).astype(ml_dtypes.bfloat16)
        in_maps.append({"xT": xT, "w3": w3, "wo2": wo2, "ident": ident,
                        "spdT": np.ascontiguousarray(spdT)})
    return in_maps


def kernel(x, spd, head_keep, w_qkv, w_out, b_out):
    assert x.shape == (B, N, DIM) and spd.shape == (B, HEADS, N, N)
    nc = _get_nc()
    in_maps = make_in_maps(x, spd, head_keep, w_qkv, w_out)
    res = run_bass_kernel_spmd(nc, in_maps, core_ids=list(range(8)))
    out = np.empty((B, N, DIM), np.float32)
    for bi in range(B):
        out[bi] = (res.results[2 * bi]["y"].astype(np.float32)
                   + res.results[2 * bi + 1]["y"].astype(np.float32))
    out += np.asarray(b_out, np.float32)[None, None, :]
    return out


# revision 5
# speedup vs baseline: 1.0305x; 1.0038x over previous
"""Trainium2 Bass kernel v2 for nn_Attention_spd.

Reference computation (b=4, n=1024, dim=512, heads=8, dim_head=64):
    qkv = x @ w_qkv ; q,k,v = split
    dots = q @ k^T * scale + spd
    attn = softmax(dots) * (head_keep * H / sum(head_keep))
    out  = (attn @ v) @ w_out + b_out

Sharding: core c handles batch c//2, local heads 4*(c%2)..+3 (DP x TP).
Host sums the two bf16 partial outputs per batch and adds b_out.

Design notes:
  - All DRAM traffic bf16 (x, w_qkv, w_out, exp(spd), y partials): ~10.9MB/core.
  - attn@v computed TRANSPOSED with v as the *moving* operand:
    out[i, d] tiles of [128 i, 65] cost only 65 PE columns each (v augmented
    with a ones column so col 64 accumulates the softmax denominator).
    The 8 accumulation groups (2 heads x 4 i-blocks) share two PSUM banks via
    the lazy bank-zero semantics: only the first group issues start=True
    (wiping the whole bank); the other 3 start with start=False and are
    zero-seeded by the pending-zero region.
  - Softmax normalization is a per-partition scalar op (reciprocal of col 64
    + tensor_scalar_mul -> bf16); no broadcast matmul.
  - Normalized [128 i, 64 d] tiles are PE-transposed (identity matmul) into
    [(s,d), i] layout packing the head pair on 128 partitions, so the output
    projection runs with K=128 (half the matmuls of the K=64 version).
  - ACT (exp) is the pacing engine: 32 x [128,1024] exp ops. The jb loop is
    software-pipelined: dots(jb+1) is emitted BEFORE attnv(jb) so the
    in-order PE queue never lockstep-stalls the ACT stream.
  - PSUM: wide pd ring (2x2 banks) + po2 ring (2x1) + two 1-bank scratch
    rings (tp/tq) for v/qk-chunk/transpose/proj tiles, used alternately.
  - q/k m0 computed for i-cols 0:512 first (minimal head before the first
    dots); remaining q/k chunks + v projection interleave into phase slack.
"""
import os
import sys

for _p in ("/opt/trn_rl_repo", os.path.expanduser("~/.axon_site/_ro/trn_rl_repo")):
    if os.path.isdir(_p) and _p not in sys.path:
        sys.path.insert(0, _p)

import numpy as np
import ml_dtypes

import concourse.bass as bass  # noqa: F401
import concourse.tile as tile
from concourse import bacc, mybir
from concourse.bass_utils import run_bass_kernel_spmd

P = 128
B, N, DIM = 4, 1024, 512
HEADS = 8
DIM_HEAD = 64
SCALE = DIM_HEAD ** -0.5
HL = 4          # heads per core (local)
F32 = mybir.dt.float32
BF16 = mybir.dt.bfloat16
MULT = mybir.AluOpType.mult
EXP = mybir.ActivationFunctionType.Exp

_NC = {}


def build_nc():
    nc = bacc.Bacc("TRN2", target_bir_lowering=False, debug=False, num_devices=8)
    xT = nc.dram_tensor("xT", [DIM, N], BF16, kind="ExternalInput").ap()
    # [qm0 | km0 | v | qm1 | km1] column blocks (q pre-scaled by SCALE)
    w3 = nc.dram_tensor("w3", [DIM, 3 * HL * DIM_HEAD], BF16, kind="ExternalInput").ap()
    # packed for K=128 proj: [(s,d), hp, dim]
    wo2 = nc.dram_tensor("wo2", [P, 2, DIM], BF16, kind="ExternalInput").ap()
    ident = nc.dram_tensor("ident", [P, P], BF16, kind="ExternalInput").ap()
    # exp(spd) bf16: [hp, ib, j, jb, s, i]
    spdT = nc.dram_tensor("spdT", [2, 2, P, 8, 2, 512], BF16, kind="ExternalInput").ap()
    y = nc.dram_tensor("y", [N, DIM], BF16, kind="ExternalOutput").ap()

    from contextlib import ExitStack

    with tile.TileContext(nc) as tc, ExitStack() as ctx:
        sb = ctx.enter_context(tc.tile_pool(name="sb", bufs=1))
        spd_pool = ctx.enter_context(tc.tile_pool(name="spd", bufs=4))
        ex_pool = ctx.enter_context(tc.tile_pool(name="ex", bufs=5))
        pr_pool = ctx.enter_context(tc.tile_pool(name="pr", bufs=5))
        attn_pool = ctx.enter_context(tc.tile_pool(name="attn", bufs=2))
        rc_pool = ctx.enter_context(tc.tile_pool(name="rc", bufs=2))
        sc2_pool = ctx.enter_context(tc.tile_pool(name="sc2", bufs=2))
        # PSUM: 4 (wide pd ring) + 2 (po2) + 1 (tp) + 1 (tq) = 8 banks
        wide = ctx.enter_context(tc.tile_pool(name="wide", bufs=2, space="PSUM"))
        npo = ctx.enter_context(tc.tile_pool(name="npo", bufs=2, space="PSUM"))
        tp = ctx.enter_context(tc.tile_pool(name="tp", bufs=1, space="PSUM"))
        tq = ctx.enter_context(tc.tile_pool(name="tq", bufs=1, space="PSUM"))

        def scratch(i):
            return tp if i % 2 == 0 else tq

        # ---- consts + warm-up ----------------------------------------------
        seed = sb.tile([P, 512], BF16, tag="seed")
        nc.gpsimd.memset(seed[:], 1.0)
        # PE p-state ramp: busy early so real matmuls hit full speed; the
        # warm matmuls also bridge the initial DMA wait
        warm = tp.tile([P, 512], F32, tag="tp", name="warm")
        for _ in range(5):
            nc.tensor.matmul(warm[:, :512], seed[0:1, 0:128], seed[0:1, 0:512],
                             start=True, stop=True)

        # ---- resident loads -------------------------------------------------
        xT_sb = sb.tile([P, 4, N], BF16)
        w3_sb = sb.tile([P, 4, 768], BF16, tag="w3")
        wo2_sb = sb.tile([P, 2, DIM], BF16, tag="wo2")
        ident_sb = sb.tile([P, P], BF16, tag="ident")
        xT_r = xT.rearrange("(kb p) n -> p kb n", p=P)
        w3_r = w3.rearrange("(kb p) m -> p kb m", p=P)
        # ordered so the first q/k projections + first spd tile land earliest
        nc.sync.dma_start(w3_sb[:, :, 0:256], w3_r[:, :, 0:256])      # q/k m0
        nc.sync.dma_start(xT_sb[:, 0:2, 0:512], xT_r[:, 0:2, 0:512])
        nc.sync.dma_start(xT_sb[:, 2:4, 0:512], xT_r[:, 2:4, 0:512])
        nc.sync.dma_start(w3_sb[:, :, 256:512], w3_r[:, :, 256:512])  # v
        st00 = spd_pool.tile([P, 8, 2, 512], BF16, tag="spd", name="spd_0_0")
        nc.sync.dma_start(st00[:, 0:4], spdT[0, 0, :, 0:4])
        nc.sync.dma_start(xT_sb[:, 0:2, 512:1024], xT_r[:, 0:2, 512:1024])
        nc.sync.dma_start(xT_sb[:, 2:4, 512:1024], xT_r[:, 2:4, 512:1024])
        nc.sync.dma_start(st00[:, 4:8], spdT[0, 0, :, 4:8])
        nc.sync.dma_start(w3_sb[:, :, 512:768], w3_r[:, :, 512:768])  # q/k m1
        nc.sync.dma_start(wo2_sb[:], wo2[:])
        nc.sync.dma_start(ident_sb[:], ident[:])

        qT_sb = sb.tile([P, 2, N], BF16, tag="qT")
        kT_sb = sb.tile([P, 2, N], BF16, tag="kT")
        v_aug = sb.tile([P, 8, HL * 65], BF16, tag="vaug")
        v_cols = v_aug[:].rearrange("p jb (h c) -> p jb h c", c=65)
        nc.vector.memset(v_cols[:, :, :, 64:65], 1.0)
        y_all = sb.tile([P, 8, DIM], BF16, tag="yall")

        # ---- minimal head: q/k m0 for i-cols 0:512 only ---------------------
        # qk: 0=q, 1=k; m: 0=heads 0/1, 1=heads 2/3; nb: i-col half
        def qk_chunk(qk, m, nb, copy_engine, pool):
            wofs = (512 if m else 0) + qk * 128
            dst = qT_sb if qk == 0 else kT_sb
            pq = pool.tile([P, 512], F32, tag=pool.name,
                           name=f"pq_{qk}_{m}_{nb}")
            for kb in range(4):
                nc.tensor.matmul(
                    pq[:],
                    w3_sb[:, kb, wofs:wofs + 128],
                    xT_sb[:, kb, nb * 512:(nb + 1) * 512],
                    start=(kb == 0),
                    stop=(kb == 3),
                )
            if copy_engine == "pool":
                nc.gpsimd.tensor_copy(dst[:, m, nb * 512:(nb + 1) * 512], pq[:])
            else:
                nc.vector.tensor_copy(dst[:, m, nb * 512:(nb + 1) * 512], pq[:])

        # q/k m0 nb0 with split copies so the first dots' last dependency
        # lands as early as possible
        pqq = tp.tile([P, 512], F32, tag="tp", name="pq_0_0_0")
        for kb in range(4):
            nc.tensor.matmul(
                pqq[:],
                w3_sb[:, kb, 0:128],
                xT_sb[:, kb, 0:512],
                start=(kb == 0),
                stop=(kb == 3),
            )
        nc.vector.tensor_copy(qT_sb[:, 0, 0:256], pqq[:, 0:256])
        nc.vector.tensor_copy(qT_sb[:, 0, 256:512], pqq[:, 256:512])
        pqk = tq.tile([P, 512], F32, tag="tq", name="pq_1_0_0")
        for kb in range(4):
            nc.tensor.matmul(
                pqk[:],
                w3_sb[:, kb, 128:256],
                xT_sb[:, kb, 0:512],
                start=(kb == 0),
                stop=(kb == 3),
            )
        nc.vector.tensor_copy(kT_sb[:, 0, 0:256], pqk[:, 0:256])
        nc.scalar.copy(kT_sb[:, 0, 256:512], pqk[:, 256:512])

        def v_proj(jb):
            pv = scratch(jb).tile([P, 512], F32, tag=scratch(jb).name,
                                  name=f"pv_{jb}")
            for kb in range(4):
                nc.tensor.matmul(
                    pv[:, :256],
                    xT_sb[:, kb, jb * 128:(jb + 1) * 128],
                    w3_sb[:, kb, 256:512],
                    start=(kb == 0),
                    stop=(kb == 3),
                )
            nc.vector.tensor_copy(
                v_cols[:, jb, :, :64],
                pv[:, :256].rearrange("p (h c) -> p h c", c=64),
            )

        # ---- attention ------------------------------------------------------
        sc2 = {}

        deferred = []

        def proj(sc, io, dst_io, ycopy_engine, pool):
            py = pool.tile([P, 512], F32, tag=pool.name, name=f"py_{dst_io}")
            for u in range(2):
                nc.tensor.matmul(
                    py[:],
                    sc[:, u, io * 128:(io + 1) * 128],
                    wo2_sb[:, u, :],
                    start=(u == 0),
                    stop=(u == 1),
                )
            if ycopy_engine == "act":
                nc.scalar.copy(y_all[:, dst_io, :], py[:])
            elif ycopy_engine == "dve":
                nc.vector.tensor_copy(y_all[:, dst_io, :], py[:])
            return py

        prev = None
        for hp in range(2):
            for ib in range(2):
                if ib == 0 and hp == 0:
                    st = st00
                else:
                    st = spd_pool.tile([P, 8, 2, 512], BF16, tag="spd",
                                       name=f"spd_{hp}_{ib}")
                    nc.sync.dma_start(st[:, 0:4], spdT[hp, ib, :, 0:4])
                    nc.sync.dma_start(st[:, 4:8], spdT[hp, ib, :, 4:8])
                if ib not in sc2:
                    sc2[ib] = sc2_pool.tile([P, 2, 512], BF16, tag="sc2",
                                            name=f"sc2_{ib}")

                # ---- prev phase normalization FIRST (frees its po2 slots
                # before this phase's attnv reuses the 2-slot ring) ----
                if prev is not None:
                    p_po2, p_hp, p_ib = prev
                    p_sc = sc2[p_ib]
                    rc = rc_pool.tile([P, 2, 4], F32, tag="rc",
                                      name=f"rc_{p_hp}_{p_ib}")
                    att_n = attn_pool.tile([P, 2, 256], BF16, tag="attn",
                                           name=f"attn_{p_hp}_{p_ib}")
                    for s in range(2):
                        nc.vector.reciprocal(
                            rc[:, s, :],
                            p_po2[s][:].rearrange("p (ic c) -> p ic c", c=128)[:, :, 64],
                        )
                    # normalization on DVE (GPSIMD cannot access PSUM on HW);
                    # s0 now, s1 deferred into the jb0 body (its attnv batch
                    # is a slot behind, so the burst can be spread)
                    for ic in range(4):
                        nc.vector.tensor_scalar_mul(
                            att_n[:, 0, ic * 64:(ic + 1) * 64],
                            p_po2[0][:, ic * 128:ic * 128 + 64],
                            rc[:, 0, ic:ic + 1],
                        )

                po2 = [npo.tile([P, 512], F32, tag="po", name=f"po2_{hp}_{ib}_{s}")
                       for s in range(2)]
                prs = {}

                def dots(jb):
                    pd = wide.tile([P, 1024], F32, tag="big",
                                   name=f"pd_{hp}_{ib}_{jb}")
                    for s in range(2):
                        nc.tensor.matmul(
                            pd[:, s * 512:(s + 1) * 512],
                            kT_sb[64 * s:64 * s + 64, hp, jb * 128:(jb + 1) * 128],
                            qT_sb[64 * s:64 * s + 64, hp, ib * 512:(ib + 1) * 512],
                            start=True,
                            stop=True,
                        )
                    return pd

                if hp == 0 and ib == 0:
                    # first dots split into i-halves: each sub-matmul starts
                    # as soon as its half of the q copy lands
                    pd = wide.tile([P, 1024], F32, tag="big", name="pd_0_0_0")
                    for s in range(2):
                        for ihalf in range(2):
                            nc.tensor.matmul(
                                pd[:, s * 512 + ihalf * 256:s * 512 + (ihalf + 1) * 256],
                                kT_sb[64 * s:64 * s + 64, 0, 0:128],
                                qT_sb[64 * s:64 * s + 64, 0, ihalf * 256:(ihalf + 1) * 256],
                                start=True,
                                stop=True,
                            )
                else:
                    pd = dots(0)
                for jb in range(8):
                    ex = ex_pool.tile([P, 1024], BF16, tag="ex",
                                      name=f"ex_{hp}_{ib}_{jb}")
                    pr = pr_pool.tile([P, 1024], BF16, tag="pr",
                                      name=f"pr_{hp}_{ib}_{jb}")
                    nc.scalar.activation(ex[:], pd[:], EXP)
                    nc.vector.tensor_tensor(
                        pr[:], ex[:],
                        st[:, jb].rearrange("p s i -> p (s i)"),
                        MULT,
                    )
                    if jb == 1 and prev is not None:
                        for ic in range(4):
                            nc.vector.tensor_scalar_mul(
                                att_n[:, 1, ic * 64:(ic + 1) * 64],
                                prev[0][1][:, ic * 128:ic * 128 + 64],
                                rc[:, 1, ic:ic + 1],
                            )
                    # software pipelining: next dots queued on PE BEFORE this
                    # unit's attnv (which waits on ACT+DVE)
                    if jb < 7:
                        pd = dots(jb + 1)
                    # v projection feeds attnv of phase (0,0) just in time
                    if ib == 0 and hp == 0:
                        v_proj(jb)
                    def attnv(jbx, s):
                        h = 2 * hp + s
                        for ic in range(4):
                            nc.tensor.matmul(
                                po2[s][:, ic * 128:ic * 128 + 65],
                                prs[jbx][:, s * 512 + ic * 128:s * 512 + (ic + 1) * 128],
                                v_aug[:, jbx, h * 65:(h + 1) * 65],
                                start=(jbx == 0 and ic == 0),
                                stop=(jbx == 7),
                                skip_group_check=(ic > 0),
                            )

                    prs[jb] = pr
                    attnv(jb, 0)
                    # s1 attnv one slot behind: if its pr isn't ready yet it
                    # parks in the wait queue WITHOUT blocking next-jb dots
                    if jb > 0:
                        attnv(jb - 1, 1)
                    if jb == 7:
                        attnv(7, 1)

                    # ---- interleaved work in this phase's PE slack ----
                    # phase order (hp,ib): (0,0) (0,1) (1,0) (1,1); remaining
                    # q/k chunks staged 1+ phase before their first use
                    if hp == 0 and ib == 0:
                        if jb == 1:
                            qk_chunk(1, 0, 1, "dve", tq)   # k m0 nb1 (dots jb4+)
                        if jb == 2:
                            qk_chunk(0, 0, 1, "dve", tp)   # q m0 nb1 (phase (0,1))
                    if hp == 0 and ib == 1:
                        if jb == 1:
                            qk_chunk(1, 1, 0, "dve", tq)   # k m1 nb0 (phase (1,0))
                        if jb == 3:
                            qk_chunk(0, 1, 0, "dve", tp)   # q m1 nb0 (phase (1,0))
                    if hp == 1 and ib == 0:
                        if jb == 0:
                            qk_chunk(1, 1, 1, "dve", tq)   # k m1 nb1 (dots jb4+)
                        if jb == 2:
                            qk_chunk(0, 1, 1, "dve", tp)   # q m1 nb1 (phase (1,1))

                    if prev is not None:
                        p_po2, p_hp, p_ib = prev
                        p_sc = sc2[p_ib]
                        # deferred io0/io1 ycopies EARLY in this jb slot (and
                        # before the T block, whose tiles reuse the py slots)
                        if p_hp == 1 and jb in (6, 7):
                            dio = jb - 6
                            nc.vector.tensor_copy(
                                y_all[:, p_ib * 4 + dio, :],
                                deferred[dio][0][:])
                            if jb == 7:
                                nc.sync.dma_start(
                                    y[p_ib * 512:p_ib * 512 + 256, :]
                                    .rearrange("(io p) q -> p io q", p=P),
                                    y_all[:, p_ib * 4:p_ib * 4 + 2, :])
                        if jb in (3, 4, 5, 6):
                            ic = jb - 3
                            pool = scratch(ic)
                            tt = pool.tile([P, P], BF16, tag=pool.name,
                                           name=f"tt_{p_hp}_{p_ib}_{ic}")
                            for s in range(2):
                                nc.tensor.transpose(
                                    tt[64 * s:64 * s + 64, :],
                                    att_n[:, s, ic * 64:(ic + 1) * 64],
                                    ident_sb[:],
                                )
                            nc.vector.tensor_copy(
                                p_sc[:, p_hp, ic * 128:(ic + 1) * 128], tt[:])
                        if p_hp == 1:
                            if jb in (4, 5, 6, 7):
                                io = jb - 4
                                pyt = proj(p_sc, io, p_ib * 4 + io, "defer",
                                           scratch(io + 1))
                                deferred.append((pyt, p_ib * 4 + io))

                prev = (po2, hp, ib)

        # ---- flush: last phase's epilogue + proj(ib=1), engine-parallel -----
        # deferred io2/io3 y-copies of ib0 first: ACT is free once the exp
        # stream ends, and this keeps them off phase (1,1)'s busy DVE
        for pyt, dst_io in deferred[2:]:
            nc.scalar.copy(y_all[:, dst_io, :], pyt[:])
            nc.sync.dma_start(y[dst_io * 128:(dst_io + 1) * 128, :],
                              y_all[:, dst_io, :])
        p_po2, p_hp, p_ib = prev
        p_sc = sc2[p_ib]
        rc = rc_pool.tile([P, 2, 4], F32, tag="rc", name="rc_flush")
        att_n = attn_pool.tile([P, 2, 256], BF16, tag="attn", name="attn_flush")
        for s in range(2):
            nc.vector.reciprocal(
                rc[:, s, :],
                p_po2[s][:].rearrange("p (ic c) -> p ic c", c=128)[:, :, 64],
            )
        for ic in range(4):
            # normalization: whole ic on one engine, alternating, so each
            # transpose pair is gated by a single fast engine
            for s in range(2):
                if ic % 2 == 1:
                    nc.scalar.activation(
                        att_n[:, s, ic * 64:(ic + 1) * 64],
                        p_po2[s][:, ic * 128:ic * 128 + 64],
                        mybir.ActivationFunctionType.Copy,
                        scale=rc[:, s, ic:ic + 1],
                    )
                else:
                    nc.vector.tensor_scalar_mul(
                        att_n[:, s, ic * 64:(ic + 1) * 64],
                        p_po2[s][:, ic * 128:ic * 128 + 64],
                        rc[:, s, ic:ic + 1],
                    )
        tts = []
        for ic in range(4):
            tt = wide.tile([P, P], BF16, tag="big", name=f"tt_flush_{ic}")
            for s in range(2):
                nc.tensor.transpose(
                    tt[64 * s:64 * s + 64, :],
                    att_n[:, s, ic * 64:(ic + 1) * 64],
                    ident_sb[:],
                )
            tts.append(tt)
            # wide ring is 2-deep: copy must follow within the pair
            if ic % 2 == 1:
                for icc in (ic - 1, ic):
                    nc.vector.tensor_copy(
                        p_sc[:, p_hp, icc * 128:(icc + 1) * 128], tts[icc][:])
        for ic in range(4):
            py = scratch(ic).tile([P, 512], F32, tag=scratch(ic).name,
                                  name=f"py_flush_{ic}")
            for u in range(2):
                nc.tensor.matmul(
                    py[:],
                    p_sc[:, u, ic * 128:(ic + 1) * 128],
                    wo2_sb[:, u, :],
                    start=(u == 0),
                    stop=(u == 1),
                )
            if ic % 2 == 1:
                nc.scalar.copy(y_all[:, p_ib * 4 + ic, :], py[:])
            else:
                nc.vector.tensor_copy(y_all[:, p_ib * 4 + ic, :], py[:])
            nc.sync.dma_start(
                y[p_ib * 512 + ic * 128:p_ib * 512 + (ic + 1) * 128, :],
                y_all[:, p_ib * 4 + ic, :])

    nc.compile()
    return nc


def _get_nc():
    if "v2" not in _NC:
        _NC["v2"] = build_nc()
    return _NC["v2"]


def make_in_maps(x, spd, head_keep, w_qkv, w_out):
    x = np.asarray(x, np.float32)
    spd = np.asarray(spd, np.float32)
    keep = np.asarray(head_keep, np.float32)
    w_qkv = np.asarray(w_qkv, np.float32)
    w_out = np.asarray(w_out, np.float32)
    cfac = keep * (HEADS / keep.sum())
    ident = np.eye(P, dtype=ml_dtypes.bfloat16)

    in_maps = []
    for c in range(8):
        bi, hh = divmod(c, 2)
        h0 = hh * HL
        hs = slice(h0 * DIM_HEAD, (h0 + HL) * DIM_HEAD)
        xT = np.ascontiguousarray(x[bi].T.astype(ml_dtypes.bfloat16))
        q_cols = w_qkv[:, hs] * np.float32(SCALE)
        k_cols = w_qkv[:, DIM + h0 * DIM_HEAD:DIM + (h0 + HL) * DIM_HEAD]
        v_cols_h = w_qkv[:, 2 * DIM + h0 * DIM_HEAD:2 * DIM + (h0 + HL) * DIM_HEAD]
        w3 = np.ascontiguousarray(np.concatenate(
            [q_cols[:, :128], k_cols[:, :128], v_cols_h,
             q_cols[:, 128:], k_cols[:, 128:]],
            axis=1,
        ).astype(ml_dtypes.bfloat16))
        # wo2[(s,d), hp, :] = w_out row of head (h0+2hp+s), dim d, * cfac
        wo_rows = w_out[hs, :] * np.repeat(cfac[h0:h0 + HL], DIM_HEAD)[:, None]
        wo4 = wo_rows.reshape(2, 2, DIM_HEAD, DIM)      # [hp, s, d, dim]
        wo2 = np.ascontiguousarray(
            wo4.transpose(1, 2, 0, 3).reshape(2 * DIM_HEAD, 2, DIM)
            .astype(ml_dtypes.bfloat16))
        sp = spd[bi, h0:h0 + HL]  # [HL, i, j] with h = 2*hp + s
        # [hp, s, ib, ii, jb, jj] -> [hp, ib, jj, jb, s, ii]
        spdT = sp.reshape(2, 2, 2, 512, 8, 128).transpose(0, 2, 5, 4, 1, 3)
        spdT = np.exp(spdT).astype(ml_dtypes.bfloat16)
        in_maps.append({"xT": xT, "w3": w3, "wo2": wo2, "ident": ident,
                        "spdT": np.ascontiguousarray(spdT)})
    return in_maps


def kernel(x, spd, head_keep, w_qkv, w_out, b_out):
    assert x.shape == (B, N, DIM) and spd.shape == (B, HEADS, N, N)
    nc = _get_nc()
    in_maps = make_in_maps(x, spd, head_keep, w_qkv, w_out)
    res = run_bass_kernel_spmd(nc, in_maps, core_ids=list(range(8)))
    out = np.empty((B, N, DIM), np.float32)
    for bi in range(B):
        out[bi] = (res.results[2 * bi]["y"].astype(np.float32)
                   + res.results[2 * bi + 1]["y"].astype(np.float32))
    out += np.asarray(b_out, np.float32)[None, None, :]
    return out


# revision 6
# speedup vs baseline: 1.0336x; 1.0030x over previous
"""Trainium2 Bass kernel v2 for nn_Attention_spd.

Reference computation (b=4, n=1024, dim=512, heads=8, dim_head=64):
    qkv = x @ w_qkv ; q,k,v = split
    dots = q @ k^T * scale + spd
    attn = softmax(dots) * (head_keep * H / sum(head_keep))
    out  = (attn @ v) @ w_out + b_out

Sharding: core c handles batch c//2, local heads 4*(c%2)..+3 (DP x TP).
Host sums the two bf16 partial outputs per batch and adds b_out.

Design notes:
  - All DRAM traffic bf16 (x, w_qkv, w_out, exp(spd), y partials): ~10.9MB/core.
  - attn@v computed TRANSPOSED with v as the *moving* operand:
    out[i, d] tiles of [128 i, 65] cost only 65 PE columns each (v augmented
    with a ones column so col 64 accumulates the softmax denominator).
    The 8 accumulation groups (2 heads x 4 i-blocks) share two PSUM banks via
    the lazy bank-zero semantics: only the first group issues start=True
    (wiping the whole bank); the other 3 start with start=False and are
    zero-seeded by the pending-zero region.
  - Softmax normalization is a per-partition scalar op (reciprocal of col 64
    + tensor_scalar_mul -> bf16); no broadcast matmul.
  - Normalized [128 i, 64 d] tiles are PE-transposed (identity matmul) into
    [(s,d), i] layout packing the head pair on 128 partitions, so the output
    projection runs with K=128 (half the matmuls of the K=64 version).
  - ACT (exp) is the pacing engine: 32 x [128,1024] exp ops. The jb loop is
    software-pipelined: dots(jb+1) is emitted BEFORE attnv(jb) so the
    in-order PE queue never lockstep-stalls the ACT stream.
  - PSUM: wide pd ring (2x2 banks) + po2 ring (2x1) + two 1-bank scratch
    rings (tp/tq) for v/qk-chunk/transpose/proj tiles, used alternately.
  - q/k m0 computed for i-cols 0:512 first (minimal head before the first
    dots); remaining q/k chunks + v projection interleave into phase slack.
"""
import os
import sys

for _p in ("/opt/trn_rl_repo", os.path.expanduser("~/.axon_site/_ro/trn_rl_repo")):
    if os.path.isdir(_p) and _p not in sys.path:
        sys.path.insert(0, _p)

import numpy as np
import ml_dtypes

import concourse.bass as bass  # noqa: F401
import concourse.tile as tile
from concourse import bacc, mybir
from concourse.bass_utils import run_bass_kernel_spmd

P = 128
B, N, DIM = 4, 1024, 512
HEADS = 8
DIM_HEAD = 64
SCALE = DIM_HEAD ** -0.5
HL = 4          # heads per core (local)
F32 = mybir.dt.float32
BF16 = mybir.dt.bfloat16
MULT = mybir.AluOpType.mult
EXP = mybir.ActivationFunctionType.Exp

_NC = {}


def build_nc():
    nc = bacc.Bacc("TRN2", target_bir_lowering=False, debug=False, num_devices=8)
    xT = nc.dram_tensor("xT", [DIM, N], BF16, kind="ExternalInput").ap()
    # [qm0 | km0 | v | qm1 | km1] column blocks (q pre-scaled by SCALE)
    w3 = nc.dram_tensor("w3", [DIM, 3 * HL * DIM_HEAD], BF16, kind="ExternalInput").ap()
    # packed for K=128 proj: [(s,d), hp, dim]
    wo2 = nc.dram_tensor("wo2", [P, 2, DIM], BF16, kind="ExternalInput").ap()
    ident = nc.dram_tensor("ident", [P, P], BF16, kind="ExternalInput").ap()
    # exp(spd) bf16: [hp, ib, j, jb, s, i]
    spdT = nc.dram_tensor("spdT", [2, 2, P, 8, 2, 512], BF16, kind="ExternalInput").ap()
    y = nc.dram_tensor("y", [N, DIM], BF16, kind="ExternalOutput").ap()

    from contextlib import ExitStack

    with tile.TileContext(nc) as tc, ExitStack() as ctx:
        sb = ctx.enter_context(tc.tile_pool(name="sb", bufs=1))
        spd_pool = ctx.enter_context(tc.tile_pool(name="spd", bufs=4))
        ex_pool = ctx.enter_context(tc.tile_pool(name="ex", bufs=5))
        pr_pool = ctx.enter_context(tc.tile_pool(name="pr", bufs=5))
        attn_pool = ctx.enter_context(tc.tile_pool(name="attn", bufs=2))
        rc_pool = ctx.enter_context(tc.tile_pool(name="rc", bufs=2))
        sc2_pool = ctx.enter_context(tc.tile_pool(name="sc2", bufs=2))
        # PSUM: 4 (wide pd ring) + 2 (po2) + 1 (tp) + 1 (tq) = 8 banks
        wide = ctx.enter_context(tc.tile_pool(name="wide", bufs=2, space="PSUM"))
        npo = ctx.enter_context(tc.tile_pool(name="npo", bufs=2, space="PSUM"))
        tp = ctx.enter_context(tc.tile_pool(name="tp", bufs=1, space="PSUM"))
        tq = ctx.enter_context(tc.tile_pool(name="tq", bufs=1, space="PSUM"))

        def scratch(i):
            return tp if i % 2 == 0 else tq

        # ---- consts + warm-up ----------------------------------------------
        seed = sb.tile([P, 512], BF16, tag="seed")
        nc.gpsimd.memset(seed[:], 1.0)
        # PE p-state ramp: busy early so real matmuls hit full speed; the
        # warm matmuls also bridge the initial DMA wait
        warm = tp.tile([P, 512], F32, tag="tp", name="warm")
        for _ in range(5):
            nc.tensor.matmul(warm[:, :512], seed[0:1, 0:128], seed[0:1, 0:512],
                             start=True, stop=True)

        # ---- resident loads -------------------------------------------------
        xT_sb = sb.tile([P, 4, N], BF16)
        w3_sb = sb.tile([P, 4, 768], BF16, tag="w3")
        wo2_sb = sb.tile([P, 2, DIM], BF16, tag="wo2")
        ident_sb = sb.tile([P, P], BF16, tag="ident")
        xT_r = xT.rearrange("(kb p) n -> p kb n", p=P)
        w3_r = w3.rearrange("(kb p) m -> p kb m", p=P)
        # ordered so the first q/k projections + first spd tile land earliest
        nc.sync.dma_start(w3_sb[:, :, 0:256], w3_r[:, :, 0:256])      # q/k m0
        nc.sync.dma_start(xT_sb[:, 0:2, 0:512], xT_r[:, 0:2, 0:512])
        nc.sync.dma_start(xT_sb[:, 2:4, 0:512], xT_r[:, 2:4, 0:512])
        nc.sync.dma_start(w3_sb[:, :, 256:512], w3_r[:, :, 256:512])  # v
        st00 = spd_pool.tile([P, 8, 2, 512], BF16, tag="spd", name="spd_0_0")
        nc.sync.dma_start(st00[:, 0:4], spdT[0, 0, :, 0:4])
        nc.sync.dma_start(xT_sb[:, 0:2, 512:1024], xT_r[:, 0:2, 512:1024])
        nc.sync.dma_start(xT_sb[:, 2:4, 512:1024], xT_r[:, 2:4, 512:1024])
        nc.sync.dma_start(st00[:, 4:8], spdT[0, 0, :, 4:8])
        nc.sync.dma_start(w3_sb[:, :, 512:768], w3_r[:, :, 512:768])  # q/k m1
        nc.sync.dma_start(wo2_sb[:], wo2[:])
        nc.sync.dma_start(ident_sb[:], ident[:])

        qT_sb = sb.tile([P, 2, N], BF16, tag="qT")
        kT_sb = sb.tile([P, 2, N], BF16, tag="kT")
        v_aug = sb.tile([P, 8, HL * 65], BF16, tag="vaug")
        v_cols = v_aug[:].rearrange("p jb (h c) -> p jb h c", c=65)
        nc.vector.memset(v_cols[:, :, :, 64:65], 1.0)
        y_all = sb.tile([P, 8, DIM], BF16, tag="yall")

        # ---- minimal head: q/k m0 for i-cols 0:512 only ---------------------
        # qk: 0=q, 1=k; m: 0=heads 0/1, 1=heads 2/3; nb: i-col half
        def qk_chunk(qk, m, nb, copy_engine, pool):
            wofs = (512 if m else 0) + qk * 128
            dst = qT_sb if qk == 0 else kT_sb
            pq = pool.tile([P, 512], F32, tag=pool.name,
                           name=f"pq_{qk}_{m}_{nb}")
            for kb in range(4):
                nc.tensor.matmul(
                    pq[:],
                    w3_sb[:, kb, wofs:wofs + 128],
                    xT_sb[:, kb, nb * 512:(nb + 1) * 512],
                    start=(kb == 0),
                    stop=(kb == 3),
                )
            if copy_engine == "pool":
                nc.gpsimd.tensor_copy(dst[:, m, nb * 512:(nb + 1) * 512], pq[:])
            else:
                nc.vector.tensor_copy(dst[:, m, nb * 512:(nb + 1) * 512], pq[:])

        # q/k m0 nb0 with split copies so the first dots' last dependency
        # lands as early as possible
        pqq = tp.tile([P, 512], F32, tag="tp", name="pq_0_0_0")
        for kb in range(4):
            nc.tensor.matmul(
                pqq[:],
                w3_sb[:, kb, 0:128],
                xT_sb[:, kb, 0:512],
                start=(kb == 0),
                stop=(kb == 3),
            )
        nc.vector.tensor_copy(qT_sb[:, 0, 0:256], pqq[:, 0:256])
        nc.vector.tensor_copy(qT_sb[:, 0, 256:512], pqq[:, 256:512])
        pqk = tq.tile([P, 512], F32, tag="tq", name="pq_1_0_0")
        for kb in range(4):
            nc.tensor.matmul(
                pqk[:],
                w3_sb[:, kb, 128:256],
                xT_sb[:, kb, 0:512],
                start=(kb == 0),
                stop=(kb == 3),
            )
        nc.vector.tensor_copy(kT_sb[:, 0, 0:256], pqk[:, 0:256])
        nc.scalar.copy(kT_sb[:, 0, 256:512], pqk[:, 256:512])

        def v_proj(jb):
            pv = scratch(jb).tile([P, 512], F32, tag=scratch(jb).name,
                                  name=f"pv_{jb}")
            for kb in range(4):
                nc.tensor.matmul(
                    pv[:, :256],
                    xT_sb[:, kb, jb * 128:(jb + 1) * 128],
                    w3_sb[:, kb, 256:512],
                    start=(kb == 0),
                    stop=(kb == 3),
                )
            nc.vector.tensor_copy(
                v_cols[:, jb, :, :64],
                pv[:, :256].rearrange("p (h c) -> p h c", c=64),
            )

        # ---- attention ------------------------------------------------------
        sc2 = {}

        deferred = []

        def proj(sc, io, dst_io, ycopy_engine, pool):
            py = pool.tile([P, 512], F32, tag=pool.name, name=f"py_{dst_io}")
            for u in range(2):
                nc.tensor.matmul(
                    py[:],
                    sc[:, u, io * 128:(io + 1) * 128],
                    wo2_sb[:, u, :],
                    start=(u == 0),
                    stop=(u == 1),
                )
            if ycopy_engine == "act":
                nc.scalar.copy(y_all[:, dst_io, :], py[:])
            elif ycopy_engine == "dve":
                nc.vector.tensor_copy(y_all[:, dst_io, :], py[:])
            return py

        prev = None
        for hp in range(2):
            for ib in range(2):
                if ib == 0 and hp == 0:
                    st = st00
                else:
                    st = spd_pool.tile([P, 8, 2, 512], BF16, tag="spd",
                                       name=f"spd_{hp}_{ib}")
                    nc.sync.dma_start(st[:, 0:4], spdT[hp, ib, :, 0:4])
                    nc.sync.dma_start(st[:, 4:8], spdT[hp, ib, :, 4:8])
                if ib not in sc2:
                    sc2[ib] = sc2_pool.tile([P, 2, 512], BF16, tag="sc2",
                                            name=f"sc2_{ib}")

                # ---- prev phase normalization FIRST (frees its po2 slots
                # before this phase's attnv reuses the 2-slot ring) ----
                if prev is not None:
                    p_po2, p_hp, p_ib = prev
                    p_sc = sc2[p_ib]
                    rc = rc_pool.tile([P, 2, 4], F32, tag="rc",
                                      name=f"rc_{p_hp}_{p_ib}")
                    att_n = attn_pool.tile([P, 2, 256], BF16, tag="attn",
                                           name=f"attn_{p_hp}_{p_ib}")
                    for s in range(2):
                        nc.vector.reciprocal(
                            rc[:, s, :],
                            p_po2[s][:].rearrange("p (ic c) -> p ic c", c=128)[:, :, 64],
                        )
                    # normalization on DVE (GPSIMD cannot access PSUM on HW);
                    # s0 now, s1 deferred into the jb0 body (its attnv batch
                    # is a slot behind, so the burst can be spread)
                    for ic in range(4):
                        nc.vector.tensor_scalar_mul(
                            att_n[:, 0, ic * 64:(ic + 1) * 64],
                            p_po2[0][:, ic * 128:ic * 128 + 64],
                            rc[:, 0, ic:ic + 1],
                        )

                po2 = [npo.tile([P, 512], F32, tag="po", name=f"po2_{hp}_{ib}_{s}")
                       for s in range(2)]
                prs = {}

                def dots(jb):
                    pd = wide.tile([P, 1024], F32, tag="big",
                                   name=f"pd_{hp}_{ib}_{jb}")
                    for s in range(2):
                        nc.tensor.matmul(
                            pd[:, s * 512:(s + 1) * 512],
                            kT_sb[64 * s:64 * s + 64, hp, jb * 128:(jb + 1) * 128],
                            qT_sb[64 * s:64 * s + 64, hp, ib * 512:(ib + 1) * 512],
                            start=True,
                            stop=True,
                        )
                    return pd

                if hp == 0 and ib == 0:
                    # first dots split into i-halves: each sub-matmul starts
                    # as soon as its half of the q copy lands
                    pd = wide.tile([P, 1024], F32, tag="big", name="pd_0_0_0")
                    for s in range(2):
                        for ihalf in range(2):
                            nc.tensor.matmul(
                                pd[:, s * 512 + ihalf * 256:s * 512 + (ihalf + 1) * 256],
                                kT_sb[64 * s:64 * s + 64, 0, 0:128],
                                qT_sb[64 * s:64 * s + 64, 0, ihalf * 256:(ihalf + 1) * 256],
                                start=True,
                                stop=True,
                            )
                else:
                    pd = dots(0)
                for jb in range(8):
                    ex = ex_pool.tile([P, 1024], BF16, tag="ex",
                                      name=f"ex_{hp}_{ib}_{jb}")
                    pr = pr_pool.tile([P, 1024], BF16, tag="pr",
                                      name=f"pr_{hp}_{ib}_{jb}")
                    nc.scalar.activation(ex[:], pd[:], EXP)
                    nc.vector.tensor_tensor(
                        pr[:], ex[:],
                        st[:, jb].rearrange("p s i -> p (s i)"),
                        MULT,
                    )
                    if jb == 1 and prev is not None:
                        for ic in range(4):
                            nc.vector.tensor_scalar_mul(
                                att_n[:, 1, ic * 64:(ic + 1) * 64],
                                prev[0][1][:, ic * 128:ic * 128 + 64],
                                rc[:, 1, ic:ic + 1],
                            )
                    # software pipelining: next dots queued on PE BEFORE this
                    # unit's attnv (which waits on ACT+DVE)
                    if jb < 7:
                        pd = dots(jb + 1)
                    # v projection feeds attnv of phase (0,0) just in time
                    if ib == 0 and hp == 0:
                        v_proj(jb)
                    def attnv(jbx, s):
                        h = 2 * hp + s
                        for ic in range(4):
                            nc.tensor.matmul(
                                po2[s][:, ic * 128:ic * 128 + 65],
                                prs[jbx][:, s * 512 + ic * 128:s * 512 + (ic + 1) * 128],
                                v_aug[:, jbx, h * 65:(h + 1) * 65],
                                start=(jbx == 0 and ic == 0),
                                stop=(jbx == 7),
                                skip_group_check=(ic > 0),
                            )

                    prs[jb] = pr
                    attnv(jb, 0)
                    # s1 attnv one slot behind: if its pr isn't ready yet it
                    # parks in the wait queue WITHOUT blocking next-jb dots
                    if jb > 0:
                        attnv(jb - 1, 1)
                    if jb == 7:
                        attnv(7, 1)

                    # ---- interleaved work in this phase's PE slack ----
                    # phase order (hp,ib): (0,0) (0,1) (1,0) (1,1); remaining
                    # q/k chunks staged 1+ phase before their first use
                    if hp == 0 and ib == 0:
                        if jb == 1:
                            qk_chunk(1, 0, 1, "dve", tq)   # k m0 nb1 (dots jb4+)
                        if jb == 2:
                            qk_chunk(0, 0, 1, "dve", tp)   # q m0 nb1 (phase (0,1))
                    if hp == 0 and ib == 1:
                        if jb == 1:
                            qk_chunk(1, 1, 0, "dve", tq)   # k m1 nb0 (phase (1,0))
                        if jb == 3:
                            qk_chunk(0, 1, 0, "dve", tp)   # q m1 nb0 (phase (1,0))
                    if hp == 1 and ib == 0:
                        if jb == 0:
                            qk_chunk(1, 1, 1, "dve", tq)   # k m1 nb1 (dots jb4+)
                        if jb == 2:
                            qk_chunk(0, 1, 1, "dve", tp)   # q m1 nb1 (phase (1,1))

                    if prev is not None:
                        p_po2, p_hp, p_ib = prev
                        p_sc = sc2[p_ib]
                        # deferred io0/io1 ycopies EARLY in this jb slot (and
                        # before the T block, whose tiles reuse the py slots)

                        if jb in (3, 4, 5, 6):
                            ic = jb - 3
                            pool = scratch(ic)
                            tt = pool.tile([P, P], BF16, tag=pool.name,
                                           name=f"tt_{p_hp}_{p_ib}_{ic}")
                            for s in range(2):
                                nc.tensor.transpose(
                                    tt[64 * s:64 * s + 64, :],
                                    att_n[:, s, ic * 64:(ic + 1) * 64],
                                    ident_sb[:],
                                )
                            nc.vector.tensor_copy(
                                p_sc[:, p_hp, ic * 128:(ic + 1) * 128], tt[:])
                        if p_hp == 1:
                            if jb in (4, 5, 6, 7):
                                io = jb - 4
                                pyt = proj(p_sc, io, p_ib * 4 + io, "defer",
                                           scratch(io + 1))
                                deferred.append((pyt, p_ib * 4 + io))

                prev = (po2, hp, ib)

        # ---- flush: last phase's epilogue + proj(ib=1), engine-parallel -----
        # deferred io2/io3 y-copies of ib0 first: ACT is free once the exp
        # stream ends, and this keeps them off phase (1,1)'s busy DVE
        for pyt, dst_io in deferred[2:]:
            nc.scalar.copy(y_all[:, dst_io, :], pyt[:])
            nc.sync.dma_start(y[dst_io * 128:(dst_io + 1) * 128, :],
                              y_all[:, dst_io, :])
        p_po2, p_hp, p_ib = prev
        p_sc = sc2[p_ib]
        rc = rc_pool.tile([P, 2, 4], F32, tag="rc", name="rc_flush")
        att_n = attn_pool.tile([P, 2, 256], BF16, tag="attn", name="attn_flush")
        for s in range(2):
            nc.vector.reciprocal(
                rc[:, s, :],
                p_po2[s][:].rearrange("p (ic c) -> p ic c", c=128)[:, :, 64],
            )
        # ib0 io0/io1 y-copies here: behind rc in the DVE queue (so they no
        # longer delay the last TT), parallel to the ACT-side normalization
        for dio in (0, 1):
            nc.vector.tensor_copy(y_all[:, dio, :], deferred[dio][0][:])
        nc.sync.dma_start(
            y[0:256, :].rearrange("(io p) q -> p io q", p=P), y_all[:, 0:2, :])
        for ic in range(4):
            # normalization: whole ic on one engine, alternating, so each
            # transpose pair is gated by a single fast engine
            for s in range(2):
                if ic % 2 == 1:
                    nc.scalar.activation(
                        att_n[:, s, ic * 64:(ic + 1) * 64],
                        p_po2[s][:, ic * 128:ic * 128 + 64],
                        mybir.ActivationFunctionType.Copy,
                        scale=rc[:, s, ic:ic + 1],
                    )
                else:
                    nc.vector.tensor_scalar_mul(
                        att_n[:, s, ic * 64:(ic + 1) * 64],
                        p_po2[s][:, ic * 128:ic * 128 + 64],
                        rc[:, s, ic:ic + 1],
                    )
        tts = []
        for ic in range(4):
            tt = wide.tile([P, P], BF16, tag="big", name=f"tt_flush_{ic}")
            for s in range(2):
                nc.tensor.transpose(
                    tt[64 * s:64 * s + 64, :],
                    att_n[:, s, ic * 64:(ic + 1) * 64],
                    ident_sb[:],
                )
            tts.append(tt)
            # wide ring is 2-deep: copy must follow within the pair
            if ic % 2 == 1:
                for icc in (ic - 1, ic):
                    nc.vector.tensor_copy(
                        p_sc[:, p_hp, icc * 128:(icc + 1) * 128], tts[icc][:])
        for ic in range(4):
            py = scratch(ic).tile([P, 512], F32, tag=scratch(ic).name,
                                  name=f"py_flush_{ic}")
            for u in range(2):
                nc.tensor.matmul(
                    py[:],
                    p_sc[:, u, ic * 128:(ic + 1) * 128],
                    wo2_sb[:, u, :],
                    start=(u == 0),
                    stop=(u == 1),
                )
            if ic % 2 == 1:
                nc.scalar.copy(y_all[:, p_ib * 4 + ic, :], py[:])
            else:
                nc.vector.tensor_copy(y_all[:, p_ib * 4 + ic, :], py[:])
            if ic == 1:
                # pair the first two chunks: one less HWDGE slot ahead of the
                # critical last-chunk dma
                nc.sync.dma_start(
                    y[p_ib * 512:p_ib * 512 + 256, :]
                    .rearrange("(io p) q -> p io q", p=P),
                    y_all[:, p_ib * 4:p_ib * 4 + 2, :])
            elif ic > 1:
                nc.sync.dma_start(
                    y[p_ib * 512 + ic * 128:p_ib * 512 + (ic + 1) * 128, :],
                    y_all[:, p_ib * 4 + ic, :])

    nc.compile()
    return nc


def _get_nc():
    if "v2" not in _NC:
        _NC["v2"] = build_nc()
    return _NC["v2"]


def make_in_maps(x, spd, head_keep, w_qkv, w_out):
    x = np.asarray(x, np.float32)
    spd = np.asarray(spd, np.float32)
    keep = np.asarray(head_keep, np.float32)
    w_qkv = np.asarray(w_qkv, np.float32)
    w_out = np.asarray(w_out, np.float32)
    cfac = keep * (HEADS / keep.sum())
    ident = np.eye(P, dtype=ml_dtypes.bfloat16)

    in_maps = []
    for c in range(8):
        bi, hh = divmod(c, 2)
        h0 = hh * HL
        hs = slice(h0 * DIM_HEAD, (h0 + HL) * DIM_HEAD)
        xT = np.ascontiguousarray(x[bi].T.astype(ml_dtypes.bfloat16))
        q_cols = w_qkv[:, hs] * np.float32(SCALE)
        k_cols = w_qkv[:, DIM + h0 * DIM_HEAD:DIM + (h0 + HL) * DIM_HEAD]
        v_cols_h = w_qkv[:, 2 * DIM + h0 * DIM_HEAD:2 * DIM + (h0 + HL) * DIM_HEAD]
        w3 = np.ascontiguousarray(np.concatenate(
            [q_cols[:, :128], k_cols[:, :128], v_cols_h,
             q_cols[:, 128:], k_cols[:, 128:]],
            axis=1,
        ).astype(ml_dtypes.bfloat16))
        # wo2[(s,d), hp, :] = w_out row of head (h0+2hp+s), dim d, * cfac
        wo_rows = w_out[hs, :] * np.repeat(cfac[h0:h0 + HL], DIM_HEAD)[:, None]
        wo4 = wo_rows.reshape(2, 2, DIM_HEAD, DIM)      # [hp, s, d, dim]
        wo2 = np.ascontiguousarray(
            wo4.transpose(1, 2, 0, 3).reshape(2 * DIM_HEAD, 2, DIM)
            .astype(ml_dtypes.bfloat16))
        sp = spd[bi, h0:h0 + HL]  # [HL, i, j] with h = 2*hp + s
        # [hp, s, ib, ii, jb, jj] -> [hp, ib, jj, jb, s, ii]
        spdT = sp.reshape(2, 2, 2, 512, 8, 128).transpose(0, 2, 5, 4, 1, 3)
        spdT = np.exp(spdT).astype(ml_dtypes.bfloat16)
        in_maps.append({"xT": xT, "w3": w3, "wo2": wo2, "ident": ident,
                        "spdT": np.ascontiguousarray(spdT)})
    return in_maps


def kernel(x, spd, head_keep, w_qkv, w_out, b_out):
    assert x.shape == (B, N, DIM) and spd.shape == (B, HEADS, N, N)
    nc = _get_nc()
    in_maps = make_in_maps(x, spd, head_keep, w_qkv, w_out)
    res = run_bass_kernel_spmd(nc, in_maps, core_ids=list(range(8)))
    out = np.empty((B, N, DIM), np.float32)
    for bi in range(B):
        out[bi] = (res.results[2 * bi]["y"].astype(np.float32)
                   + res.results[2 * bi + 1]["y"].astype(np.float32))
    out += np.asarray(b_out, np.float32)[None, None, :]
    return out


# revision 7
# speedup vs baseline: 1.0364x; 1.0027x over previous
"""Trainium2 Bass kernel v2 for nn_Attention_spd.

Reference computation (b=4, n=1024, dim=512, heads=8, dim_head=64):
    qkv = x @ w_qkv ; q,k,v = split
    dots = q @ k^T * scale + spd
    attn = softmax(dots) * (head_keep * H / sum(head_keep))
    out  = (attn @ v) @ w_out + b_out

Sharding: core c handles batch c//2, local heads 4*(c%2)..+3 (DP x TP).
Host sums the two bf16 partial outputs per batch and adds b_out.

Design notes:
  - All DRAM traffic bf16 (x, w_qkv, w_out, exp(spd), y partials): ~10.9MB/core.
  - attn@v computed TRANSPOSED with v as the *moving* operand:
    out[i, d] tiles of [128 i, 65] cost only 65 PE columns each (v augmented
    with a ones column so col 64 accumulates the softmax denominator).
    The 8 accumulation groups (2 heads x 4 i-blocks) share two PSUM banks via
    the lazy bank-zero semantics: only the first group issues start=True
    (wiping the whole bank); the other 3 start with start=False and are
    zero-seeded by the pending-zero region.
  - Softmax normalization is a per-partition scalar op (reciprocal of col 64
    + tensor_scalar_mul -> bf16); no broadcast matmul.
  - Normalized [128 i, 64 d] tiles are PE-transposed (identity matmul) into
    [(s,d), i] layout packing the head pair on 128 partitions, so the output
    projection runs with K=128 (half the matmuls of the K=64 version).
  - ACT (exp) is the pacing engine: 32 x [128,1024] exp ops. The jb loop is
    software-pipelined: dots(jb+1) is emitted BEFORE attnv(jb) so the
    in-order PE queue never lockstep-stalls the ACT stream.
  - PSUM: wide pd ring (2x2 banks) + po2 ring (2x1) + two 1-bank scratch
    rings (tp/tq) for v/qk-chunk/transpose/proj tiles, used alternately.
  - q/k m0 computed for i-cols 0:512 first (minimal head before the first
    dots); remaining q/k chunks + v projection interleave into phase slack.
"""
import os
import sys

for _p in ("/opt/trn_rl_repo", os.path.expanduser("~/.axon_site/_ro/trn_rl_repo")):
    if os.path.isdir(_p) and _p not in sys.path:
        sys.path.insert(0, _p)

import numpy as np
import ml_dtypes

import concourse.bass as bass  # noqa: F401
import concourse.tile as tile
from concourse import bacc, mybir
from concourse.bass_utils import run_bass_kernel_spmd

P = 128
B, N, DIM = 4, 1024, 512
HEADS = 8
DIM_HEAD = 64
SCALE = DIM_HEAD ** -0.5
HL = 4          # heads per core (local)
F32 = mybir.dt.float32
BF16 = mybir.dt.bfloat16
MULT = mybir.AluOpType.mult
EXP = mybir.ActivationFunctionType.Exp

_NC = {}


def build_nc():
    nc = bacc.Bacc("TRN2", target_bir_lowering=False, debug=False, num_devices=8)
    xT = nc.dram_tensor("xT", [DIM, N], BF16, kind="ExternalInput").ap()
    # [qm0 | km0 | v | qm1 | km1] column blocks (q pre-scaled by SCALE)
    w3 = nc.dram_tensor("w3", [DIM, 3 * HL * DIM_HEAD], BF16, kind="ExternalInput").ap()
    # packed for K=128 proj: [(s,d), hp, dim]
    wo2 = nc.dram_tensor("wo2", [P, 2, DIM], BF16, kind="ExternalInput").ap()
    ident = nc.dram_tensor("ident", [P, P], BF16, kind="ExternalInput").ap()
    # exp(spd) bf16: [hp, ib, j, jb, s, i]
    spdT = nc.dram_tensor("spdT", [2, 2, P, 8, 2, 512], BF16, kind="ExternalInput").ap()
    y = nc.dram_tensor("y", [N, DIM], BF16, kind="ExternalOutput").ap()

    from contextlib import ExitStack

    with tile.TileContext(nc) as tc, ExitStack() as ctx:
        sb = ctx.enter_context(tc.tile_pool(name="sb", bufs=1))
        spd_pool = ctx.enter_context(tc.tile_pool(name="spd", bufs=4))
        ex_pool = ctx.enter_context(tc.tile_pool(name="ex", bufs=5))
        pr_pool = ctx.enter_context(tc.tile_pool(name="pr", bufs=5))
        attn_pool = ctx.enter_context(tc.tile_pool(name="attn", bufs=2))
        rc_pool = ctx.enter_context(tc.tile_pool(name="rc", bufs=2))
        sc2_pool = ctx.enter_context(tc.tile_pool(name="sc2", bufs=2))
        # PSUM: 4 (wide pd ring) + 2 (po2) + 1 (tp) + 1 (tq) = 8 banks
        wide = ctx.enter_context(tc.tile_pool(name="wide", bufs=2, space="PSUM"))
        npo = ctx.enter_context(tc.tile_pool(name="npo", bufs=2, space="PSUM"))
        tp = ctx.enter_context(tc.tile_pool(name="tp", bufs=1, space="PSUM"))
        tq = ctx.enter_context(tc.tile_pool(name="tq", bufs=1, space="PSUM"))

        def scratch(i):
            return tp if i % 2 == 0 else tq

        # ---- consts + warm-up ----------------------------------------------
        seed = sb.tile([P, 512], BF16, tag="seed")
        nc.gpsimd.memset(seed[:], 1.0)
        # PE p-state ramp: busy early so real matmuls hit full speed; the
        # warm matmuls also bridge the initial DMA wait
        warm = tp.tile([P, 512], F32, tag="tp", name="warm")
        for _ in range(5):
            nc.tensor.matmul(warm[:, :512], seed[0:1, 0:128], seed[0:1, 0:512],
                             start=True, stop=True)

        # ---- resident loads -------------------------------------------------
        xT_sb = sb.tile([P, 4, N], BF16)
        w3_sb = sb.tile([P, 4, 768], BF16, tag="w3")
        wo2_sb = sb.tile([P, 2, DIM], BF16, tag="wo2")
        ident_sb = sb.tile([P, P], BF16, tag="ident")
        xT_r = xT.rearrange("(kb p) n -> p kb n", p=P)
        w3_r = w3.rearrange("(kb p) m -> p kb m", p=P)
        # ordered so the first q/k projections + first spd tile land earliest
        nc.sync.dma_start(w3_sb[:, :, 0:256], w3_r[:, :, 0:256])      # q/k m0
        nc.sync.dma_start(xT_sb[:, 0:2, 0:512], xT_r[:, 0:2, 0:512])
        nc.sync.dma_start(xT_sb[:, 2:4, 0:512], xT_r[:, 2:4, 0:512])
        nc.sync.dma_start(w3_sb[:, :, 256:512], w3_r[:, :, 256:512])  # v
        st00 = spd_pool.tile([P, 8, 2, 512], BF16, tag="spd", name="spd_0_0")
        nc.sync.dma_start(st00[:, 0:4], spdT[0, 0, :, 0:4])
        nc.sync.dma_start(xT_sb[:, 0:2, 512:1024], xT_r[:, 0:2, 512:1024])
        nc.sync.dma_start(xT_sb[:, 2:4, 512:1024], xT_r[:, 2:4, 512:1024])
        nc.sync.dma_start(st00[:, 4:8], spdT[0, 0, :, 4:8])
        nc.sync.dma_start(w3_sb[:, :, 512:768], w3_r[:, :, 512:768])  # q/k m1
        nc.sync.dma_start(wo2_sb[:], wo2[:])
        nc.sync.dma_start(ident_sb[:], ident[:])

        qT_sb = sb.tile([P, 2, N], BF16, tag="qT")
        kT_sb = sb.tile([P, 2, N], BF16, tag="kT")
        v_aug = sb.tile([P, 8, HL * 65], BF16, tag="vaug")
        v_cols = v_aug[:].rearrange("p jb (h c) -> p jb h c", c=65)
        nc.vector.memset(v_cols[:, :, :, 64:65], 1.0)
        y_all = sb.tile([P, 8, DIM], BF16, tag="yall")

        # ---- minimal head: q/k m0 for i-cols 0:512 only ---------------------
        # qk: 0=q, 1=k; m: 0=heads 0/1, 1=heads 2/3; nb: i-col half
        def qk_chunk(qk, m, nb, copy_engine, pool):
            wofs = (512 if m else 0) + qk * 128
            dst = qT_sb if qk == 0 else kT_sb
            pq = pool.tile([P, 512], F32, tag=pool.name,
                           name=f"pq_{qk}_{m}_{nb}")
            for kb in range(4):
                nc.tensor.matmul(
                    pq[:],
                    w3_sb[:, kb, wofs:wofs + 128],
                    xT_sb[:, kb, nb * 512:(nb + 1) * 512],
                    start=(kb == 0),
                    stop=(kb == 3),
                )
            if copy_engine == "pool":
                nc.gpsimd.tensor_copy(dst[:, m, nb * 512:(nb + 1) * 512], pq[:])
            else:
                nc.vector.tensor_copy(dst[:, m, nb * 512:(nb + 1) * 512], pq[:])

        # q/k m0 nb0 with split copies so the first dots' last dependency
        # lands as early as possible
        pqq = tp.tile([P, 512], F32, tag="tp", name="pq_0_0_0")
        for kb in range(4):
            nc.tensor.matmul(
                pqq[:],
                w3_sb[:, kb, 0:128],
                xT_sb[:, kb, 0:512],
                start=(kb == 0),
                stop=(kb == 3),
            )
        nc.vector.tensor_copy(qT_sb[:, 0, 0:256], pqq[:, 0:256])
        nc.vector.tensor_copy(qT_sb[:, 0, 256:512], pqq[:, 256:512])
        pqk = tq.tile([P, 512], F32, tag="tq", name="pq_1_0_0")
        for kb in range(4):
            nc.tensor.matmul(
                pqk[:],
                w3_sb[:, kb, 128:256],
                xT_sb[:, kb, 0:512],
                start=(kb == 0),
                stop=(kb == 3),
            )
        nc.vector.tensor_copy(kT_sb[:, 0, 0:256], pqk[:, 0:256])
        nc.scalar.copy(kT_sb[:, 0, 256:512], pqk[:, 256:512])

        def v_proj(jb):
            pv = scratch(jb).tile([P, 512], F32, tag=scratch(jb).name,
                                  name=f"pv_{jb}")
            for kb in range(4):
                nc.tensor.matmul(
                    pv[:, :256],
                    xT_sb[:, kb, jb * 128:(jb + 1) * 128],
                    w3_sb[:, kb, 256:512],
                    start=(kb == 0),
                    stop=(kb == 3),
                )
            nc.vector.tensor_copy(
                v_cols[:, jb, :, :64],
                pv[:, :256].rearrange("p (h c) -> p h c", c=64),
            )

        # ---- attention ------------------------------------------------------
        sc2 = {}

        deferred = []

        def proj(sc, io, dst_io, ycopy_engine, pool):
            py = pool.tile([P, 512], F32, tag=pool.name, name=f"py_{dst_io}")
            for u in range(2):
                nc.tensor.matmul(
                    py[:],
                    sc[:, u, io * 128:(io + 1) * 128],
                    wo2_sb[:, u, :],
                    start=(u == 0),
                    stop=(u == 1),
                )
            if ycopy_engine == "act":
                nc.scalar.copy(y_all[:, dst_io, :], py[:])
            elif ycopy_engine == "dve":
                nc.vector.tensor_copy(y_all[:, dst_io, :], py[:])
            return py

        prev = None
        for hp in range(2):
            for ib in range(2):
                if ib == 0 and hp == 0:
                    st = st00
                else:
                    st = spd_pool.tile([P, 8, 2, 512], BF16, tag="spd",
                                       name=f"spd_{hp}_{ib}")
                    nc.sync.dma_start(st[:, 0:4], spdT[hp, ib, :, 0:4])
                    nc.sync.dma_start(st[:, 4:8], spdT[hp, ib, :, 4:8])
                if ib not in sc2:
                    sc2[ib] = sc2_pool.tile([P, 2, 512], BF16, tag="sc2",
                                            name=f"sc2_{ib}")

                # ---- prev phase normalization FIRST (frees its po2 slots
                # before this phase's attnv reuses the 2-slot ring) ----
                if prev is not None:
                    p_po2, p_hp, p_ib = prev
                    p_sc = sc2[p_ib]
                    rc = rc_pool.tile([P, 2, 4], F32, tag="rc",
                                      name=f"rc_{p_hp}_{p_ib}")
                    att_n = attn_pool.tile([P, 2, 256], BF16, tag="attn",
                                           name=f"attn_{p_hp}_{p_ib}")
                    for s in range(2):
                        nc.vector.reciprocal(
                            rc[:, s, :],
                            p_po2[s][:].rearrange("p (ic c) -> p ic c", c=128)[:, :, 64],
                        )
                    # normalization on DVE (GPSIMD cannot access PSUM on HW);
                    # s0 now, s1 deferred into the jb0 body (its attnv batch
                    # is a slot behind, so the burst can be spread)
                    for ic in range(4):
                        nc.vector.tensor_scalar_mul(
                            att_n[:, 0, ic * 64:(ic + 1) * 64],
                            p_po2[0][:, ic * 128:ic * 128 + 64],
                            rc[:, 0, ic:ic + 1],
                        )

                po2 = [npo.tile([P, 512], F32, tag="po", name=f"po2_{hp}_{ib}_{s}")
                       for s in range(2)]
                prs = {}

                def dots(jb):
                    pd = wide.tile([P, 1024], F32, tag="big",
                                   name=f"pd_{hp}_{ib}_{jb}")
                    for s in range(2):
                        nc.tensor.matmul(
                            pd[:, s * 512:(s + 1) * 512],
                            kT_sb[64 * s:64 * s + 64, hp, jb * 128:(jb + 1) * 128],
                            qT_sb[64 * s:64 * s + 64, hp, ib * 512:(ib + 1) * 512],
                            start=True,
                            stop=True,
                        )
                    return pd

                if hp == 0 and ib == 0:
                    # first dots split into i-halves: each sub-matmul starts
                    # as soon as its half of the q copy lands
                    pd = wide.tile([P, 1024], F32, tag="big", name="pd_0_0_0")
                    for s in range(2):
                        for ihalf in range(2):
                            nc.tensor.matmul(
                                pd[:, s * 512 + ihalf * 256:s * 512 + (ihalf + 1) * 256],
                                kT_sb[64 * s:64 * s + 64, 0, 0:128],
                                qT_sb[64 * s:64 * s + 64, 0, ihalf * 256:(ihalf + 1) * 256],
                                start=True,
                                stop=True,
                            )
                else:
                    pd = dots(0)
                for jb in range(8):
                    ex = ex_pool.tile([P, 1024], BF16, tag="ex",
                                      name=f"ex_{hp}_{ib}_{jb}")
                    pr = pr_pool.tile([P, 1024], BF16, tag="pr",
                                      name=f"pr_{hp}_{ib}_{jb}")
                    nc.scalar.activation(ex[:], pd[:], EXP)
                    nc.vector.tensor_tensor(
                        pr[:], ex[:],
                        st[:, jb].rearrange("p s i -> p (s i)"),
                        MULT,
                    )
                    if jb == 1 and prev is not None:
                        for ic in range(4):
                            nc.vector.tensor_scalar_mul(
                                att_n[:, 1, ic * 64:(ic + 1) * 64],
                                prev[0][1][:, ic * 128:ic * 128 + 64],
                                rc[:, 1, ic:ic + 1],
                            )
                    # software pipelining: next dots queued on PE BEFORE this
                    # unit's attnv (which waits on ACT+DVE)
                    if jb < 7:
                        pd = dots(jb + 1)
                    # v projection feeds attnv of phase (0,0) just in time
                    if ib == 0 and hp == 0:
                        v_proj(jb)
                    def attnv(jbx, s):
                        h = 2 * hp + s
                        for ic in range(4):
                            nc.tensor.matmul(
                                po2[s][:, ic * 128:ic * 128 + 65],
                                prs[jbx][:, s * 512 + ic * 128:s * 512 + (ic + 1) * 128],
                                v_aug[:, jbx, h * 65:(h + 1) * 65],
                                start=(jbx == 0 and ic == 0),
                                stop=(jbx == 7),
                                skip_group_check=(ic > 0),
                            )

                    prs[jb] = pr
                    attnv(jb, 0)
                    # s1 attnv one slot behind: if its pr isn't ready yet it
                    # parks in the wait queue WITHOUT blocking next-jb dots
                    if jb > 0:
                        attnv(jb - 1, 1)
                    if jb == 7:
                        attnv(7, 1)

                    # ---- interleaved work in this phase's PE slack ----
                    # phase order (hp,ib): (0,0) (0,1) (1,0) (1,1); remaining
                    # q/k chunks staged 1+ phase before their first use
                    if hp == 0 and ib == 0:
                        if jb == 1:
                            qk_chunk(1, 0, 1, "dve", tq)   # k m0 nb1 (dots jb4+)
                        if jb == 2:
                            qk_chunk(0, 0, 1, "dve", tp)   # q m0 nb1 (phase (0,1))
                    if hp == 0 and ib == 1:
                        if jb == 1:
                            qk_chunk(1, 1, 0, "dve", tq)   # k m1 nb0 (phase (1,0))
                        if jb == 3:
                            qk_chunk(0, 1, 0, "dve", tp)   # q m1 nb0 (phase (1,0))
                    if hp == 1 and ib == 0:
                        if jb == 0:
                            qk_chunk(1, 1, 1, "dve", tq)   # k m1 nb1 (dots jb4+)
                        if jb == 2:
                            qk_chunk(0, 1, 1, "dve", tp)   # q m1 nb1 (phase (1,1))

                    if prev is not None:
                        p_po2, p_hp, p_ib = prev
                        p_sc = sc2[p_ib]
                        # deferred io0/io1 ycopies EARLY in this jb slot (and
                        # before the T block, whose tiles reuse the py slots)

                        if jb in (3, 4, 5, 6):
                            ic = jb - 3
                            pool = scratch(ic)
                            tt = pool.tile([P, P], BF16, tag=pool.name,
                                           name=f"tt_{p_hp}_{p_ib}_{ic}")
                            for s in range(2):
                                nc.tensor.transpose(
                                    tt[64 * s:64 * s + 64, :],
                                    att_n[:, s, ic * 64:(ic + 1) * 64],
                                    ident_sb[:],
                                )
                            nc.vector.tensor_copy(
                                p_sc[:, p_hp, ic * 128:(ic + 1) * 128], tt[:])
                        if p_hp == 1:
                            if jb in (4, 5, 6, 7):
                                io = jb - 4
                                pyt = proj(p_sc, io, p_ib * 4 + io, "defer",
                                           scratch(io + 1))
                                deferred.append((pyt, p_ib * 4 + io))

                prev = (po2, hp, ib)

        # ---- flush: last phase's epilogue + proj(ib=1), engine-parallel -----
        # deferred io2/io3 y-copies of ib0 first: ACT is free once the exp
        # stream ends, and this keeps them off phase (1,1)'s busy DVE
        for pyt, dst_io in deferred[2:]:
            nc.scalar.copy(y_all[:, dst_io, :], pyt[:])
            nc.sync.dma_start(y[dst_io * 128:(dst_io + 1) * 128, :],
                              y_all[:, dst_io, :])
        p_po2, p_hp, p_ib = prev
        p_sc = sc2[p_ib]
        rc = rc_pool.tile([P, 2, 4], F32, tag="rc", name="rc_flush")
        att_n = attn_pool.tile([P, 2, 256], BF16, tag="attn", name="attn_flush")
        for s in range(2):
            nc.vector.reciprocal(
                rc[:, s, :],
                p_po2[s][:].rearrange("p (ic c) -> p ic c", c=128)[:, :, 64],
            )
        # ib0 io0/io1 y-copies here: behind rc in the DVE queue (so they no
        # longer delay the last TT), parallel to the ACT-side normalization
        for dio in (0, 1):
            nc.vector.tensor_copy(y_all[:, dio, :], deferred[dio][0][:])
        nc.sync.dma_start(
            y[0:256, :].rearrange("(io p) q -> p io q", p=P), y_all[:, 0:2, :])
        for ic in range(4):
            # normalization: whole ic on one engine, alternating, so each
            # transpose pair is gated by a single fast engine
            for s in range(2):
                if ic >= 2:
                    nc.scalar.activation(
                        att_n[:, s, ic * 64:(ic + 1) * 64],
                        p_po2[s][:, ic * 128:ic * 128 + 64],
                        mybir.ActivationFunctionType.Copy,
                        scale=rc[:, s, ic:ic + 1],
                    )
                else:
                    nc.vector.tensor_scalar_mul(
                        att_n[:, s, ic * 64:(ic + 1) * 64],
                        p_po2[s][:, ic * 128:ic * 128 + 64],
                        rc[:, s, ic:ic + 1],
                    )
        tts = []
        for ic in range(4):
            tt = wide.tile([P, P], BF16, tag="big", name=f"tt_flush_{ic}")
            for s in range(2):
                nc.tensor.transpose(
                    tt[64 * s:64 * s + 64, :],
                    att_n[:, s, ic * 64:(ic + 1) * 64],
                    ident_sb[:],
                )
            tts.append(tt)
            # wide ring is 2-deep: copy must follow within the pair
            if ic % 2 == 1:
                for icc in (ic - 1, ic):
                    nc.vector.tensor_copy(
                        p_sc[:, p_hp, icc * 128:(icc + 1) * 128], tts[icc][:])
        for ic in range(4):
            py = scratch(ic).tile([P, 512], F32, tag=scratch(ic).name,
                                  name=f"py_flush_{ic}")
            for u in range(2):
                nc.tensor.matmul(
                    py[:],
                    p_sc[:, u, ic * 128:(ic + 1) * 128],
                    wo2_sb[:, u, :],
                    start=(u == 0),
                    stop=(u == 1),
                )
            if ic % 2 == 1:
                nc.scalar.copy(y_all[:, p_ib * 4 + ic, :], py[:])
            else:
                nc.vector.tensor_copy(y_all[:, p_ib * 4 + ic, :], py[:])
            if ic == 1:
                # pair the first two chunks: one less HWDGE slot ahead of the
                # critical last-chunk dma
                nc.sync.dma_start(
                    y[p_ib * 512:p_ib * 512 + 256, :]
                    .rearrange("(io p) q -> p io q", p=P),
                    y_all[:, p_ib * 4:p_ib * 4 + 2, :])
            elif ic > 1:
                nc.sync.dma_start(
                    y[p_ib * 512 + ic * 128:p_ib * 512 + (ic + 1) * 128, :],
                    y_all[:, p_ib * 4 + ic, :])

    nc.compile()
    return nc


def _get_nc():
    if "v2" not in _NC:
        _NC["v2"] = build_nc()
    return _NC["v2"]


def make_in_maps(x, spd, head_keep, w_qkv, w_out):
    x = np.asarray(x, np.float32)
    spd = np.asarray(spd, np.float32)
    keep = np.asarray(head_keep, np.float32)
    w_qkv = np.asarray(w_qkv, np.float32)
    w_out = np.asarray(w_out, np.float32)
    cfac = keep * (HEADS / keep.sum())
    ident = np.eye(P, dtype=ml_dtypes.bfloat16)

    in_maps = []
    for c in range(8):
        bi, hh = divmod(c, 2)
        h0 = hh * HL
        hs = slice(h0 * DIM_HEAD, (h0 + HL) * DIM_HEAD)
        xT = np.ascontiguousarray(x[bi].T.astype(ml_dtypes.bfloat16))
        q_cols = w_qkv[:, hs] * np.float32(SCALE)
        k_cols = w_qkv[:, DIM + h0 * DIM_HEAD:DIM + (h0 + HL) * DIM_HEAD]
        v_cols_h = w_qkv[:, 2 * DIM + h0 * DIM_HEAD:2 * DIM + (h0 + HL) * DIM_HEAD]
        w3 = np.ascontiguousarray(np.concatenate(
            [q_cols[:, :128], k_cols[:, :128], v_cols_h,
             q_cols[:, 128:], k_cols[:, 128:]],
            axis=1,
        ).astype(ml_dtypes.bfloat16))
        # wo2[(s,d), hp, :] = w_out row of head (h0+2hp+s), dim d, * cfac
        wo_rows = w_out[hs, :] * np.repeat(cfac[h0:h0 + HL], DIM_HEAD)[:, None]
        wo4 = wo_rows.reshape(2, 2, DIM_HEAD, DIM)      # [hp, s, d, dim]
        wo2 = np.ascontiguousarray(
            wo4.transpose(1, 2, 0, 3).reshape(2 * DIM_HEAD, 2, DIM)
            .astype(ml_dtypes.bfloat16))
        sp = spd[bi, h0:h0 + HL]  # [HL, i, j] with h = 2*hp + s
        # [hp, s, ib, ii, jb, jj] -> [hp, ib, jj, jb, s, ii]
        spdT = sp.reshape(2, 2, 2, 512, 8, 128).transpose(0, 2, 5, 4, 1, 3)
        spdT = np.exp(spdT).astype(ml_dtypes.bfloat16)
        in_maps.append({"xT": xT, "w3": w3, "wo2": wo2, "ident": ident,
                        "spdT": np.ascontiguousarray(spdT)})
    return in_maps


def kernel(x, spd, head_keep, w_qkv, w_out, b_out):
    assert x.shape == (B, N, DIM) and spd.shape == (B, HEADS, N, N)
    nc = _get_nc()
    in_maps = make_in_maps(x, spd, head_keep, w_qkv, w_out)
    res = run_bass_kernel_spmd(nc, in_maps, core_ids=list(range(8)))
    out = np.empty((B, N, DIM), np.float32)
    for bi in range(B):
        out[bi] = (res.results[2 * bi]["y"].astype(np.float32)
                   + res.results[2 * bi + 1]["y"].astype(np.float32))
    out += np.asarray(b_out, np.float32)[None, None, :]
    return out


# revision 8
# speedup vs baseline: 1.0380x; 1.0016x over previous
"""Trainium2 Bass kernel v2 for nn_Attention_spd.

Reference computation (b=4, n=1024, dim=512, heads=8, dim_head=64):
    qkv = x @ w_qkv ; q,k,v = split
    dots = q @ k^T * scale + spd
    attn = softmax(dots) * (head_keep * H / sum(head_keep))
    out  = (attn @ v) @ w_out + b_out

Sharding: core c handles batch c//2, local heads 4*(c%2)..+3 (DP x TP).
Host sums the two bf16 partial outputs per batch and adds b_out.

Design notes:
  - All DRAM traffic bf16 (x, w_qkv, w_out, exp(spd), y partials): ~10.9MB/core.
  - attn@v computed TRANSPOSED with v as the *moving* operand:
    out[i, d] tiles of [128 i, 65] cost only 65 PE columns each (v augmented
    with a ones column so col 64 accumulates the softmax denominator).
    The 8 accumulation groups (2 heads x 4 i-blocks) share two PSUM banks via
    the lazy bank-zero semantics: only the first group issues start=True
    (wiping the whole bank); the other 3 start with start=False and are
    zero-seeded by the pending-zero region.
  - Softmax normalization is a per-partition scalar op (reciprocal of col 64
    + tensor_scalar_mul -> bf16); no broadcast matmul.
  - Normalized [128 i, 64 d] tiles are PE-transposed (identity matmul) into
    [(s,d), i] layout packing the head pair on 128 partitions, so the output
    projection runs with K=128 (half the matmuls of the K=64 version).
  - ACT (exp) is the pacing engine: 32 x [128,1024] exp ops. The jb loop is
    software-pipelined: dots(jb+1) is emitted BEFORE attnv(jb) so the
    in-order PE queue never lockstep-stalls the ACT stream.
  - PSUM: wide pd ring (2x2 banks) + po2 ring (2x1) + two 1-bank scratch
    rings (tp/tq) for v/qk-chunk/transpose/proj tiles, used alternately.
  - q/k m0 computed for i-cols 0:512 first (minimal head before the first
    dots); remaining q/k chunks + v projection interleave into phase slack.
"""
import os
import sys

for _p in ("/opt/trn_rl_repo", os.path.expanduser("~/.axon_site/_ro/trn_rl_repo")):
    if os.path.isdir(_p) and _p not in sys.path:
        sys.path.insert(0, _p)

import numpy as np
import ml_dtypes

import concourse.bass as bass  # noqa: F401
import concourse.tile as tile
from concourse import bacc, mybir
from concourse.bass_utils import run_bass_kernel_spmd

P = 128
B, N, DIM = 4, 1024, 512
HEADS = 8
DIM_HEAD = 64
SCALE = DIM_HEAD ** -0.5
HL = 4          # heads per core (local)
F32 = mybir.dt.float32
BF16 = mybir.dt.bfloat16
MULT = mybir.AluOpType.mult
EXP = mybir.ActivationFunctionType.Exp

_NC = {}


def build_nc():
    nc = bacc.Bacc("TRN2", target_bir_lowering=False, debug=False, num_devices=8)
    xT = nc.dram_tensor("xT", [DIM, N], BF16, kind="ExternalInput").ap()
    # [qm0 | km0 | v | qm1 | km1] column blocks (q pre-scaled by SCALE)
    w3 = nc.dram_tensor("w3", [DIM, 3 * HL * DIM_HEAD], BF16, kind="ExternalInput").ap()
    # packed for K=128 proj: [(s,d), hp, dim]
    wo2 = nc.dram_tensor("wo2", [P, 2, DIM], BF16, kind="ExternalInput").ap()
    ident = nc.dram_tensor("ident", [P, P], BF16, kind="ExternalInput").ap()
    # exp(spd) bf16: [hp, ib, j, jb, s, i]
    spdT = nc.dram_tensor("spdT", [2, 2, P, 8, 2, 512], BF16, kind="ExternalInput").ap()
    y = nc.dram_tensor("y", [N, DIM], BF16, kind="ExternalOutput").ap()

    from contextlib import ExitStack

    with tile.TileContext(nc) as tc, ExitStack() as ctx:
        sb = ctx.enter_context(tc.tile_pool(name="sb", bufs=1))
        spd_pool = ctx.enter_context(tc.tile_pool(name="spd", bufs=4))
        ex_pool = ctx.enter_context(tc.tile_pool(name="ex", bufs=5))
        pr_pool = ctx.enter_context(tc.tile_pool(name="pr", bufs=5))
        attn_pool = ctx.enter_context(tc.tile_pool(name="attn", bufs=2))
        rc_pool = ctx.enter_context(tc.tile_pool(name="rc", bufs=2))
        sc2_pool = ctx.enter_context(tc.tile_pool(name="sc2", bufs=2))
        # PSUM: 4 (wide pd ring) + 2 (po2) + 1 (tp) + 1 (tq) = 8 banks
        wide = ctx.enter_context(tc.tile_pool(name="wide", bufs=2, space="PSUM"))
        npo = ctx.enter_context(tc.tile_pool(name="npo", bufs=2, space="PSUM"))
        tp = ctx.enter_context(tc.tile_pool(name="tp", bufs=1, space="PSUM"))
        tq = ctx.enter_context(tc.tile_pool(name="tq", bufs=1, space="PSUM"))

        def scratch(i):
            return tp if i % 2 == 0 else tq

        # ---- consts + warm-up ----------------------------------------------
        seed = sb.tile([P, 512], BF16, tag="seed")
        nc.gpsimd.memset(seed[:], 1.0)
        # PE p-state ramp: busy early so real matmuls hit full speed; the
        # warm matmuls also bridge the initial DMA wait
        warm = tp.tile([P, 512], F32, tag="tp", name="warm")
        for _ in range(5):
            nc.tensor.matmul(warm[:, :512], seed[0:1, 0:128], seed[0:1, 0:512],
                             start=True, stop=True)

        # ---- resident loads -------------------------------------------------
        xT_sb = sb.tile([P, 4, N], BF16)
        w3_sb = sb.tile([P, 4, 768], BF16, tag="w3")
        wo2_sb = sb.tile([P, 2, DIM], BF16, tag="wo2")
        ident_sb = sb.tile([P, P], BF16, tag="ident")
        xT_r = xT.rearrange("(kb p) n -> p kb n", p=P)
        w3_r = w3.rearrange("(kb p) m -> p kb m", p=P)
        # ordered so the first q/k projections + first spd tile land earliest
        nc.sync.dma_start(w3_sb[:, :, 0:256], w3_r[:, :, 0:256])      # q/k m0
        nc.sync.dma_start(xT_sb[:, 0:2, 0:512], xT_r[:, 0:2, 0:512])
        nc.sync.dma_start(xT_sb[:, 2:4, 0:512], xT_r[:, 2:4, 0:512])
        nc.sync.dma_start(w3_sb[:, :, 256:512], w3_r[:, :, 256:512])  # v
        st00 = spd_pool.tile([P, 8, 2, 512], BF16, tag="spd", name="spd_0_0")
        nc.sync.dma_start(st00[:, 0:4], spdT[0, 0, :, 0:4])
        nc.sync.dma_start(xT_sb[:, 0:2, 512:1024], xT_r[:, 0:2, 512:1024])
        nc.sync.dma_start(xT_sb[:, 2:4, 512:1024], xT_r[:, 2:4, 512:1024])
        nc.sync.dma_start(st00[:, 4:8], spdT[0, 0, :, 4:8])
        nc.sync.dma_start(w3_sb[:, :, 512:768], w3_r[:, :, 512:768])  # q/k m1
        nc.sync.dma_start(wo2_sb[:], wo2[:])
        nc.sync.dma_start(ident_sb[:], ident[:])

        qT_sb = sb.tile([P, 2, N], BF16, tag="qT")
        kT_sb = sb.tile([P, 2, N], BF16, tag="kT")
        v_aug = sb.tile([P, 8, HL * 65], BF16, tag="vaug")
        v_cols = v_aug[:].rearrange("p jb (h c) -> p jb h c", c=65)
        nc.vector.memset(v_cols[:, :, :, 64:65], 1.0)
        y_all = sb.tile([P, 8, DIM], BF16, tag="yall")

        # ---- minimal head: q/k m0 for i-cols 0:512 only ---------------------
        # qk: 0=q, 1=k; m: 0=heads 0/1, 1=heads 2/3; nb: i-col half
        def qk_chunk(qk, m, nb, copy_engine, pool):
            wofs = (512 if m else 0) + qk * 128
            dst = qT_sb if qk == 0 else kT_sb
            pq = pool.tile([P, 512], F32, tag=pool.name,
                           name=f"pq_{qk}_{m}_{nb}")
            for kb in range(4):
                nc.tensor.matmul(
                    pq[:],
                    w3_sb[:, kb, wofs:wofs + 128],
                    xT_sb[:, kb, nb * 512:(nb + 1) * 512],
                    start=(kb == 0),
                    stop=(kb == 3),
                )
            if copy_engine == "pool":
                nc.gpsimd.tensor_copy(dst[:, m, nb * 512:(nb + 1) * 512], pq[:])
            else:
                nc.vector.tensor_copy(dst[:, m, nb * 512:(nb + 1) * 512], pq[:])

        # q/k m0 nb0 with split copies so the first dots' last dependency
        # lands as early as possible
        pqq = tp.tile([P, 512], F32, tag="tp", name="pq_0_0_0")
        for kb in range(4):
            nc.tensor.matmul(
                pqq[:],
                w3_sb[:, kb, 0:128],
                xT_sb[:, kb, 0:512],
                start=(kb == 0),
                stop=(kb == 3),
            )
        nc.vector.tensor_copy(qT_sb[:, 0, 0:256], pqq[:, 0:256])
        nc.vector.tensor_copy(qT_sb[:, 0, 256:512], pqq[:, 256:512])
        pqk = tq.tile([P, 512], F32, tag="tq", name="pq_1_0_0")
        for kb in range(4):
            nc.tensor.matmul(
                pqk[:],
                w3_sb[:, kb, 128:256],
                xT_sb[:, kb, 0:512],
                start=(kb == 0),
                stop=(kb == 3),
            )
        nc.vector.tensor_copy(kT_sb[:, 0, 0:256], pqk[:, 0:256])
        nc.scalar.copy(kT_sb[:, 0, 256:512], pqk[:, 256:512])

        def v_proj(jb):
            pv = scratch(jb).tile([P, 512], F32, tag=scratch(jb).name,
                                  name=f"pv_{jb}")
            for kb in range(4):
                nc.tensor.matmul(
                    pv[:, :256],
                    xT_sb[:, kb, jb * 128:(jb + 1) * 128],
                    w3_sb[:, kb, 256:512],
                    start=(kb == 0),
                    stop=(kb == 3),
                )
            nc.vector.tensor_copy(
                v_cols[:, jb, :, :64],
                pv[:, :256].rearrange("p (h c) -> p h c", c=64),
            )

        # ---- attention ------------------------------------------------------
        sc2 = {}

        deferred = []

        def proj(sc, io, dst_io, ycopy_engine, pool):
            py = pool.tile([P, 512], F32, tag=pool.name, name=f"py_{dst_io}")
            for u in range(2):
                nc.tensor.matmul(
                    py[:],
                    sc[:, u, io * 128:(io + 1) * 128],
                    wo2_sb[:, u, :],
                    start=(u == 0),
                    stop=(u == 1),
                )
            if ycopy_engine == "act":
                nc.scalar.copy(y_all[:, dst_io, :], py[:])
            elif ycopy_engine == "dve":
                nc.vector.tensor_copy(y_all[:, dst_io, :], py[:])
            return py

        prev = None
        for hp in range(2):
            for ib in range(2):
                if ib == 0 and hp == 0:
                    st = st00
                else:
                    st = spd_pool.tile([P, 8, 2, 512], BF16, tag="spd",
                                       name=f"spd_{hp}_{ib}")
                    nc.sync.dma_start(st[:, 0:4], spdT[hp, ib, :, 0:4])
                    nc.sync.dma_start(st[:, 4:8], spdT[hp, ib, :, 4:8])
                if ib not in sc2:
                    sc2[ib] = sc2_pool.tile([P, 2, 512], BF16, tag="sc2",
                                            name=f"sc2_{ib}")

                # ---- prev phase normalization FIRST (frees its po2 slots
                # before this phase's attnv reuses the 2-slot ring) ----
                if prev is not None:
                    p_po2, p_hp, p_ib = prev
                    p_sc = sc2[p_ib]
                    rc = rc_pool.tile([P, 2, 4], F32, tag="rc",
                                      name=f"rc_{p_hp}_{p_ib}")
                    att_n = attn_pool.tile([P, 2, 256], BF16, tag="attn",
                                           name=f"attn_{p_hp}_{p_ib}")
                    for s in range(2):
                        nc.vector.reciprocal(
                            rc[:, s, :],
                            p_po2[s][:].rearrange("p (ic c) -> p ic c", c=128)[:, :, 64],
                        )
                    # normalization on DVE (GPSIMD cannot access PSUM on HW);
                    # s0 now, s1 deferred into the jb0 body (its attnv batch
                    # is a slot behind, so the burst can be spread)
                    for ic in range(4):
                        nc.vector.tensor_scalar_mul(
                            att_n[:, 0, ic * 64:(ic + 1) * 64],
                            p_po2[0][:, ic * 128:ic * 128 + 64],
                            rc[:, 0, ic:ic + 1],
                        )

                po2 = [npo.tile([P, 512], F32, tag="po", name=f"po2_{hp}_{ib}_{s}")
                       for s in range(2)]
                prs = {}

                def dots(jb):
                    pd = wide.tile([P, 1024], F32, tag="big",
                                   name=f"pd_{hp}_{ib}_{jb}")
                    for s in range(2):
                        nc.tensor.matmul(
                            pd[:, s * 512:(s + 1) * 512],
                            kT_sb[64 * s:64 * s + 64, hp, jb * 128:(jb + 1) * 128],
                            qT_sb[64 * s:64 * s + 64, hp, ib * 512:(ib + 1) * 512],
                            start=True,
                            stop=True,
                        )
                    return pd

                if hp == 0 and ib == 0:
                    # first dots split into i-halves: each sub-matmul starts
                    # as soon as its half of the q copy lands
                    pd = wide.tile([P, 1024], F32, tag="big", name="pd_0_0_0")
                    for s in range(2):
                        for ihalf in range(2):
                            nc.tensor.matmul(
                                pd[:, s * 512 + ihalf * 256:s * 512 + (ihalf + 1) * 256],
                                kT_sb[64 * s:64 * s + 64, 0, 0:128],
                                qT_sb[64 * s:64 * s + 64, 0, ihalf * 256:(ihalf + 1) * 256],
                                start=True,
                                stop=True,
                            )
                else:
                    pd = dots(0)
                for jb in range(8):
                    ex = ex_pool.tile([P, 1024], BF16, tag="ex",
                                      name=f"ex_{hp}_{ib}_{jb}")
                    pr = pr_pool.tile([P, 1024], BF16, tag="pr",
                                      name=f"pr_{hp}_{ib}_{jb}")
                    nc.scalar.activation(ex[:], pd[:], EXP)
                    nc.vector.tensor_tensor(
                        pr[:], ex[:],
                        st[:, jb].rearrange("p s i -> p (s i)"),
                        MULT,
                    )
                    if jb == 1 and prev is not None:
                        for ic in range(4):
                            nc.vector.tensor_scalar_mul(
                                att_n[:, 1, ic * 64:(ic + 1) * 64],
                                prev[0][1][:, ic * 128:ic * 128 + 64],
                                rc[:, 1, ic:ic + 1],
                            )
                    # software pipelining: next dots queued on PE BEFORE this
                    # unit's attnv (which waits on ACT+DVE)
                    if jb < 7:
                        pd = dots(jb + 1)
                    # v projection feeds attnv of phase (0,0) just in time
                    if ib == 0 and hp == 0:
                        v_proj(jb)
                    def attnv(jbx, s):
                        h = 2 * hp + s
                        for ic in range(4):
                            nc.tensor.matmul(
                                po2[s][:, ic * 128:ic * 128 + 65],
                                prs[jbx][:, s * 512 + ic * 128:s * 512 + (ic + 1) * 128],
                                v_aug[:, jbx, h * 65:(h + 1) * 65],
                                start=(jbx == 0 and ic == 0),
                                stop=(jbx == 7),
                                skip_group_check=(ic > 0),
                            )

                    prs[jb] = pr
                    attnv(jb, 0)
                    # s1 attnv one slot behind: if its pr isn't ready yet it
                    # parks in the wait queue WITHOUT blocking next-jb dots
                    if jb > 0:
                        attnv(jb - 1, 1)
                    if jb == 7:
                        attnv(7, 1)

                    # ---- interleaved work in this phase's PE slack ----
                    # phase order (hp,ib): (0,0) (0,1) (1,0) (1,1); remaining
                    # q/k chunks staged 1+ phase before their first use
                    if hp == 0 and ib == 0:
                        if jb == 1:
                            qk_chunk(1, 0, 1, "dve", tq)   # k m0 nb1 (dots jb4+)
                        if jb == 2:
                            qk_chunk(0, 0, 1, "dve", tp)   # q m0 nb1 (phase (0,1))
                    if hp == 0 and ib == 1:
                        if jb == 1:
                            qk_chunk(1, 1, 0, "dve", tq)   # k m1 nb0 (phase (1,0))
                        if jb == 3:
                            qk_chunk(0, 1, 0, "dve", tp)   # q m1 nb0 (phase (1,0))
                    if hp == 1 and ib == 0:
                        if jb == 0:
                            qk_chunk(1, 1, 1, "dve", tq)   # k m1 nb1 (dots jb4+)
                        if jb == 2:
                            qk_chunk(0, 1, 1, "dve", tp)   # q m1 nb1 (phase (1,1))

                    if prev is not None:
                        p_po2, p_hp, p_ib = prev
                        p_sc = sc2[p_ib]
                        # deferred io0/io1 ycopies EARLY in this jb slot (and
                        # before the T block, whose tiles reuse the py slots)

                        if jb in (3, 4, 5, 6):
                            ic = jb - 3
                            pool = scratch(ic)
                            tt = pool.tile([P, P], BF16, tag=pool.name,
                                           name=f"tt_{p_hp}_{p_ib}_{ic}")
                            for s in range(2):
                                nc.tensor.transpose(
                                    tt[64 * s:64 * s + 64, :],
                                    att_n[:, s, ic * 64:(ic + 1) * 64],
                                    ident_sb[:],
                                )
                            nc.vector.tensor_copy(
                                p_sc[:, p_hp, ic * 128:(ic + 1) * 128], tt[:])
                        if p_hp == 1:
                            if jb in (4, 5, 6, 7):
                                io = jb - 4
                                pyt = proj(p_sc, io, p_ib * 4 + io, "defer",
                                           scratch(io + 1))
                                deferred.append((pyt, p_ib * 4 + io))

                prev = (po2, hp, ib)

        # ---- flush: last phase's epilogue + proj(ib=1), engine-parallel -----
        # deferred io2/io3 y-copies of ib0 first: ACT is free once the exp
        # stream ends, and this keeps them off phase (1,1)'s busy DVE
        for pyt, dst_io in deferred[2:]:
            nc.scalar.copy(y_all[:, dst_io, :], pyt[:])
            nc.sync.dma_start(y[dst_io * 128:(dst_io + 1) * 128, :],
                              y_all[:, dst_io, :])
        p_po2, p_hp, p_ib = prev
        p_sc = sc2[p_ib]
        rc = rc_pool.tile([P, 2, 4], F32, tag="rc", name="rc_flush")
        att_n = attn_pool.tile([P, 2, 256], BF16, tag="attn", name="attn_flush")
        for s in range(2):
            nc.vector.reciprocal(
                rc[:, s, :],
                p_po2[s][:].rearrange("p (ic c) -> p ic c", c=128)[:, :, 64],
            )

        for ic in range(4):
            # normalization: whole ic on one engine, alternating, so each
            # transpose pair is gated by a single fast engine
            for s in range(2):
                if ic >= 2:
                    nc.scalar.activation(
                        att_n[:, s, ic * 64:(ic + 1) * 64],
                        p_po2[s][:, ic * 128:ic * 128 + 64],
                        mybir.ActivationFunctionType.Copy,
                        scale=rc[:, s, ic:ic + 1],
                    )
                else:
                    nc.vector.tensor_scalar_mul(
                        att_n[:, s, ic * 64:(ic + 1) * 64],
                        p_po2[s][:, ic * 128:ic * 128 + 64],
                        rc[:, s, ic:ic + 1],
                    )
        tts = []
        for ic in range(4):
            tt = wide.tile([P, P], BF16, tag="big", name=f"tt_flush_{ic}")
            for s in range(2):
                nc.tensor.transpose(
                    tt[64 * s:64 * s + 64, :],
                    att_n[:, s, ic * 64:(ic + 1) * 64],
                    ident_sb[:],
                )
            tts.append(tt)
            # wide ring is 2-deep: copy must follow within the pair
            if ic % 2 == 1:
                for icc in (ic - 1, ic):
                    nc.vector.tensor_copy(
                        p_sc[:, p_hp, icc * 128:(icc + 1) * 128], tts[icc][:])
        # ib0 io0/io1 y-copies on ACT: its flush idle window absorbs them,
        # keeping DVE clear for the last TT multiply and the sc2 drains
        for dio in (0, 1):
            nc.scalar.copy(y_all[:, dio, :], deferred[dio][0][:])
        nc.sync.dma_start(
            y[0:256, :].rearrange("(io p) q -> p io q", p=P), y_all[:, 0:2, :])
        for ic in range(4):
            py = scratch(ic).tile([P, 512], F32, tag=scratch(ic).name,
                                  name=f"py_flush_{ic}")
            for u in range(2):
                nc.tensor.matmul(
                    py[:],
                    p_sc[:, u, ic * 128:(ic + 1) * 128],
                    wo2_sb[:, u, :],
                    start=(u == 0),
                    stop=(u == 1),
                )
            if ic % 2 == 1:
                nc.scalar.copy(y_all[:, p_ib * 4 + ic, :], py[:])
            else:
                nc.vector.tensor_copy(y_all[:, p_ib * 4 + ic, :], py[:])
            if ic == 1:
                # pair the first two chunks: one less HWDGE slot ahead of the
                # critical last-chunk dma
                nc.sync.dma_start(
                    y[p_ib * 512:p_ib * 512 + 256, :]
                    .rearrange("(io p) q -> p io q", p=P),
                    y_all[:, p_ib * 4:p_ib * 4 + 2, :])
            elif ic > 1:
                nc.sync.dma_start(
                    y[p_ib * 512 + ic * 128:p_ib * 512 + (ic + 1) * 128, :],
                    y_all[:, p_ib * 4 + ic, :])

    nc.compile()
    return nc


def _get_nc():
    if "v2" not in _NC:
        _NC["v2"] = build_nc()
    return _NC["v2"]


def make_in_maps(x, spd, head_keep, w_qkv, w_out):
    x = np.asarray(x, np.float32)
    spd = np.asarray(spd, np.float32)
    keep = np.asarray(head_keep, np.float32)
    w_qkv = np.asarray(w_qkv, np.float32)
    w_out = np.asarray(w_out, np.float32)
    cfac = keep * (HEADS / keep.sum())
    ident = np.eye(P, dtype=ml_dtypes.bfloat16)

    in_maps = []
    for c in range(8):
        bi, hh = divmod(c, 2)
        h0 = hh * HL
        hs = slice(h0 * DIM_HEAD, (h0 + HL) * DIM_HEAD)
        xT = np.ascontiguousarray(x[bi].T.astype(ml_dtypes.bfloat16))
        q_cols = w_qkv[:, hs] * np.float32(SCALE)
        k_cols = w_qkv[:, DIM + h0 * DIM_HEAD:DIM + (h0 + HL) * DIM_HEAD]
        v_cols_h = w_qkv[:, 2 * DIM + h0 * DIM_HEAD:2 * DIM + (h0 + HL) * DIM_HEAD]
        w3 = np.ascontiguousarray(np.concatenate(
            [q_cols[:, :128], k_cols[:, :128], v_cols_h,
             q_cols[:, 128:], k_cols[:, 128:]],
            axis=1,
        ).astype(ml_dtypes.bfloat16))
        # wo2[(s,d), hp, :] = w_out row of head (h0+2hp+s), dim d, * cfac
        wo_rows = w_out[hs, :] * np.repeat(cfac[h0:h0 + HL], DIM_HEAD)[:, None]
        wo4 = wo_rows.reshape(2, 2, DIM_HEAD, DIM)      # [hp, s, d, dim]
        wo2 = np.ascontiguousarray(
            wo4.transpose(1, 2, 0, 3).reshape(2 * DIM_HEAD, 2, DIM)
            .astype(ml_dtypes.bfloat16))
        sp = spd[bi, h0:h0 + HL]  # [HL, i, j] with h = 2*hp + s
        # [hp, s, ib, ii, jb, jj] -> [hp, ib, jj, jb, s, ii]
        spdT = sp.reshape(2, 2, 2, 512, 8, 128).transpose(0, 2, 5, 4, 1, 3)
        spdT = np.exp(spdT).astype(ml_dtypes.bfloat16)
        in_maps.append({"xT": xT, "w3": w3, "wo2": wo2, "ident": ident,
                        "spdT": np.ascontiguousarray(spdT)})
    return in_maps


def kernel(x, spd, head_keep, w_qkv, w_out, b_out):
    assert x.shape == (B, N, DIM) and spd.shape == (B, HEADS, N, N)
    nc = _get_nc()
    in_maps = make_in_maps(x, spd, head_keep, w_qkv, w_out)
    res = run_bass_kernel_spmd(nc, in_maps, core_ids=list(range(8)))
    out = np.empty((B, N, DIM), np.float32)
    for bi in range(B):
        out[bi] = (res.results[2 * bi]["y"].astype(np.float32)
                   + res.results[2 * bi + 1]["y"].astype(np.float32))
    out += np.asarray(b_out, np.float32)[None, None, :]
    return out


# revision 9
# speedup vs baseline: 1.0412x; 1.0030x over previous
"""Trainium2 Bass kernel v2 for nn_Attention_spd.

Reference computation (b=4, n=1024, dim=512, heads=8, dim_head=64):
    qkv = x @ w_qkv ; q,k,v = split
    dots = q @ k^T * scale + spd
    attn = softmax(dots) * (head_keep * H / sum(head_keep))
    out  = (attn @ v) @ w_out + b_out

Sharding: core c handles batch c//2, local heads 4*(c%2)..+3 (DP x TP).
Host sums the two bf16 partial outputs per batch and adds b_out.

Design notes:
  - All DRAM traffic bf16 (x, w_qkv, w_out, exp(spd), y partials): ~10.9MB/core.
  - attn@v computed TRANSPOSED with v as the *moving* operand:
    out[i, d] tiles of [128 i, 65] cost only 65 PE columns each (v augmented
    with a ones column so col 64 accumulates the softmax denominator).
    The 8 accumulation groups (2 heads x 4 i-blocks) share two PSUM banks via
    the lazy bank-zero semantics: only the first group issues start=True
    (wiping the whole bank); the other 3 start with start=False and are
    zero-seeded by the pending-zero region.
  - Softmax normalization is a per-partition scalar op (reciprocal of col 64
    + tensor_scalar_mul -> bf16); no broadcast matmul.
  - Normalized [128 i, 64 d] tiles are PE-transposed (identity matmul) into
    [(s,d), i] layout packing the head pair on 128 partitions, so the output
    projection runs with K=128 (half the matmuls of the K=64 version).
  - ACT (exp) is the pacing engine: 32 x [128,1024] exp ops. The jb loop is
    software-pipelined: dots(jb+1) is emitted BEFORE attnv(jb) so the
    in-order PE queue never lockstep-stalls the ACT stream.
  - PSUM: wide pd ring (2x2 banks) + po2 ring (2x1) + two 1-bank scratch
    rings (tp/tq) for v/qk-chunk/transpose/proj tiles, used alternately.
  - q/k m0 computed for i-cols 0:512 first (minimal head before the first
    dots); remaining q/k chunks + v projection interleave into phase slack.
"""
import os
import sys

for _p in ("/opt/trn_rl_repo", os.path.expanduser("~/.axon_site/_ro/trn_rl_repo")):
    if os.path.isdir(_p) and _p not in sys.path:
        sys.path.insert(0, _p)

import numpy as np
import ml_dtypes

import concourse.bass as bass  # noqa: F401
import concourse.tile as tile
from concourse import bacc, mybir
from concourse.bass_utils import run_bass_kernel_spmd

P = 128
B, N, DIM = 4, 1024, 512
HEADS = 8
DIM_HEAD = 64
SCALE = DIM_HEAD ** -0.5
HL = 4          # heads per core (local)
F32 = mybir.dt.float32
BF16 = mybir.dt.bfloat16
MULT = mybir.AluOpType.mult
EXP = mybir.ActivationFunctionType.Exp

_NC = {}


def build_nc():
    nc = bacc.Bacc("TRN2", target_bir_lowering=False, debug=False, num_devices=8)
    xT = nc.dram_tensor("xT", [DIM, N], BF16, kind="ExternalInput").ap()
    # [qm0 | km0 | v | qm1 | km1] column blocks (q pre-scaled by SCALE)
    w3 = nc.dram_tensor("w3", [DIM, 3 * HL * DIM_HEAD], BF16, kind="ExternalInput").ap()
    # packed for K=128 proj: [(s,d), hp, dim]
    wo2 = nc.dram_tensor("wo2", [P, 2, DIM], BF16, kind="ExternalInput").ap()
    ident = nc.dram_tensor("ident", [P, P], BF16, kind="ExternalInput").ap()
    # exp(spd) bf16: [hp, ib, j, jb, s, i]
    spdT = nc.dram_tensor("spdT", [2, 2, P, 8, 2, 512], BF16, kind="ExternalInput").ap()
    y = nc.dram_tensor("y", [N, DIM], BF16, kind="ExternalOutput").ap()

    from contextlib import ExitStack

    with tile.TileContext(nc) as tc, ExitStack() as ctx:
        sb = ctx.enter_context(tc.tile_pool(name="sb", bufs=1))
        spd_pool = ctx.enter_context(tc.tile_pool(name="spd", bufs=4))
        ex_pool = ctx.enter_context(tc.tile_pool(name="ex", bufs=5))
        pr_pool = ctx.enter_context(tc.tile_pool(name="pr", bufs=5))
        attn_pool = ctx.enter_context(tc.tile_pool(name="attn", bufs=2))
        rc_pool = ctx.enter_context(tc.tile_pool(name="rc", bufs=2))
        sc2_pool = ctx.enter_context(tc.tile_pool(name="sc2", bufs=2))
        # PSUM: 4 (wide pd ring) + 2 (po2) + 1 (tp) + 1 (tq) = 8 banks
        wide = ctx.enter_context(tc.tile_pool(name="wide", bufs=2, space="PSUM"))
        npo = ctx.enter_context(tc.tile_pool(name="npo", bufs=2, space="PSUM"))
        tp = ctx.enter_context(tc.tile_pool(name="tp", bufs=1, space="PSUM"))
        tq = ctx.enter_context(tc.tile_pool(name="tq", bufs=1, space="PSUM"))

        def scratch(i):
            return tp if i % 2 == 0 else tq

        # ---- consts + warm-up ----------------------------------------------
        seed = sb.tile([P, 512], BF16, tag="seed")
        nc.gpsimd.memset(seed[:], 1.0)
        # dummy exp: forces the ACT table load to run at program start
        # instead of blocking the first real exp
        nc.scalar.activation(seed[0:1, 0:1], seed[0:1, 0:1], EXP)
        # PE p-state ramp: busy early so real matmuls hit full speed; the
        # warm matmuls also bridge the initial DMA wait
        warm = tp.tile([P, 512], F32, tag="tp", name="warm")
        for _ in range(5):
            nc.tensor.matmul(warm[:, :512], seed[0:1, 0:128], seed[0:1, 0:512],
                             start=True, stop=True)

        # ---- resident loads -------------------------------------------------
        xT_sb = sb.tile([P, 4, N], BF16)
        w3_sb = sb.tile([P, 4, 768], BF16, tag="w3")
        wo2_sb = sb.tile([P, 2, DIM], BF16, tag="wo2")
        ident_sb = sb.tile([P, P], BF16, tag="ident")
        xT_r = xT.rearrange("(kb p) n -> p kb n", p=P)
        w3_r = w3.rearrange("(kb p) m -> p kb m", p=P)
        # ordered so the first q/k projections + first spd tile land earliest
        nc.sync.dma_start(w3_sb[:, :, 0:256], w3_r[:, :, 0:256])      # q/k m0
        nc.sync.dma_start(xT_sb[:, 0:2, 0:512], xT_r[:, 0:2, 0:512])
        nc.sync.dma_start(xT_sb[:, 2:4, 0:512], xT_r[:, 2:4, 0:512])
        nc.sync.dma_start(w3_sb[:, :, 256:512], w3_r[:, :, 256:512])  # v
        st00 = spd_pool.tile([P, 8, 2, 512], BF16, tag="spd", name="spd_0_0")
        nc.sync.dma_start(st00[:, 0:4], spdT[0, 0, :, 0:4])
        nc.sync.dma_start(xT_sb[:, 0:2, 512:1024], xT_r[:, 0:2, 512:1024])
        nc.sync.dma_start(xT_sb[:, 2:4, 512:1024], xT_r[:, 2:4, 512:1024])
        nc.sync.dma_start(st00[:, 4:8], spdT[0, 0, :, 4:8])
        nc.sync.dma_start(w3_sb[:, :, 512:768], w3_r[:, :, 512:768])  # q/k m1
        nc.sync.dma_start(wo2_sb[:], wo2[:])
        nc.sync.dma_start(ident_sb[:], ident[:])

        qT_sb = sb.tile([P, 2, N], BF16, tag="qT")
        kT_sb = sb.tile([P, 2, N], BF16, tag="kT")
        v_aug = sb.tile([P, 8, HL * 65], BF16, tag="vaug")
        v_cols = v_aug[:].rearrange("p jb (h c) -> p jb h c", c=65)
        nc.vector.memset(v_cols[:, :, :, 64:65], 1.0)
        y_all = sb.tile([P, 8, DIM], BF16, tag="yall")

        # ---- minimal head: q/k m0 for i-cols 0:512 only ---------------------
        # qk: 0=q, 1=k; m: 0=heads 0/1, 1=heads 2/3; nb: i-col half
        def qk_chunk(qk, m, nb, copy_engine, pool):
            wofs = (512 if m else 0) + qk * 128
            dst = qT_sb if qk == 0 else kT_sb
            pq = pool.tile([P, 512], F32, tag=pool.name,
                           name=f"pq_{qk}_{m}_{nb}")
            for kb in range(4):
                nc.tensor.matmul(
                    pq[:],
                    w3_sb[:, kb, wofs:wofs + 128],
                    xT_sb[:, kb, nb * 512:(nb + 1) * 512],
                    start=(kb == 0),
                    stop=(kb == 3),
                )
            if copy_engine == "pool":
                nc.gpsimd.tensor_copy(dst[:, m, nb * 512:(nb + 1) * 512], pq[:])
            else:
                nc.vector.tensor_copy(dst[:, m, nb * 512:(nb + 1) * 512], pq[:])

        # q/k m0 nb0 with split copies so the first dots' last dependency
        # lands as early as possible
        pqq = tp.tile([P, 512], F32, tag="tp", name="pq_0_0_0")
        for kb in range(4):
            nc.tensor.matmul(
                pqq[:],
                w3_sb[:, kb, 0:128],
                xT_sb[:, kb, 0:512],
                start=(kb == 0),
                stop=(kb == 3),
            )
        nc.vector.tensor_copy(qT_sb[:, 0, 0:256], pqq[:, 0:256])
        nc.vector.tensor_copy(qT_sb[:, 0, 256:512], pqq[:, 256:512])
        pqk = tq.tile([P, 512], F32, tag="tq", name="pq_1_0_0")
        for kb in range(4):
            nc.tensor.matmul(
                pqk[:],
                w3_sb[:, kb, 128:256],
                xT_sb[:, kb, 0:512],
                start=(kb == 0),
                stop=(kb == 3),
            )
        nc.vector.tensor_copy(kT_sb[:, 0, 0:256], pqk[:, 0:256])
        nc.vector.tensor_copy(kT_sb[:, 0, 256:512], pqk[:, 256:512])

        def v_proj(jb):
            pv = scratch(jb).tile([P, 512], F32, tag=scratch(jb).name,
                                  name=f"pv_{jb}")
            for kb in range(4):
                nc.tensor.matmul(
                    pv[:, :256],
                    xT_sb[:, kb, jb * 128:(jb + 1) * 128],
                    w3_sb[:, kb, 256:512],
                    start=(kb == 0),
                    stop=(kb == 3),
                )
            nc.vector.tensor_copy(
                v_cols[:, jb, :, :64],
                pv[:, :256].rearrange("p (h c) -> p h c", c=64),
            )

        # ---- attention ------------------------------------------------------
        sc2 = {}

        deferred = []

        def proj(sc, io, dst_io, ycopy_engine, pool):
            py = pool.tile([P, 512], F32, tag=pool.name, name=f"py_{dst_io}")
            for u in range(2):
                nc.tensor.matmul(
                    py[:],
                    sc[:, u, io * 128:(io + 1) * 128],
                    wo2_sb[:, u, :],
                    start=(u == 0),
                    stop=(u == 1),
                )
            if ycopy_engine == "act":
                nc.scalar.copy(y_all[:, dst_io, :], py[:])
            elif ycopy_engine == "dve":
                nc.vector.tensor_copy(y_all[:, dst_io, :], py[:])
            return py

        prev = None
        for hp in range(2):
            for ib in range(2):
                if ib == 0 and hp == 0:
                    st = st00
                else:
                    st = spd_pool.tile([P, 8, 2, 512], BF16, tag="spd",
                                       name=f"spd_{hp}_{ib}")
                    nc.sync.dma_start(st[:, 0:4], spdT[hp, ib, :, 0:4])
                    nc.sync.dma_start(st[:, 4:8], spdT[hp, ib, :, 4:8])
                if ib not in sc2:
                    sc2[ib] = sc2_pool.tile([P, 2, 512], BF16, tag="sc2",
                                            name=f"sc2_{ib}")

                # ---- prev phase normalization FIRST (frees its po2 slots
                # before this phase's attnv reuses the 2-slot ring) ----
                if prev is not None:
                    p_po2, p_hp, p_ib = prev
                    p_sc = sc2[p_ib]
                    rc = rc_pool.tile([P, 2, 4], F32, tag="rc",
                                      name=f"rc_{p_hp}_{p_ib}")
                    att_n = attn_pool.tile([P, 2, 256], BF16, tag="attn",
                                           name=f"attn_{p_hp}_{p_ib}")
                    for s in range(2):
                        nc.vector.reciprocal(
                            rc[:, s, :],
                            p_po2[s][:].rearrange("p (ic c) -> p ic c", c=128)[:, :, 64],
                        )
                    # normalization on DVE (GPSIMD cannot access PSUM on HW);
                    # s0 now, s1 deferred into the jb0 body (its attnv batch
                    # is a slot behind, so the burst can be spread)
                    for ic in range(4):
                        nc.vector.tensor_scalar_mul(
                            att_n[:, 0, ic * 64:(ic + 1) * 64],
                            p_po2[0][:, ic * 128:ic * 128 + 64],
                            rc[:, 0, ic:ic + 1],
                        )

                po2 = [npo.tile([P, 512], F32, tag="po", name=f"po2_{hp}_{ib}_{s}")
                       for s in range(2)]
                prs = {}

                def dots(jb):
                    pd = wide.tile([P, 1024], F32, tag="big",
                                   name=f"pd_{hp}_{ib}_{jb}")
                    for s in range(2):
                        nc.tensor.matmul(
                            pd[:, s * 512:(s + 1) * 512],
                            kT_sb[64 * s:64 * s + 64, hp, jb * 128:(jb + 1) * 128],
                            qT_sb[64 * s:64 * s + 64, hp, ib * 512:(ib + 1) * 512],
                            start=True,
                            stop=True,
                        )
                    return pd

                if hp == 0 and ib == 0:
                    # first dots split into i-halves: each sub-matmul starts
                    # as soon as its half of the q copy lands
                    pd = wide.tile([P, 1024], F32, tag="big", name="pd_0_0_0")
                    for s in range(2):
                        for ihalf in range(2):
                            nc.tensor.matmul(
                                pd[:, s * 512 + ihalf * 256:s * 512 + (ihalf + 1) * 256],
                                kT_sb[64 * s:64 * s + 64, 0, 0:128],
                                qT_sb[64 * s:64 * s + 64, 0, ihalf * 256:(ihalf + 1) * 256],
                                start=True,
                                stop=True,
                            )
                else:
                    pd = dots(0)
                for jb in range(8):
                    ex = ex_pool.tile([P, 1024], BF16, tag="ex",
                                      name=f"ex_{hp}_{ib}_{jb}")
                    pr = pr_pool.tile([P, 1024], BF16, tag="pr",
                                      name=f"pr_{hp}_{ib}_{jb}")
                    nc.scalar.activation(ex[:], pd[:], EXP)
                    nc.vector.tensor_tensor(
                        pr[:], ex[:],
                        st[:, jb].rearrange("p s i -> p (s i)"),
                        MULT,
                    )
                    if jb == 1 and prev is not None:
                        for ic in range(4):
                            nc.vector.tensor_scalar_mul(
                                att_n[:, 1, ic * 64:(ic + 1) * 64],
                                prev[0][1][:, ic * 128:ic * 128 + 64],
                                rc[:, 1, ic:ic + 1],
                            )
                    # software pipelining: next dots queued on PE BEFORE this
                    # unit's attnv (which waits on ACT+DVE)
                    if jb < 7:
                        pd = dots(jb + 1)
                    # v projection feeds attnv of phase (0,0) just in time
                    if ib == 0 and hp == 0:
                        v_proj(jb)
                    def attnv(jbx, s):
                        h = 2 * hp + s
                        for ic in range(4):
                            nc.tensor.matmul(
                                po2[s][:, ic * 128:ic * 128 + 65],
                                prs[jbx][:, s * 512 + ic * 128:s * 512 + (ic + 1) * 128],
                                v_aug[:, jbx, h * 65:(h + 1) * 65],
                                start=(jbx == 0 and ic == 0),
                                stop=(jbx == 7),
                                skip_group_check=(ic > 0),
                            )

                    prs[jb] = pr
                    attnv(jb, 0)
                    # s1 attnv one slot behind: if its pr isn't ready yet it
                    # parks in the wait queue WITHOUT blocking next-jb dots
                    if jb > 0:
                        attnv(jb - 1, 1)
                    if jb == 7:
                        attnv(7, 1)

                    # ---- interleaved work in this phase's PE slack ----
                    # phase order (hp,ib): (0,0) (0,1) (1,0) (1,1); remaining
                    # q/k chunks staged 1+ phase before their first use
                    if hp == 0 and ib == 0:
                        if jb == 1:
                            qk_chunk(1, 0, 1, "dve", tq)   # k m0 nb1 (dots jb4+)
                        if jb == 2:
                            qk_chunk(0, 0, 1, "dve", tp)   # q m0 nb1 (phase (0,1))
                    if hp == 0 and ib == 1:
                        if jb == 1:
                            qk_chunk(1, 1, 0, "dve", tq)   # k m1 nb0 (phase (1,0))
                        if jb == 3:
                            qk_chunk(0, 1, 0, "dve", tp)   # q m1 nb0 (phase (1,0))
                    if hp == 1 and ib == 0:
                        if jb == 0:
                            qk_chunk(1, 1, 1, "dve", tq)   # k m1 nb1 (dots jb4+)
                        if jb == 2:
                            qk_chunk(0, 1, 1, "dve", tp)   # q m1 nb1 (phase (1,1))

                    if prev is not None:
                        p_po2, p_hp, p_ib = prev
                        p_sc = sc2[p_ib]
                        # deferred io0/io1 ycopies EARLY in this jb slot (and
                        # before the T block, whose tiles reuse the py slots)

                        if jb in (3, 4, 5, 6):
                            ic = jb - 3
                            pool = scratch(ic)
                            tt = pool.tile([P, P], BF16, tag=pool.name,
                                           name=f"tt_{p_hp}_{p_ib}_{ic}")
                            for s in range(2):
                                nc.tensor.transpose(
                                    tt[64 * s:64 * s + 64, :],
                                    att_n[:, s, ic * 64:(ic + 1) * 64],
                                    ident_sb[:],
                                )
                            nc.vector.tensor_copy(
                                p_sc[:, p_hp, ic * 128:(ic + 1) * 128], tt[:])
                        if p_hp == 1:
                            if jb in (4, 5, 6, 7):
                                io = jb - 4
                                pyt = proj(p_sc, io, p_ib * 4 + io, "defer",
                                           scratch(io + 1))
                                deferred.append((pyt, p_ib * 4 + io))

                prev = (po2, hp, ib)

        # ---- flush: last phase's epilogue + proj(ib=1), engine-parallel -----
        # deferred io2/io3 y-copies of ib0 first: ACT is free once the exp
        # stream ends, and this keeps them off phase (1,1)'s busy DVE
        for pyt, dst_io in deferred[2:]:
            nc.scalar.copy(y_all[:, dst_io, :], pyt[:])
            nc.sync.dma_start(y[dst_io * 128:(dst_io + 1) * 128, :],
                              y_all[:, dst_io, :])
        p_po2, p_hp, p_ib = prev
        p_sc = sc2[p_ib]
        rc = rc_pool.tile([P, 2, 4], F32, tag="rc", name="rc_flush")
        att_n = attn_pool.tile([P, 2, 256], BF16, tag="attn", name="attn_flush")
        for s in range(2):
            nc.vector.reciprocal(
                rc[:, s, :],
                p_po2[s][:].rearrange("p (ic c) -> p ic c", c=128)[:, :, 64],
            )

        for ic in range(4):
            # normalization: whole ic on one engine, alternating, so each
            # transpose pair is gated by a single fast engine
            for s in range(2):
                if ic >= 2:
                    nc.scalar.activation(
                        att_n[:, s, ic * 64:(ic + 1) * 64],
                        p_po2[s][:, ic * 128:ic * 128 + 64],
                        mybir.ActivationFunctionType.Copy,
                        scale=rc[:, s, ic:ic + 1],
                    )
                else:
                    nc.vector.tensor_scalar_mul(
                        att_n[:, s, ic * 64:(ic + 1) * 64],
                        p_po2[s][:, ic * 128:ic * 128 + 64],
                        rc[:, s, ic:ic + 1],
                    )
        tts = []
        for ic in range(4):
            tt = wide.tile([P, P], BF16, tag="big", name=f"tt_flush_{ic}")
            for s in range(2):
                nc.tensor.transpose(
                    tt[64 * s:64 * s + 64, :],
                    att_n[:, s, ic * 64:(ic + 1) * 64],
                    ident_sb[:],
                )
            tts.append(tt)
            # wide ring is 2-deep: copy must follow within the pair
            if ic % 2 == 1:
                for icc in (ic - 1, ic):
                    nc.vector.tensor_copy(
                        p_sc[:, p_hp, icc * 128:(icc + 1) * 128], tts[icc][:])
        # ib0 io0/io1 y-copies on ACT: its flush idle window absorbs them,
        # keeping DVE clear for the last TT multiply and the sc2 drains
        for dio in (0, 1):
            nc.scalar.copy(y_all[:, dio, :], deferred[dio][0][:])
        nc.sync.dma_start(
            y[0:256, :].rearrange("(io p) q -> p io q", p=P), y_all[:, 0:2, :])
        for ic in range(4):
            py = scratch(ic).tile([P, 512], F32, tag=scratch(ic).name,
                                  name=f"py_flush_{ic}")
            for u in range(2):
                nc.tensor.matmul(
                    py[:],
                    p_sc[:, u, ic * 128:(ic + 1) * 128],
                    wo2_sb[:, u, :],
                    start=(u == 0),
                    stop=(u == 1),
                )
            if ic % 2 == 1:
                nc.scalar.copy(y_all[:, p_ib * 4 + ic, :], py[:])
            else:
                nc.vector.tensor_copy(y_all[:, p_ib * 4 + ic, :], py[:])
            if ic == 1:
                # pair the first two chunks: one less HWDGE slot ahead of the
                # critical last-chunk dma
                nc.sync.dma_start(
                    y[p_ib * 512:p_ib * 512 + 256, :]
                    .rearrange("(io p) q -> p io q", p=P),
                    y_all[:, p_ib * 4:p_ib * 4 + 2, :])
            elif ic > 1:
                nc.sync.dma_start(
                    y[p_ib * 512 + ic * 128:p_ib * 512 + (ic + 1) * 128, :],
                    y_all[:, p_ib * 4 + ic, :])

    nc.compile()
    return nc


def _get_nc():
    if "v2" not in _NC:
        _NC["v2"] = build_nc()
    return _NC["v2"]


def make_in_maps(x, spd, head_keep, w_qkv, w_out):
    x = np.asarray(x, np.float32)
    spd = np.asarray(spd, np.float32)
    keep = np.asarray(head_keep, np.float32)
    w_qkv = np.asarray(w_qkv, np.float32)
    w_out = np.asarray(w_out, np.float32)
    cfac = keep * (HEADS / keep.sum())
    ident = np.eye(P, dtype=ml_dtypes.bfloat16)

    in_maps = []
    for c in range(8):
        bi, hh = divmod(c, 2)
        h0 = hh * HL
        hs = slice(h0 * DIM_HEAD, (h0 + HL) * DIM_HEAD)
        xT = np.ascontiguousarray(x[bi].T.astype(ml_dtypes.bfloat16))
        q_cols = w_qkv[:, hs] * np.float32(SCALE)
        k_cols = w_qkv[:, DIM + h0 * DIM_HEAD:DIM + (h0 + HL) * DIM_HEAD]
        v_cols_h = w_qkv[:, 2 * DIM + h0 * DIM_HEAD:2 * DIM + (h0 + HL) * DIM_HEAD]
        w3 = np.ascontiguousarray(np.concatenate(
            [q_cols[:, :128], k_cols[:, :128], v_cols_h,
             q_cols[:, 128:], k_cols[:, 128:]],
            axis=1,
        ).astype(ml_dtypes.bfloat16))
        # wo2[(s,d), hp, :] = w_out row of head (h0+2hp+s), dim d, * cfac
        wo_rows = w_out[hs, :] * np.repeat(cfac[h0:h0 + HL], DIM_HEAD)[:, None]
        wo4 = wo_rows.reshape(2, 2, DIM_HEAD, DIM)      # [hp, s, d, dim]
        wo2 = np.ascontiguousarray(
            wo4.transpose(1, 2, 0, 3).reshape(2 * DIM_HEAD, 2, DIM)
            .astype(ml_dtypes.bfloat16))
        sp = spd[bi, h0:h0 + HL]  # [HL, i, j] with h = 2*hp + s
        # [hp, s, ib, ii, jb, jj] -> [hp, ib, jj, jb, s, ii]
        spdT = sp.reshape(2, 2, 2, 512, 8, 128).transpose(0, 2, 5, 4, 1, 3)
        spdT = np.exp(spdT).astype(ml_dtypes.bfloat16)
        in_maps.append({"xT": xT, "w3": w3, "wo2": wo2, "ident": ident,
                        "spdT": np.ascontiguousarray(spdT)})
    return in_maps


def kernel(x, spd, head_keep, w_qkv, w_out, b_out):
    assert x.shape == (B, N, DIM) and spd.shape == (B, HEADS, N, N)
    nc = _get_nc()
    in_maps = make_in_maps(x, spd, head_keep, w_qkv, w_out)
    res = run_bass_kernel_spmd(nc, in_maps, core_ids=list(range(8)))
    out = np.empty((B, N, DIM), np.float32)
    for bi in range(B):
        out[bi] = (res.results[2 * bi]["y"].astype(np.float32)
                   + res.results[2 * bi + 1]["y"].astype(np.float32))
    out += np.asarray(b_out, np.float32)[None, None, :]
    return out


# revision 10
# speedup vs baseline: 1.0482x; 1.0067x over previous
"""Trainium2 Bass kernel v2 for nn_Attention_spd.

Reference computation (b=4, n=1024, dim=512, heads=8, dim_head=64):
    qkv = x @ w_qkv ; q,k,v = split
    dots = q @ k^T * scale + spd
    attn = softmax(dots) * (head_keep * H / sum(head_keep))
    out  = (attn @ v) @ w_out + b_out

Sharding: core c handles batch c//2, local heads 4*(c%2)..+3 (DP x TP).
Host sums the two bf16 partial outputs per batch and adds b_out.

Design notes:
  - All DRAM traffic bf16 (x, w_qkv, w_out, exp(spd), y partials): ~10.9MB/core.
  - attn@v computed TRANSPOSED with v as the *moving* operand:
    out[i, d] tiles of [128 i, 65] cost only 65 PE columns each (v augmented
    with a ones column so col 64 accumulates the softmax denominator).
    The 8 accumulation groups (2 heads x 4 i-blocks) share two PSUM banks via
    the lazy bank-zero semantics: only the first group issues start=True
    (wiping the whole bank); the other 3 start with start=False and are
    zero-seeded by the pending-zero region.
  - Softmax normalization is a per-partition scalar op (reciprocal of col 64
    + tensor_scalar_mul -> bf16); no broadcast matmul.
  - Normalized [128 i, 64 d] tiles are PE-transposed (identity matmul) into
    [(s,d), i] layout packing the head pair on 128 partitions, so the output
    projection runs with K=128 (half the matmuls of the K=64 version).
  - ACT (exp) is the pacing engine: 32 x [128,1024] exp ops. The jb loop is
    software-pipelined: dots(jb+1) is emitted BEFORE attnv(jb) so the
    in-order PE queue never lockstep-stalls the ACT stream.
  - PSUM: wide pd ring (2x2 banks) + po2 ring (2x1) + two 1-bank scratch
    rings (tp/tq) for v/qk-chunk/transpose/proj tiles, used alternately.
  - q/k m0 computed for i-cols 0:512 first (minimal head before the first
    dots); remaining q/k chunks + v projection interleave into phase slack.
"""
import os
import sys

for _p in ("/opt/trn_rl_repo", os.path.expanduser("~/.axon_site/_ro/trn_rl_repo")):
    if os.path.isdir(_p) and _p not in sys.path:
        sys.path.insert(0, _p)

import numpy as np
import ml_dtypes

import concourse.bass as bass  # noqa: F401
import concourse.tile as tile
from concourse import bacc, mybir
from concourse.bass_utils import run_bass_kernel_spmd

P = 128
B, N, DIM = 4, 1024, 512
HEADS = 8
DIM_HEAD = 64
SCALE = DIM_HEAD ** -0.5
HL = 4          # heads per core (local)
F32 = mybir.dt.float32
BF16 = mybir.dt.bfloat16
MULT = mybir.AluOpType.mult
EXP = mybir.ActivationFunctionType.Exp

_NC = {}


def build_nc():
    nc = bacc.Bacc("TRN2", target_bir_lowering=False, debug=False, num_devices=8)
    xT = nc.dram_tensor("xT", [DIM, N], BF16, kind="ExternalInput").ap()
    # [qm0 | km0 | v | qm1 | km1] column blocks (q pre-scaled by SCALE)
    w3 = nc.dram_tensor("w3", [DIM, 3 * HL * DIM_HEAD], BF16, kind="ExternalInput").ap()
    # packed for K=128 proj: [(s,d), hp, dim]
    wo2 = nc.dram_tensor("wo2", [P, 2, DIM], BF16, kind="ExternalInput").ap()
    ident = nc.dram_tensor("ident", [P, P], BF16, kind="ExternalInput").ap()
    # exp(spd) bf16: [hp, ib, j, jb, s, i]
    spdT = nc.dram_tensor("spdT", [2, 2, P, 8, 2, 512], BF16, kind="ExternalInput").ap()
    y = nc.dram_tensor("y", [N, DIM], BF16, kind="ExternalOutput").ap()

    from contextlib import ExitStack

    with tile.TileContext(nc) as tc, ExitStack() as ctx:
        sb = ctx.enter_context(tc.tile_pool(name="sb", bufs=1))
        spd_pool = ctx.enter_context(tc.tile_pool(name="spd", bufs=4))
        ex_pool = ctx.enter_context(tc.tile_pool(name="ex", bufs=5))
        pr_pool = ctx.enter_context(tc.tile_pool(name="pr", bufs=5))
        attn_pool = ctx.enter_context(tc.tile_pool(name="attn", bufs=2))
        rc_pool = ctx.enter_context(tc.tile_pool(name="rc", bufs=2))
        sc2_pool = ctx.enter_context(tc.tile_pool(name="sc2", bufs=2))
        # PSUM: 4 (wide pd ring) + 2 (po2) + 1 (tp) + 1 (tq) = 8 banks
        wide = ctx.enter_context(tc.tile_pool(name="wide", bufs=2, space="PSUM"))
        npo = ctx.enter_context(tc.tile_pool(name="npo", bufs=2, space="PSUM"))
        tp = ctx.enter_context(tc.tile_pool(name="tp", bufs=1, space="PSUM"))
        tq = ctx.enter_context(tc.tile_pool(name="tq", bufs=1, space="PSUM"))

        def scratch(i):
            return tp if i % 2 == 0 else tq

        # ---- consts + warm-up ----------------------------------------------
        seed = sb.tile([P, 512], BF16, tag="seed")
        nc.gpsimd.memset(seed[:], 1.0)
        # dummy exp: forces the ACT table load to run at program start
        # instead of blocking the first real exp. Output goes to a separate
        # scratch so the warm matmuls' read of seed isn't serialized behind it
        dum = sb.tile([1, 1], BF16, tag="dum")
        nc.scalar.activation(dum[:], seed[0:1, 0:1], EXP)
        # PE p-state ramp: busy early so real matmuls hit full speed; the
        # warm matmuls also bridge the initial DMA wait
        warm = tp.tile([P, 512], F32, tag="tp", name="warm")
        for _ in range(5):
            nc.tensor.matmul(warm[:, :512], seed[0:1, 0:128], seed[0:1, 0:512],
                             start=True, stop=True)

        # ---- resident loads -------------------------------------------------
        xT_sb = sb.tile([P, 4, N], BF16)
        w3_sb = sb.tile([P, 4, 768], BF16, tag="w3")
        wo2_sb = sb.tile([P, 2, DIM], BF16, tag="wo2")
        ident_sb = sb.tile([P, P], BF16, tag="ident")
        xT_r = xT.rearrange("(kb p) n -> p kb n", p=P)
        w3_r = w3.rearrange("(kb p) m -> p kb m", p=P)
        # ordered so the first q/k projections + first spd tile land earliest
        nc.sync.dma_start(w3_sb[:, :, 0:256], w3_r[:, :, 0:256])      # q/k m0
        nc.sync.dma_start(xT_sb[:, 0:2, 0:512], xT_r[:, 0:2, 0:512])
        nc.sync.dma_start(xT_sb[:, 2:4, 0:512], xT_r[:, 2:4, 0:512])
        nc.sync.dma_start(w3_sb[:, :, 256:512], w3_r[:, :, 256:512])  # v
        st00 = spd_pool.tile([P, 8, 2, 512], BF16, tag="spd", name="spd_0_0")
        nc.sync.dma_start(st00[:, 0:4], spdT[0, 0, :, 0:4])
        nc.sync.dma_start(xT_sb[:, 0:2, 512:1024], xT_r[:, 0:2, 512:1024])
        nc.sync.dma_start(xT_sb[:, 2:4, 512:1024], xT_r[:, 2:4, 512:1024])
        nc.sync.dma_start(st00[:, 4:8], spdT[0, 0, :, 4:8])
        nc.sync.dma_start(w3_sb[:, :, 512:768], w3_r[:, :, 512:768])  # q/k m1
        nc.sync.dma_start(wo2_sb[:], wo2[:])
        nc.sync.dma_start(ident_sb[:], ident[:])

        qT_sb = sb.tile([P, 2, N], BF16, tag="qT")
        kT_sb = sb.tile([P, 2, N], BF16, tag="kT")
        v_aug = sb.tile([P, 8, HL * 65], BF16, tag="vaug")
        v_cols = v_aug[:].rearrange("p jb (h c) -> p jb h c", c=65)
        nc.vector.memset(v_cols[:, :, :, 64:65], 1.0)
        y_all = sb.tile([P, 8, DIM], BF16, tag="yall")

        # ---- minimal head: q/k m0 for i-cols 0:512 only ---------------------
        # qk: 0=q, 1=k; m: 0=heads 0/1, 1=heads 2/3; nb: i-col half
        def qk_chunk(qk, m, nb, copy_engine, pool):
            wofs = (512 if m else 0) + qk * 128
            dst = qT_sb if qk == 0 else kT_sb
            pq = pool.tile([P, 512], F32, tag=pool.name,
                           name=f"pq_{qk}_{m}_{nb}")
            for kb in range(4):
                nc.tensor.matmul(
                    pq[:],
                    w3_sb[:, kb, wofs:wofs + 128],
                    xT_sb[:, kb, nb * 512:(nb + 1) * 512],
                    start=(kb == 0),
                    stop=(kb == 3),
                )
            if copy_engine == "pool":
                nc.gpsimd.tensor_copy(dst[:, m, nb * 512:(nb + 1) * 512], pq[:])
            else:
                nc.vector.tensor_copy(dst[:, m, nb * 512:(nb + 1) * 512], pq[:])

        # q/k m0 nb0 with split copies so the first dots' last dependency
        # lands as early as possible
        pqq = tp.tile([P, 512], F32, tag="tp", name="pq_0_0_0")
        for kb in range(4):
            nc.tensor.matmul(
                pqq[:],
                w3_sb[:, kb, 0:128],
                xT_sb[:, kb, 0:512],
                start=(kb == 0),
                stop=(kb == 3),
            )
        nc.vector.tensor_copy(qT_sb[:, 0, 0:256], pqq[:, 0:256])
        nc.vector.tensor_copy(qT_sb[:, 0, 256:512], pqq[:, 256:512])
        pqk = tq.tile([P, 512], F32, tag="tq", name="pq_1_0_0")
        for kb in range(4):
            nc.tensor.matmul(
                pqk[:],
                w3_sb[:, kb, 128:256],
                xT_sb[:, kb, 0:512],
                start=(kb == 0),
                stop=(kb == 3),
            )
        nc.vector.tensor_copy(kT_sb[:, 0, 0:256], pqk[:, 0:256])
        nc.vector.tensor_copy(kT_sb[:, 0, 256:512], pqk[:, 256:512])

        def v_proj(jb):
            pv = scratch(jb).tile([P, 512], F32, tag=scratch(jb).name,
                                  name=f"pv_{jb}")
            for kb in range(4):
                nc.tensor.matmul(
                    pv[:, :256],
                    xT_sb[:, kb, jb * 128:(jb + 1) * 128],
                    w3_sb[:, kb, 256:512],
                    start=(kb == 0),
                    stop=(kb == 3),
                )
            nc.vector.tensor_copy(
                v_cols[:, jb, :, :64],
                pv[:, :256].rearrange("p (h c) -> p h c", c=64),
            )

        # ---- attention ------------------------------------------------------
        sc2 = {}

        deferred = []

        def proj(sc, io, dst_io, ycopy_engine, pool):
            py = pool.tile([P, 512], F32, tag=pool.name, name=f"py_{dst_io}")
            for u in range(2):
                nc.tensor.matmul(
                    py[:],
                    sc[:, u, io * 128:(io + 1) * 128],
                    wo2_sb[:, u, :],
                    start=(u == 0),
                    stop=(u == 1),
                )
            if ycopy_engine == "act":
                nc.scalar.copy(y_all[:, dst_io, :], py[:])
            elif ycopy_engine == "dve":
                nc.vector.tensor_copy(y_all[:, dst_io, :], py[:])
            return py

        prev = None
        for hp in range(2):
            for ib in range(2):
                if ib == 0 and hp == 0:
                    st = st00
                else:
                    st = spd_pool.tile([P, 8, 2, 512], BF16, tag="spd",
                                       name=f"spd_{hp}_{ib}")
                    nc.sync.dma_start(st[:, 0:4], spdT[hp, ib, :, 0:4])
                    nc.sync.dma_start(st[:, 4:8], spdT[hp, ib, :, 4:8])
                if ib not in sc2:
                    sc2[ib] = sc2_pool.tile([P, 2, 512], BF16, tag="sc2",
                                            name=f"sc2_{ib}")

                # ---- prev phase normalization FIRST (frees its po2 slots
                # before this phase's attnv reuses the 2-slot ring) ----
                if prev is not None:
                    p_po2, p_hp, p_ib = prev
                    p_sc = sc2[p_ib]
                    rc = rc_pool.tile([P, 2, 4], F32, tag="rc",
                                      name=f"rc_{p_hp}_{p_ib}")
                    att_n = attn_pool.tile([P, 2, 256], BF16, tag="attn",
                                           name=f"attn_{p_hp}_{p_ib}")
                    for s in range(2):
                        nc.vector.reciprocal(
                            rc[:, s, :],
                            p_po2[s][:].rearrange("p (ic c) -> p ic c", c=128)[:, :, 64],
                        )
                    # normalization on DVE (GPSIMD cannot access PSUM on HW);
                    # s0 now, s1 deferred into the jb0 body (its attnv batch
                    # is a slot behind, so the burst can be spread)
                    for ic in range(4):
                        nc.vector.tensor_scalar_mul(
                            att_n[:, 0, ic * 64:(ic + 1) * 64],
                            p_po2[0][:, ic * 128:ic * 128 + 64],
                            rc[:, 0, ic:ic + 1],
                        )

                po2 = [npo.tile([P, 512], F32, tag="po", name=f"po2_{hp}_{ib}_{s}")
                       for s in range(2)]
                prs = {}

                def dots(jb):
                    pd = wide.tile([P, 1024], F32, tag="big",
                                   name=f"pd_{hp}_{ib}_{jb}")
                    for s in range(2):
                        nc.tensor.matmul(
                            pd[:, s * 512:(s + 1) * 512],
                            kT_sb[64 * s:64 * s + 64, hp, jb * 128:(jb + 1) * 128],
                            qT_sb[64 * s:64 * s + 64, hp, ib * 512:(ib + 1) * 512],
                            start=True,
                            stop=True,
                        )
                    return pd

                if hp == 0 and ib == 0:
                    # first dots split into i-halves: each sub-matmul starts
                    # as soon as its half of the q copy lands
                    pd = wide.tile([P, 1024], F32, tag="big", name="pd_0_0_0")
                    for s in range(2):
                        for ihalf in range(2):
                            nc.tensor.matmul(
                                pd[:, s * 512 + ihalf * 256:s * 512 + (ihalf + 1) * 256],
                                kT_sb[64 * s:64 * s + 64, 0, 0:128],
                                qT_sb[64 * s:64 * s + 64, 0, ihalf * 256:(ihalf + 1) * 256],
                                start=True,
                                stop=True,
                            )
                else:
                    pd = dots(0)
                for jb in range(8):
                    ex = ex_pool.tile([P, 1024], BF16, tag="ex",
                                      name=f"ex_{hp}_{ib}_{jb}")
                    pr = pr_pool.tile([P, 1024], BF16, tag="pr",
                                      name=f"pr_{hp}_{ib}_{jb}")
                    nc.scalar.activation(ex[:], pd[:], EXP)
                    nc.vector.tensor_tensor(
                        pr[:], ex[:],
                        st[:, jb].rearrange("p s i -> p (s i)"),
                        MULT,
                    )
                    if jb == 1 and prev is not None:
                        for ic in range(4):
                            nc.vector.tensor_scalar_mul(
                                att_n[:, 1, ic * 64:(ic + 1) * 64],
                                prev[0][1][:, ic * 128:ic * 128 + 64],
                                rc[:, 1, ic:ic + 1],
                            )
                    # software pipelining: next dots queued on PE BEFORE this
                    # unit's attnv (which waits on ACT+DVE)
                    if jb < 7:
                        pd = dots(jb + 1)
                    # v projection feeds attnv of phase (0,0) just in time
                    if ib == 0 and hp == 0:
                        v_proj(jb)
                    def attnv(jbx, s):
                        h = 2 * hp + s
                        for ic in range(4):
                            nc.tensor.matmul(
                                po2[s][:, ic * 128:ic * 128 + 65],
                                prs[jbx][:, s * 512 + ic * 128:s * 512 + (ic + 1) * 128],
                                v_aug[:, jbx, h * 65:(h + 1) * 65],
                                start=(jbx == 0 and ic == 0),
                                stop=(jbx == 7),
                                skip_group_check=(ic > 0),
                            )

                    prs[jb] = pr
                    attnv(jb, 0)
                    # s1 attnv one slot behind: if its pr isn't ready yet it
                    # parks in the wait queue WITHOUT blocking next-jb dots
                    if jb > 0:
                        attnv(jb - 1, 1)
                    if jb == 7:
                        attnv(7, 1)

                    # ---- interleaved work in this phase's PE slack ----
                    # phase order (hp,ib): (0,0) (0,1) (1,0) (1,1); remaining
                    # q/k chunks staged 1+ phase before their first use
                    if hp == 0 and ib == 0:
                        if jb == 1:
                            qk_chunk(1, 0, 1, "dve", tq)   # k m0 nb1 (dots jb4+)
                        if jb == 2:
                            qk_chunk(0, 0, 1, "dve", tp)   # q m0 nb1 (phase (0,1))
                    if hp == 0 and ib == 1:
                        if jb == 1:
                            qk_chunk(1, 1, 0, "dve", tq)   # k m1 nb0 (phase (1,0))
                        if jb == 3:
                            qk_chunk(0, 1, 0, "dve", tp)   # q m1 nb0 (phase (1,0))
                    if hp == 1 and ib == 0:
                        if jb == 0:
                            qk_chunk(1, 1, 1, "dve", tq)   # k m1 nb1 (dots jb4+)
                        if jb == 2:
                            qk_chunk(0, 1, 1, "dve", tp)   # q m1 nb1 (phase (1,1))

                    if prev is not None:
                        p_po2, p_hp, p_ib = prev
                        p_sc = sc2[p_ib]
                        # deferred io0/io1 ycopies EARLY in this jb slot (and
                        # before the T block, whose tiles reuse the py slots)

                        if jb in (3, 4, 5, 6):
                            ic = jb - 3
                            pool = scratch(ic)
                            tt = pool.tile([P, P], BF16, tag=pool.name,
                                           name=f"tt_{p_hp}_{p_ib}_{ic}")
                            for s in range(2):
                                nc.tensor.transpose(
                                    tt[64 * s:64 * s + 64, :],
                                    att_n[:, s, ic * 64:(ic + 1) * 64],
                                    ident_sb[:],
                                )
                            nc.vector.tensor_copy(
                                p_sc[:, p_hp, ic * 128:(ic + 1) * 128], tt[:])
                        if p_hp == 1:
                            if jb in (4, 5, 6, 7):
                                io = jb - 4
                                pyt = proj(p_sc, io, p_ib * 4 + io, "defer",
                                           scratch(io + 1))
                                deferred.append((pyt, p_ib * 4 + io))

                prev = (po2, hp, ib)

        # ---- flush: last phase's epilogue + proj(ib=1), engine-parallel -----
        # deferred io2/io3 y-copies of ib0 first: ACT is free once the exp
        # stream ends, and this keeps them off phase (1,1)'s busy DVE
        for pyt, dst_io in deferred[2:]:
            nc.scalar.copy(y_all[:, dst_io, :], pyt[:])
            nc.sync.dma_start(y[dst_io * 128:(dst_io + 1) * 128, :],
                              y_all[:, dst_io, :])
        p_po2, p_hp, p_ib = prev
        p_sc = sc2[p_ib]
        rc = rc_pool.tile([P, 2, 4], F32, tag="rc", name="rc_flush")
        att_n = attn_pool.tile([P, 2, 256], BF16, tag="attn", name="attn_flush")
        for s in range(2):
            nc.vector.reciprocal(
                rc[:, s, :],
                p_po2[s][:].rearrange("p (ic c) -> p ic c", c=128)[:, :, 64],
            )

        for ic in range(4):
            # normalization: whole ic on one engine, alternating, so each
            # transpose pair is gated by a single fast engine
            for s in range(2):
                if ic >= 2:
                    nc.scalar.activation(
                        att_n[:, s, ic * 64:(ic + 1) * 64],
                        p_po2[s][:, ic * 128:ic * 128 + 64],
                        mybir.ActivationFunctionType.Copy,
                        scale=rc[:, s, ic:ic + 1],
                    )
                else:
                    nc.vector.tensor_scalar_mul(
                        att_n[:, s, ic * 64:(ic + 1) * 64],
                        p_po2[s][:, ic * 128:ic * 128 + 64],
                        rc[:, s, ic:ic + 1],
                    )
        tts = []
        for ic in range(4):
            tt = wide.tile([P, P], BF16, tag="big", name=f"tt_flush_{ic}")
            for s in range(2):
                nc.tensor.transpose(
                    tt[64 * s:64 * s + 64, :],
                    att_n[:, s, ic * 64:(ic + 1) * 64],
                    ident_sb[:],
                )
            tts.append(tt)
            # wide ring is 2-deep: copy must follow within the pair
            if ic % 2 == 1:
                for icc in (ic - 1, ic):
                    nc.vector.tensor_copy(
                        p_sc[:, p_hp, icc * 128:(icc + 1) * 128], tts[icc][:])
        # ib0 io0/io1 y-copies on ACT: its flush idle window absorbs them,
        # keeping DVE clear for the last TT multiply and the sc2 drains
        for dio in (0, 1):
            nc.scalar.copy(y_all[:, dio, :], deferred[dio][0][:])
        nc.sync.dma_start(
            y[0:256, :].rearrange("(io p) q -> p io q", p=P), y_all[:, 0:2, :])
        for ic in range(4):
            py = scratch(ic).tile([P, 512], F32, tag=scratch(ic).name,
                                  name=f"py_flush_{ic}")
            for u in range(2):
                nc.tensor.matmul(
                    py[:],
                    p_sc[:, u, ic * 128:(ic + 1) * 128],
                    wo2_sb[:, u, :],
                    start=(u == 0),
                    stop=(u == 1),
                )
            if ic % 2 == 1:
                nc.scalar.copy(y_all[:, p_ib * 4 + ic, :], py[:])
            else:
                nc.vector.tensor_copy(y_all[:, p_ib * 4 + ic, :], py[:])
            if ic == 1:
                # pair the first two chunks: one less HWDGE slot ahead of the
                # critical last-chunk dma
                nc.sync.dma_start(
                    y[p_ib * 512:p_ib * 512 + 256, :]
                    .rearrange("(io p) q -> p io q", p=P),
                    y_all[:, p_ib * 4:p_ib * 4 + 2, :])
            elif ic > 1:
                nc.sync.dma_start(
                    y[p_ib * 512 + ic * 128:p_ib * 512 + (ic + 1) * 128, :],
                    y_all[:, p_ib * 4 + ic, :])

    nc.compile()
    return nc


def _get_nc():
    if "v2" not in _NC:
        _NC["v2"] = build_nc()
    return _NC["v2"]


def make_in_maps(x, spd, head_keep, w_qkv, w_out):
    x = np.asarray(x, np.float32)
    spd = np.asarray(spd, np.float32)
    keep = np.asarray(head_keep, np.float32)
    w_qkv = np.asarray(w_qkv, np.float32)
    w_out = np.asarray(w_out, np.float32)
    cfac = keep * (HEADS / keep.sum())
    ident = np.eye(P, dtype=ml_dtypes.bfloat16)

    in_maps = []
    for c in range(8):
        bi, hh = divmod(c, 2)
        h0 = hh * HL
        hs = slice(h0 * DIM_HEAD, (h0 + HL) * DIM_HEAD)
        xT = np.ascontiguousarray(x[bi].T.astype(ml_dtypes.bfloat16))
        q_cols = w_qkv[:, hs] * np.float32(SCALE)
        k_cols = w_qkv[:, DIM + h0 * DIM_HEAD:DIM + (h0 + HL) * DIM_HEAD]
        v_cols_h = w_qkv[:, 2 * DIM + h0 * DIM_HEAD:2 * DIM + (h0 + HL) * DIM_HEAD]
        w3 = np.ascontiguousarray(np.concatenate(
            [q_cols[:, :128], k_cols[:, :128], v_cols_h,
             q_cols[:, 128:], k_cols[:, 128:]],
            axis=1,
        ).astype(ml_dtypes.bfloat16))
        # wo2[(s,d), hp, :] = w_out row of head (h0+2hp+s), dim d, * cfac
        wo_rows = w_out[hs, :] * np.repeat(cfac[h0:h0 + HL], DIM_HEAD)[:, None]
        wo4 = wo_rows.reshape(2, 2, DIM_HEAD, DIM)      # [hp, s, d, dim]
        wo2 = np.ascontiguousarray(
            wo4.transpose(1, 2, 0, 3).reshape(2 * DIM_HEAD, 2, DIM)
            .astype(ml_dtypes.bfloat16))
        sp = spd[bi, h0:h0 + HL]  # [HL, i, j] with h = 2*hp + s
        # [hp, s, ib, ii, jb, jj] -> [hp, ib, jj, jb, s, ii]
        spdT = sp.reshape(2, 2, 2, 512, 8, 128).transpose(0, 2, 5, 4, 1, 3)
        spdT = np.exp(spdT).astype(ml_dtypes.bfloat16)
        in_maps.append({"xT": xT, "w3": w3, "wo2": wo2, "ident": ident,
                        "spdT": np.ascontiguousarray(spdT)})
    return in_maps


def kernel(x, spd, head_keep, w_qkv, w_out, b_out):
    assert x.shape == (B, N, DIM) and spd.shape == (B, HEADS, N, N)
    nc = _get_nc()
    in_maps = make_in_maps(x, spd, head_keep, w_qkv, w_out)
    res = run_bass_kernel_spmd(nc, in_maps, core_ids=list(range(8)))
    out = np.empty((B, N, DIM), np.float32)
    for bi in range(B):
        out[bi] = (res.results[2 * bi]["y"].astype(np.float32)
                   + res.results[2 * bi + 1]["y"].astype(np.float32))
    out += np.asarray(b_out, np.float32)[None, None, :]
    return out
